# revision 21
# baseline (speedup 1.0000x reference)
"""Trainium2 Bass kernel for nn_Attention_37074157699663.

Copula attention: MLP preprocess (phi/psi + ReLU), energy = comp_dec . comp_lis,
adjacent-column Pearson correlation over the batch, bivariate Gaussian copula
pdf, softmax, context. Data-parallel over batch across 8 NeuronCores; the three
per-column stat vectors (sum e, sum e^2, sum e[t]e[t+1]) are AllReduced.

The reference computes norm.pdf(e) (underflows fp32 to 0) times exp(copula
exponent) (overflows fp32 to inf), so reference rows are NaN wherever any
column overflows. We reproduce that exactly by computing the overflow mask
explicitly (expo > ln(float32 max)) instead of trusting device exp() inf
semantics, then injecting NaN per row.
"""

import os

import numpy as np
import ml_dtypes

import concourse.bacc as bacc
import concourse.bass as bass
import concourse.tile as tile
from concourse import mybir
from concourse.bass_utils import run_bass_kernel_spmd
from concourse.masks import make_identity
from bass_rust import AxisListType

BF16 = mybir.dt.bfloat16
F32 = mybir.dt.float32
AF = mybir.ActivationFunctionType
OP = mybir.AluOpType
ts = bass.ts

B, T, D, M = 64, 2048, 512, 512
NCORES = 8
BC = B // NCORES        # 8 batch rows per core
TC = 512                # t-chunk for MM1/MM2 (one PSUM bank)
NTC = T // TC           # 4
NDT = D // 128          # 4 contraction tiles
NMT = M // 128          # 4 m tiles
NCH = T // 128          # 16 chunks for the context matmul
TP = T // 128           # 16: free size of [128, TP] column-stat tiles

INV_SQRT_2PI = 0.3989422804014327
TWO_PI = 6.283185307179586   # Rsqrt(2*pi*sd) == INV_SQRT_2PI * rsqrt(sd)
EXP_OVF = 88.722839          # ln(float32 max): fp32 exp(x) == inf for x above this
E_BIG = 20.0                 # e > 20  =>  INV_SQRT_2PI * exp(-e^2/2) is exactly 0 in fp32
CLAMP_LO = -103.0
CLAMP_HI = 88.0
BINV = 1.0 / B


def _col_ap(flat_ap):
    """[2048]-flat AP -> [128, 16] (t = p*16 + f)."""
    return flat_ap.rearrange("(p f) -> p f", p=128)


def _build_nc():
    nc = bacc.Bacc("TRN2", target_bir_lowering=False)

    lisT_d = nc.dram_tensor("lis_t", [BC, D, T], BF16, kind="ExternalInput")
    lisN_d = nc.dram_tensor("lis_n", [BC, T, D], BF16, kind="ExternalInput")
    decT_d = nc.dram_tensor("dec_t", [D, BC], BF16, kind="ExternalInput")
    psiT_d = nc.dram_tensor("psi_t", [D, M], BF16, kind="ExternalInput")
    phiT_d = nc.dram_tensor("phi_t", [D, M], BF16, kind="ExternalInput")
    psib_d = nc.dram_tensor("psi_bias", [M], F32, kind="ExternalInput")
    phib_d = nc.dram_tensor("phi_bias", [M], F32, kind="ExternalInput")

    att_d = nc.dram_tensor("att", [BC, T], F32, kind="ExternalOutput")
    ctx_d = nc.dram_tensor("ctx", [BC, D], F32, kind="ExternalOutput")
    flg_d = nc.dram_tensor("flg", [1, 1], F32, kind="ExternalOutput")

    with tile.TileContext(nc) as tc:
        _body(nc, tc, lisT_d, lisN_d, decT_d, psiT_d, phiT_d, psib_d, phib_d,
              att_d, ctx_d, flg_d)
    nc.compile()
    return nc


def _body(nc, tc, lisT_d, lisN_d, decT_d, psiT_d, phiT_d, psib_d, phib_d,
          att_d, ctx_d, flg_d):
    with (
        tc.tile_pool(name="wp", bufs=1) as wp,
        tc.tile_pool(name="lt_pool", bufs=3) as lt_pool,
        tc.tile_pool(name="ln_pool", bufs=4) as ln_pool,
        tc.tile_pool(name="cl_pool", bufs=2) as cl_pool,
        tc.tile_pool(name="big", bufs=1) as big,
        tc.tile_pool(name="st_pool", bufs=2) as st_pool,
        tc.tile_pool(name="col", bufs=1) as col,
        tc.tile_pool(name="pp", bufs=2, space="PSUM") as pp,
        tc.tile_pool(name="ep", bufs=1, space="PSUM") as ep,
        tc.tile_pool(name="dram", bufs=1, space="DRAM") as dram,
    ):
        # ---------------- constants / weights ----------------
        psiT = wp.tile([128, NDT, M], BF16)
        nc.sync.dma_start(out=psiT[:], in_=psiT_d[:].rearrange("(dt p) m -> p dt m", p=128))
        phiT = wp.tile([128, NDT, M], BF16)
        nc.sync.dma_start(out=phiT[:], in_=phiT_d[:].rearrange("(dt p) m -> p dt m", p=128))
        decT = wp.tile([128, NDT, BC], BF16)
        nc.sync.dma_start(out=decT[:], in_=decT_d[:].rearrange("(dt p) b -> p dt b", p=128))
        psib = wp.tile([128, NMT], F32)
        nc.sync.dma_start(out=psib[:], in_=psib_d[:].rearrange("(mt p) -> p mt", p=128))
        phib = wp.tile([128, NMT], F32)
        nc.sync.dma_start(out=phib[:], in_=phib_d[:].rearrange("(mt p) -> p mt", p=128))

        ident = wp.tile([128, 128], F32)
        make_identity(nc, ident)
        ones8 = wp.tile([BC, 1], F32)
        nc.vector.memset(ones8, 1.0)
        ones16 = wp.tile([128, TP], F32)
        nc.vector.memset(ones16, 1.0)
        nan1 = wp.tile([128, 1], F32)
        nc.vector.memset(nan1, float("nan"))
        zero1 = wp.tile([128, 1], F32)
        nc.vector.memset(zero1, 0.0)

        # ---------------- comp_dec = relu(dec @ phi_w.T + phi_b) ----------------
        comp_decT = wp.tile([128, NMT, BC], BF16)   # [m-part, mt, b]
        for mt in range(NMT):
            ps_dec = pp.tile([128, BC], F32, tag="ps_small")
            for dt in range(NDT):
                nc.tensor.matmul(
                    ps_dec[:],
                    phiT[:, dt, ts(mt, 128)],
                    decT[:, dt, :],
                    start=(dt == 0),
                    stop=(dt == NDT - 1),
                )
            nc.scalar.activation(out=comp_decT[:, mt, :], in_=ps_dec[:],
                                 func=AF.Relu, bias=phib[:, mt:mt + 1], scale=1.0)

        # ---------------- main loop: comp_lis + energy ----------------
        e_sb = big.tile([BC, T], F32)
        for b in range(BC):
            ps_e = ep.tile([1, T], F32, tag="eps")
            for tci in range(NTC):
                lt = lt_pool.tile([128, NDT, TC], BF16)
                nc.sync.dma_start(
                    out=lt[:],
                    in_=lisT_d[b].rearrange("(dt p) t -> p dt t", p=128)[:, :, ts(tci, TC)],
                )
                clis = cl_pool.tile([128, NMT, TC], BF16)
                for mt in range(NMT):
                    ps_l = pp.tile([128, TC], F32, tag="ps_mm1")
                    for dt in range(NDT):
                        nc.tensor.matmul(
                            ps_l[:],
                            psiT[:, dt, ts(mt, 128)],
                            lt[:, dt, :],
                            start=(dt == 0),
                            stop=(dt == NDT - 1),
                        )
                    if mt % 2 == 0:
                        nc.scalar.activation(out=clis[:, mt, :], in_=ps_l[:],
                                             func=AF.Relu, bias=psib[:, mt:mt + 1], scale=1.0)
                    else:
                        nc.vector.tensor_scalar(clis[:, mt, :], ps_l[:],
                                                psib[:, mt:mt + 1], 0.0, OP.add, OP.max)
                for mt in range(NMT):
                    nc.tensor.matmul(
                        ps_e[:, ts(tci, TC)],
                        comp_decT[:, mt, b:b + 1],
                        clis[:, mt, :],
                        start=(mt == 0),
                        stop=(mt == NMT - 1),
                    )
            # engine accesses must start at a 32-aligned partition: copy the
            # M=1 PSUM row to an SBUF stage at partition 0, then DMA-scatter
            # into row b
            e_stage = st_pool.tile([1, T], F32, name="e_stage")
            if b % 2 == 0:
                nc.scalar.copy(out=e_stage[:], in_=ps_e[0:1, :])
            else:
                nc.vector.tensor_copy(out=e_stage[:], in_=ps_e[0:1, :])
            nc.sync.dma_start(out=e_sb[b:b + 1, :], in_=e_stage[:])

        # ---------------- pre-collective elementwise (overlaps AllReduce) -------
        sq = big.tile([BC, T], F32)
        nc.vector.tensor_mul(sq[:], e_sb[:], e_sb[:])
        prod = big.tile([BC, T], F32)
        nc.vector.memset(prod[:, T - 1:T], 0.0)
        nc.vector.tensor_mul(prod[:, 0:T - 1], e_sb[:, 0:T - 1], e_sb[:, 1:T])

        # ---------------- batch stats + AllReduce ----------------
        cc_in = dram.tile([3, T], F32)
        cc_out = dram.tile([3, T], F32)
        scr = dram.tile([8, T], F32)   # scratch rows: shifted-ss + coef rows + uo

        for k, src in enumerate((e_sb, sq, prod)):
            ps_s = ep.tile([1, T], F32, tag="eps", name=f"ps_stat{k}")
            for tci in range(NTC):
                nc.tensor.matmul(
                    ps_s[:, ts(tci, TC)],
                    ones8[:],
                    src[0:BC, ts(tci, TC)],
                    start=True,
                    stop=True,
                )
            st_row = st_pool.tile([1, T], F32)
            if k % 2 == 0:
                nc.scalar.copy(out=st_row[:], in_=ps_s[:])
            else:
                nc.vector.tensor_copy(out=st_row[:], in_=ps_s[:])
            nc.sync.dma_start(out=cc_in[k:k + 1, :], in_=st_row[:])

        nc.gpsimd.collective_compute(
            "AllReduce",
            OP.add,
            replica_groups=[list(range(NCORES))],
            ins=[cc_in.opt()],
            outs=[cc_out.opt()],
        )

        # ---------------- column stats in [128, 16] layout ----------------
        cc_flat = cc_out.rearrange("a b -> (a b)")
        gs1 = col.tile([128, TP], F32)
        nc.sync.dma_start(out=gs1[:], in_=_col_ap(cc_flat[0:T]))
        gs1S = col.tile([128, TP], F32)
        nc.sync.dma_start(out=gs1S[:], in_=_col_ap(cc_flat[1:T + 1]))
        gs2 = col.tile([128, TP], F32)
        nc.sync.dma_start(out=gs2[:], in_=_col_ap(cc_flat[T:2 * T]))
        gs12 = col.tile([128, TP], F32)
        nc.sync.dma_start(out=gs12[:], in_=_col_ap(cc_flat[2 * T:3 * T]))

        m2 = col.tile([128, TP], F32)
        nc.vector.scalar_tensor_tensor(m2[:], gs1[:], BINV, gs1[:], OP.mult, OP.mult)
        ss = col.tile([128, TP], F32)
        nc.vector.tensor_sub(ss[:], gs2[:], m2[:])
        scr_flat = scr.rearrange("a b -> (a b)")
        nc.sync.dma_start(out=_col_ap(scr_flat[0:T]), in_=ss[:])
        ssS = col.tile([128, TP], F32)
        nc.sync.dma_start(out=ssS[:], in_=_col_ap(scr_flat[1:T + 1]))

        sprod = col.tile([128, TP], F32)
        nc.vector.scalar_tensor_tensor(sprod[:], gs1[:], BINV, gs1S[:], OP.mult, OP.mult)
        num = col.tile([128, TP], F32)
        nc.vector.tensor_sub(num[:], gs12[:], sprod[:])
        d2 = col.tile([128, TP], F32)
        nc.vector.tensor_mul(d2[:], ss[:], ssS[:])
        rsq = col.tile([128, TP], F32)
        nc.scalar.activation(out=rsq[:], in_=d2[:], func=AF.Sqrt)
        nc.vector.reciprocal(rsq[:], rsq[:])
        rv = col.tile([128, TP], F32)
        nc.vector.tensor_mul(rv[:], num[:], rsq[:])
        rr2 = col.tile([128, TP], F32)
        nc.vector.tensor_mul(rr2[:], rv[:], rv[:])
        det = col.tile([128, TP], F32)
        nc.vector.tensor_scalar(det[:], rr2[:], -1.0, 1.0, OP.mult, OP.add)
        degen = col.tile([128, TP], F32)
        nc.vector.tensor_scalar(degen[:], det[:], 0.01, None, OP.is_lt)
        ndeg = col.tile([128, TP], F32)
        nc.vector.tensor_scalar(ndeg[:], degen[:], -1.0, 1.0, OP.mult, OP.add)
        # safe_det = det where det >= 0.01 else 1.0 (masks are exactly 0/1)
        sd = col.tile([128, TP], F32)
        nc.vector.tensor_mul(sd[:], det[:], ndeg[:])
        nc.vector.tensor_add(sd[:], sd[:], degen[:])
        invsd = col.tile([128, TP], F32)
        nc.vector.reciprocal(invsd[:], sd[:])
        c1 = col.tile([128, TP], F32)
        nc.vector.tensor_mul(c1[:], rv[:], invsd[:])
        c2 = col.tile([128, TP], F32)
        nc.vector.scalar_tensor_tensor(c2[:], c1[:], 0.5, rv[:], OP.mult, OP.mult)
        ccoef = col.tile([128, TP], F32)
        nc.scalar.activation(out=ccoef[:], in_=sd[:], func=AF.Sqrt, scale=TWO_PI)
        nc.vector.reciprocal(ccoef[:], ccoef[:])

        # coef rows -> DRAM -> [BC, T-2] broadcasts
        for k, srct in enumerate((c1, c2, ccoef, ndeg)):
            nc.sync.dma_start(out=_col_ap(scr_flat[(k + 1) * T:(k + 2) * T]), in_=srct[:])
        c1_8 = big.tile([BC, T - 2], F32)
        nc.gpsimd.dma_start(out=c1_8[:], in_=scr[1:2, 0:T - 2].to_broadcast([BC, T - 2]))
        c2_8 = big.tile([BC, T - 2], F32)
        nc.gpsimd.dma_start(out=c2_8[:], in_=scr[2:3, 0:T - 2].to_broadcast([BC, T - 2]))
        ccoef8 = big.tile([BC, T - 2], F32)
        nc.gpsimd.dma_start(out=ccoef8[:], in_=scr[3:4, 0:T - 2].to_broadcast([BC, T - 2]))
        ndeg8 = big.tile([BC, T - 2], F32)
        nc.gpsimd.dma_start(out=ndeg8[:], in_=scr[4:5, 0:T - 2].to_broadcast([BC, T - 2]))

        # ---------------- fallback flag: any all-zero energy column ------------
        gs2row = big.tile([1, T], F32)
        nc.sync.dma_start(out=gs2row[:], in_=cc_out[1:2, :])
        nc.vector.tensor_scalar(gs2row[:], gs2row[:], 0.0, None, OP.is_le)
        uo1 = wp.tile([1, 1], F32)
        nc.vector.reduce_max(uo1[:], gs2row[:], axis=AxisListType.X)
        nc.sync.dma_start(out=flg_d[:], in_=uo1[:])
        nc.sync.dma_start(out=scr[5:6, 0:1], in_=uo1[:])
        uo8 = wp.tile([BC, 1], F32)
        nc.gpsimd.dma_start(out=uo8[:], in_=scr[5:6, 0:1].to_broadcast([BC, 1]))

        # ---------------- copula pdf grid [BC, T-2] ----------------
        # att column i in 1..T-2 uses x1 = e[:, i-1], x2 = e[:, i], r = rr[i-1]
        x1 = e_sb[:, 0:T - 2]
        x2 = e_sb[:, 1:T - 1]

        tA = big.tile([BC, T - 2], F32)      # scratch A
        nc.vector.tensor_add(tA[:], sq[:, 0:T - 2], sq[:, 1:T - 1])   # x1^2 + x2^2
        expo = big.tile([BC, T - 2], F32)
        nc.vector.tensor_mul(expo[:], prod[:, 0:T - 2], c1_8[:])      # c1 * x1 x2
        nc.vector.tensor_mul(tA[:], tA[:], c2_8[:])                   # c2 * (x1^2+x2^2)
        nc.vector.tensor_sub(expo[:], expo[:], tA[:])                 # the copula exponent

        # NaN positions: fp32 exp(expo) overflows AND marginal underflowed to 0,
        # in a non-degenerate column.
        nc.vector.tensor_scalar(tA[:], expo[:], EXP_OVF, None, OP.is_gt)
        nc.vector.scalar_tensor_tensor(tA[:], x2, E_BIG, tA[:], OP.is_gt, OP.mult)
        nc.vector.tensor_mul(tA[:], tA[:], ndeg8[:])                  # and not degen
        nan_row = wp.tile([BC, 1], F32)
        nc.vector.reduce_max(nan_row[:], tA[:], axis=AxisListType.X)

        # marginal = norm.pdf(e), with exact zeros where it underflows
        marg = big.tile([BC, T], F32)
        nc.vector.tensor_scalar(marg[:], sq[:], -0.5, CLAMP_LO, OP.mult, OP.max)
        nc.scalar.activation(out=marg[:], in_=marg[:], func=AF.Exp)
        nc.vector.tensor_scalar(tA[:], x2, E_BIG, None, OP.is_le)     # marginal nonzero mask
        nc.vector.tensor_mul(marg[:, 1:T - 1], marg[:, 1:T - 1], tA[:])
        nc.vector.tensor_scalar(tA[:], x1, E_BIG, None, OP.is_le)
        nc.vector.tensor_mul(marg[:, 0:1], marg[:, 0:1], tA[:, 0:1])
        nc.vector.tensor_scalar(tA[:, 0:1], e_sb[:, T - 1:T], E_BIG, None, OP.is_le)
        nc.vector.tensor_mul(marg[:, T - 1:T], marg[:, T - 1:T], tA[:, 0:1])
        # note: cols 1..T-2 masked by x2; col 0 by x1[:,0]=e[:,0]; col T-1 by e[:,T-1]

        # cop (finite branch) and energy_f1
        nc.vector.tensor_scalar(expo[:], expo[:], CLAMP_HI, CLAMP_LO, OP.min, OP.max)
        nc.scalar.activation(out=expo[:], in_=expo[:], func=AF.Exp)
        ef1 = big.tile([BC, T], F32)
        nc.vector.tensor_mul(ef1[:, 1:T - 1], marg[:, 1:T - 1], expo[:])
        nc.vector.tensor_mul(ef1[:, 1:T - 1], ef1[:, 1:T - 1], ccoef8[:])
        # degenerate columns: cop = 10.0; exact 0/1 blend
        nc.vector.tensor_mul(ef1[:, 1:T - 1], ef1[:, 1:T - 1], ndeg8[:])
        nc.vector.scalar_tensor_tensor(tA[:], ndeg8[:], -10.0, marg[:, 1:T - 1],
                                       OP.mult, OP.mult)              # -10*ndeg*marg
        nc.vector.scalar_tensor_tensor(tA[:], marg[:, 1:T - 1], 10.0, tA[:],
                                       OP.mult, OP.add)               # 10*marg*degen
        nc.vector.tensor_add(ef1[:, 1:T - 1], ef1[:, 1:T - 1], tA[:])
        nc.vector.tensor_copy(out=ef1[:, 0:1], in_=marg[:, 0:1])
        nc.vector.tensor_copy(out=ef1[:, T - 1:T], in_=marg[:, T - 1:T])

        # ---------------- softmaxes ----------------
        def softmax_rows(dst, srcv, tag):
            rmax = wp.tile([BC, 1], F32, name=f"rmax_{tag}")
            nc.vector.reduce_max(rmax[:], srcv[:], axis=AxisListType.X)
            nc.vector.tensor_scalar(rmax[:], rmax[:], -1.0, None, OP.mult)
            nc.vector.tensor_scalar(dst[:], srcv[:], rmax[:], CLAMP_LO, OP.add, OP.max)
            nc.scalar.activation(out=dst[:], in_=dst[:], func=AF.Exp)
            rsum = wp.tile([BC, 1], F32, name=f"rsum_{tag}")
            nc.vector.reduce_sum(rsum[:], dst[:], axis=AxisListType.X)
            nc.vector.reciprocal(rsum[:], rsum[:])
            nc.vector.tensor_scalar(dst[:], dst[:], rsum[:], None, OP.mult)

        acs = big.tile([BC, T], F32)         # copula softmax
        softmax_rows(acs, ef1, "cop")
        sme = big.tile([BC, T], F32)         # plain softmax of energy
        softmax_rows(sme, e_sb, "eng")

        # blend: att_safe = uo * softmax(e) + (1-uo) * copula_score
        uo8inv = wp.tile([BC, 1], F32)
        nc.vector.tensor_scalar(uo8inv[:], uo8[:], -1.0, 1.0, OP.mult, OP.add)
        nc.vector.tensor_scalar(acs[:], acs[:], uo8inv[:], None, OP.mult)
        nc.vector.tensor_scalar(sme[:], sme[:], uo8[:], None, OP.mult)
        nc.vector.tensor_add(acs[:], acs[:], sme[:])

        # NaN rows (copula branch only)
        mask2 = wp.tile([BC, 1], F32)
        nc.vector.tensor_mul(mask2[:], nan_row[:], uo8inv[:])
        mask2u = wp.tile([BC, 1], mybir.dt.uint8)
        nc.vector.tensor_copy(out=mask2u[:], in_=mask2[:])
        nan_col = wp.tile([BC, 1], F32)
        nc.vector.select(nan_col[:], mask2u[:], nan1[0:BC, :], zero1[0:BC, :])

        att_out = big.tile([BC, T], F32)
        nc.vector.tensor_scalar(att_out[:], acs[:], nan_col[:], None, OP.add)
        nc.sync.dma_start(out=att_d[:], in_=att_out[:])

        # ---------------- context = sum_t att[b,t] * lis[b,t,:] ----------------
        attT = wp.tile([128, NCH, BC], BF16)
        for ch in range(NCH):
            ps_t = pp.tile([128, BC], F32, tag="ps_small", name="ps_tp")
            nc.tensor.transpose(ps_t[:, 0:BC], acs[0:BC, ts(ch, 128)], ident[0:BC, 0:BC])
            nc.vector.tensor_copy(out=attT[:, ch, :], in_=ps_t[:, 0:BC])

        ctx_sb = big.tile([BC, D], F32)
        for b in range(BC):
            ps_c = pp.tile([1, D], F32, tag="ps_small", name="ps_ctx")
            for q in range(NCH // 4):
                ln = ln_pool.tile([128, 4, D], BF16)
                nc.sync.dma_start(
                    out=ln[:],
                    in_=lisN_d[b].rearrange("(ch p) d -> p ch d", p=128)[:, 4 * q:4 * q + 4, :],
                )
                for j in range(4):
                    ch = 4 * q + j
                    nc.tensor.matmul(
                        ps_c[:],
                        attT[:, ch, b:b + 1],
                        ln[:, j, :],
                        start=(ch == 0),
                        stop=(ch == NCH - 1),
                    )
            c_stage = st_pool.tile([1, D], F32, name="c_stage")
            if b % 2 == 0:
                nc.scalar.copy(out=c_stage[:], in_=ps_c[0:1, :])
            else:
                nc.vector.tensor_copy(out=c_stage[:], in_=ps_c[0:1, :])
            nc.sync.dma_start(out=ctx_sb[b:b + 1, :], in_=c_stage[:])

        nc.vector.tensor_scalar(ctx_sb[:], ctx_sb[:], nan_col[:], None, OP.add)
        nc.sync.dma_start(out=ctx_d[:], in_=ctx_sb[:])


_NC_CACHE = {}
LAST_RESULTS = None


def _install_trace_shim():
    """The agent container's antenv stub lacks axon_hooks; register the NTFF
    profile hook ourselves so run_bass_kernel_spmd(trace=True) works."""
    import sys
    import types

    try:
        from antenv.axon_hooks import get_axon_ntff_profile_hook  # noqa: F401
    except ImportError:
        import antenv

        mod = types.ModuleType("antenv.axon_hooks")
        mod._hook = None
        mod.set_axon_ntff_profile_hook = lambda h: setattr(mod, "_hook", h)
        mod.get_axon_ntff_profile_hook = lambda: mod._hook
        sys.modules["antenv.axon_hooks"] = mod
        antenv.axon_hooks = mod
        try:
            from trn_agent_boot.trn_boot import _ntff_profile_via_ctypes
            mod._hook = _ntff_profile_via_ctypes("/opt/axon/libaxon_pjrt.so")
        except Exception:
            pass
    import concourse.bass_utils as bu
    bu.upload_artifacts = lambda tmpdir: tmpdir


def _get_nc():
    if "nc" not in _NC_CACHE:
        _NC_CACHE["nc"] = _build_nc()
    return _NC_CACHE["nc"]


def kernel(decoder_state, listener_feature, phi_w, phi_b, psi_w, psi_b):
    global LAST_RESULTS
    bf16 = ml_dtypes.bfloat16

    dec = np.asarray(decoder_state, np.float32)[:, 0, :]          # [B, D]
    lis_bf = np.asarray(listener_feature, np.float32).astype(bf16)  # [B, T, D]
    psiT = np.ascontiguousarray(np.asarray(psi_w, np.float32).T).astype(bf16)
    phiT = np.ascontiguousarray(np.asarray(phi_w, np.float32).T).astype(bf16)
    psib = np.asarray(psi_b, np.float32)
    phib = np.asarray(phi_b, np.float32)

    in_maps = []
    for c in range(NCORES):
        bs = slice(c * BC, (c + 1) * BC)
        in_maps.append({
            "lis_t": np.ascontiguousarray(lis_bf[bs].transpose(0, 2, 1)),
            "lis_n": np.ascontiguousarray(lis_bf[bs]),
            "dec_t": np.ascontiguousarray(dec[bs].T).astype(bf16),
            "psi_t": psiT,
            "phi_t": phiT,
            "psi_bias": psib,
            "phi_bias": phib,
        })

    trace = bool(os.environ.get("KERNEL_TRACE"))
    if trace:
        _install_trace_shim()
    nc = _get_nc()
    res = run_bass_kernel_spmd(
        nc,
        in_maps,
        core_ids=list(range(NCORES)),
        trace=trace,
    )
    LAST_RESULTS = res

    att = np.concatenate([res.results[c]["att"] for c in range(NCORES)], axis=0)
    ctx = np.concatenate([res.results[c]["ctx"] for c in range(NCORES)], axis=0)
    flag = float(res.results[0]["flg"][0, 0])
    count_original = np.int32(1 if flag > 0.5 else 0)
    count_copula = np.int32(1 - count_original)
    return att, ctx, count_original, count_copula


# revision 29
# speedup vs baseline: 1.0101x; 1.0101x over previous
"""Trainium2 Bass kernel for nn_Attention_37074157699663.

Copula attention: MLP preprocess (phi/psi + ReLU), energy = comp_dec . comp_lis,
adjacent-column Pearson correlation over the batch, bivariate Gaussian copula
pdf, softmax, context. Data-parallel over batch across 8 NeuronCores; the three
per-column stat vectors (sum e, sum e^2, sum e[t]e[t+1]) are AllReduced.

The reference computes norm.pdf(e) (underflows fp32 to 0) times exp(copula
exponent) (overflows fp32 to inf), so reference rows are NaN wherever any
column overflows. We reproduce that exactly by computing the overflow mask
explicitly (expo > ln(float32 max)) instead of trusting device exp() inf
semantics, then injecting NaN per row.
"""

import os

import numpy as np
import ml_dtypes

import concourse.bacc as bacc
import concourse.bass as bass
import concourse.tile as tile
from concourse import mybir
from concourse.bass_utils import run_bass_kernel_spmd
from concourse.masks import make_identity
from bass_rust import AxisListType

BF16 = mybir.dt.bfloat16
F32 = mybir.dt.float32
AF = mybir.ActivationFunctionType
OP = mybir.AluOpType
ts = bass.ts

B, T, D, M = 64, 2048, 512, 512
NCORES = 8
BC = B // NCORES        # 8 batch rows per core
TC = 512                # t-chunk for MM1/MM2 (one PSUM bank)
NTC = T // TC           # 4
NDT = D // 128          # 4 contraction tiles
NMT = M // 128          # 4 m tiles
NCH = T // 128          # 16 chunks for the context matmul
TP = T // 128           # 16: free size of [128, TP] column-stat tiles

INV_SQRT_2PI = 0.3989422804014327
TWO_PI = 6.283185307179586   # Rsqrt(2*pi*sd) == INV_SQRT_2PI * rsqrt(sd)
EXP_OVF = 88.722839          # ln(float32 max): fp32 exp(x) == inf for x above this
E_BIG = 20.0                 # e > 20  =>  INV_SQRT_2PI * exp(-e^2/2) is exactly 0 in fp32
CLAMP_LO = -103.0
CLAMP_HI = 88.0
BINV = 1.0 / B


def _col_ap(flat_ap):
    """[2048]-flat AP -> [128, 16] (t = p*16 + f)."""
    return flat_ap.rearrange("(p f) -> p f", p=128)


def _build_nc():
    nc = bacc.Bacc("TRN2", target_bir_lowering=False)

    lisT_d = nc.dram_tensor("lis_t", [BC, D, T], BF16, kind="ExternalInput")
    lisN_d = nc.dram_tensor("lis_n", [BC, T, D], BF16, kind="ExternalInput")
    decT_d = nc.dram_tensor("dec_t", [D, BC], BF16, kind="ExternalInput")
    psiT_d = nc.dram_tensor("psi_t", [D, M], BF16, kind="ExternalInput")
    phiT_d = nc.dram_tensor("phi_t", [D, M], BF16, kind="ExternalInput")
    psib_d = nc.dram_tensor("psi_bias", [M], F32, kind="ExternalInput")
    phib_d = nc.dram_tensor("phi_bias", [M], F32, kind="ExternalInput")

    att_d = nc.dram_tensor("att", [BC, T], F32, kind="ExternalOutput")
    ctx_d = nc.dram_tensor("ctx", [BC, D], F32, kind="ExternalOutput")
    flg_d = nc.dram_tensor("flg", [1, 1], F32, kind="ExternalOutput")

    with tile.TileContext(nc) as tc:
        _body(nc, tc, lisT_d, lisN_d, decT_d, psiT_d, phiT_d, psib_d, phib_d,
              att_d, ctx_d, flg_d)
    nc.compile()
    return nc


def _body(nc, tc, lisT_d, lisN_d, decT_d, psiT_d, phiT_d, psib_d, phib_d,
          att_d, ctx_d, flg_d):
    with (
        tc.tile_pool(name="wp", bufs=1) as wp,
        tc.tile_pool(name="lt_pool", bufs=4) as lt_pool,
        tc.tile_pool(name="ln_pool", bufs=6) as ln_pool,
        tc.tile_pool(name="cl_pool", bufs=2) as cl_pool,
        tc.tile_pool(name="big", bufs=1) as big,
        tc.tile_pool(name="st_pool", bufs=2) as st_pool,
        tc.tile_pool(name="col", bufs=1) as col,
        tc.tile_pool(name="pp", bufs=3, space="PSUM") as pp,
        tc.tile_pool(name="ep", bufs=2, space="PSUM") as ep,
        tc.tile_pool(name="dram", bufs=1, space="DRAM") as dram,
    ):
        # ---------------- constants / weights ----------------
        psiT = wp.tile([128, NDT, M], BF16)
        nc.sync.dma_start(out=psiT[:], in_=psiT_d[:].rearrange("(dt p) m -> p dt m", p=128))
        phiT = wp.tile([128, NDT, M], BF16)
        nc.sync.dma_start(out=phiT[:], in_=phiT_d[:].rearrange("(dt p) m -> p dt m", p=128))
        decT = wp.tile([128, NDT, BC], BF16)
        nc.sync.dma_start(out=decT[:], in_=decT_d[:].rearrange("(dt p) b -> p dt b", p=128))
        psib = wp.tile([128, NMT], F32)
        nc.sync.dma_start(out=psib[:], in_=psib_d[:].rearrange("(mt p) -> p mt", p=128))
        phib = wp.tile([128, NMT], F32)
        nc.sync.dma_start(out=phib[:], in_=phib_d[:].rearrange("(mt p) -> p mt", p=128))

        ident = wp.tile([128, 128], F32)
        make_identity(nc, ident)
        ones8 = wp.tile([BC, 1], F32)
        nc.vector.memset(ones8, 1.0)
        ones16 = wp.tile([128, TP], F32)
        nc.vector.memset(ones16, 1.0)
        nan1 = wp.tile([128, 1], F32)
        nc.vector.memset(nan1, float("nan"))
        zero1 = wp.tile([128, 1], F32)
        nc.vector.memset(zero1, 0.0)

        # ---------------- comp_dec = relu(dec @ phi_w.T + phi_b) ----------------
        comp_decT = wp.tile([128, NMT, BC], BF16)   # [m-part, mt, b]
        for mt in range(NMT):
            ps_dec = pp.tile([128, BC], F32, tag="ps_small")
            for dt in range(NDT):
                nc.tensor.matmul(
                    ps_dec[:],
                    phiT[:, dt, ts(mt, 128)],
                    decT[:, dt, :],
                    start=(dt == 0),
                    stop=(dt == NDT - 1),
                )
            nc.scalar.activation(out=comp_decT[:, mt, :], in_=ps_dec[:],
                                 func=AF.Relu, bias=phib[:, mt:mt + 1], scale=1.0)

        # ---------------- main loop: comp_lis + energy ----------------
        e_sb = big.tile([BC, T], F32)
        for b in range(BC):
            for tci in range(NTC):
                lt = lt_pool.tile([128, NDT, TC], BF16)
                nc.sync.dma_start(
                    out=lt[:],
                    in_=lisT_d[b].rearrange("(dt p) t -> p dt t", p=128)[:, :, ts(tci, TC)],
                )
                clis = cl_pool.tile([128, NMT, TC], BF16)
                for mt in range(NMT):
                    ps_l = pp.tile([128, TC], F32, tag="ps_mm1")
                    for dt in range(NDT):
                        nc.tensor.matmul(
                            ps_l[:],
                            psiT[:, dt, ts(mt, 128)],
                            lt[:, dt, :],
                            start=(dt == 0),
                            stop=(dt == NDT - 1),
                        )
                    if mt % 2 == 0:
                        nc.scalar.activation(out=clis[:, mt, :], in_=ps_l[:],
                                             func=AF.Relu, bias=psib[:, mt:mt + 1], scale=1.0)
                    else:
                        nc.vector.tensor_scalar(clis[:, mt, :], ps_l[:],
                                                psib[:, mt:mt + 1], 0.0, OP.add, OP.max)
                ps_e = ep.tile([1, TC], F32, tag="eps")
                for mt in range(NMT):
                    nc.tensor.matmul(
                        ps_e[:],
                        comp_decT[:, mt, b:b + 1],
                        clis[:, mt, :],
                        start=(mt == 0),
                        stop=(mt == NMT - 1),
                    )
                # engine accesses must start at a 32-aligned partition: copy
                # the M=1 PSUM row to an SBUF stage at partition 0, then
                # DMA-scatter into row b
                e_stage = st_pool.tile([1, TC], F32, name="e_stage")
                if (b + tci) % 2 == 0:
                    nc.scalar.copy(out=e_stage[:], in_=ps_e[0:1, :])
                else:
                    nc.vector.tensor_copy(out=e_stage[:], in_=ps_e[0:1, :])
                nc.sync.dma_start(out=e_sb[b:b + 1, ts(tci, TC)], in_=e_stage[:])

        # ---------------- softmax helper ----------------
        def softmax_rows(dst, srcv, tag):
            rmax = wp.tile([BC, 1], F32, name=f"rmax_{tag}")
            nc.vector.reduce_max(rmax[:], srcv[:], axis=AxisListType.X)
            nc.vector.tensor_scalar(rmax[:], rmax[:], -1.0, None, OP.mult)
            nc.vector.tensor_scalar(dst[:], srcv[:], rmax[:], CLAMP_LO, OP.add, OP.max)
            nc.scalar.activation(out=dst[:], in_=dst[:], func=AF.Exp)
            rsum = wp.tile([BC, 1], F32, name=f"rsum_{tag}")
            nc.vector.reduce_sum(rsum[:], dst[:], axis=AxisListType.X)
            nc.vector.reciprocal(rsum[:], rsum[:])
            nc.vector.tensor_scalar(dst[:], dst[:], rsum[:], None, OP.mult)

        # ---------------- batch stats + AllReduce ----------------
        cc_in = dram.tile([3, T], F32)
        cc_out = dram.tile([3, T], F32)
        scr = dram.tile([8, T], F32)   # scratch rows: shifted-ss + coef rows + uo

        sq = big.tile([BC, T], F32)
        nc.vector.tensor_mul(sq[:], e_sb[:], e_sb[:])
        prod = big.tile([BC, T], F32)
        nc.vector.memset(prod[:, T - 1:T], 0.0)
        nc.vector.tensor_mul(prod[:, 0:T - 1], e_sb[:, 0:T - 1], e_sb[:, 1:T])

        for k, src in enumerate((e_sb, sq, prod)):
            st_row = st_pool.tile([1, T], F32)
            for tci in range(NTC):
                ps_s = ep.tile([1, TC], F32, tag="eps", name="ps_stat")
                nc.tensor.matmul(
                    ps_s[:],
                    ones8[:],
                    src[0:BC, ts(tci, TC)],
                    start=True,
                    stop=True,
                )
                if (k + tci) % 2 == 0:
                    nc.scalar.copy(out=st_row[:, ts(tci, TC)], in_=ps_s[0:1, :])
                else:
                    nc.vector.tensor_copy(out=st_row[:, ts(tci, TC)], in_=ps_s[0:1, :])
            nc.sync.dma_start(out=cc_in[k:k + 1, :], in_=st_row[:])

        nc.gpsimd.collective_compute(
            "AllReduce",
            OP.add,
            replica_groups=[list(range(NCORES))],
            ins=[cc_in.opt()],
            outs=[cc_out.opt()],
        )

        # ---------------- local-only work overlapping the AllReduce ------------
        # marginal = norm.pdf(e) with exact zeros where it underflows in fp32
        marg = big.tile([BC, T], F32, tag="shared_b")
        nc.vector.tensor_scalar(marg[:], sq[:], -0.5, CLAMP_LO, OP.mult, OP.max)
        nc.scalar.activation(out=marg[:], in_=marg[:], func=AF.Exp)
        mz = big.tile([BC, T], F32)
        nc.gpsimd.tensor_scalar(mz[:], e_sb[:], E_BIG, None, OP.is_le)
        nc.vector.tensor_mul(marg[:], marg[:], mz[:])
        sqs = big.tile([BC, T - 2], F32)
        nc.gpsimd.tensor_add(sqs[:], sq[:, 0:T - 2], sq[:, 1:T - 1])   # x1^2 + x2^2
        sme = big.tile([BC, T], F32)
        softmax_rows(sme, e_sb, "eng")   # plain-softmax fallback branch

        # ---------------- column stats in [128, 16] layout ----------------
        cc_flat = cc_out.rearrange("a b -> (a b)")
        gs1 = col.tile([128, TP], F32)
        nc.sync.dma_start(out=gs1[:], in_=_col_ap(cc_flat[0:T]))
        gs1S = col.tile([128, TP], F32)
        nc.sync.dma_start(out=gs1S[:], in_=_col_ap(cc_flat[1:T + 1]))
        gs2 = col.tile([128, TP], F32)
        nc.sync.dma_start(out=gs2[:], in_=_col_ap(cc_flat[T:2 * T]))
        gs12 = col.tile([128, TP], F32)
        nc.sync.dma_start(out=gs12[:], in_=_col_ap(cc_flat[2 * T:3 * T]))

        m2 = col.tile([128, TP], F32)
        nc.vector.scalar_tensor_tensor(m2[:], gs1[:], BINV, gs1[:], OP.mult, OP.mult)
        ss = col.tile([128, TP], F32)
        nc.vector.tensor_sub(ss[:], gs2[:], m2[:])
        scr_flat = scr.rearrange("a b -> (a b)")
        nc.sync.dma_start(out=_col_ap(scr_flat[0:T]), in_=ss[:])
        ssS = col.tile([128, TP], F32)
        nc.sync.dma_start(out=ssS[:], in_=_col_ap(scr_flat[1:T + 1]))

        sprod = col.tile([128, TP], F32)
        nc.vector.scalar_tensor_tensor(sprod[:], gs1[:], BINV, gs1S[:], OP.mult, OP.mult)
        num = col.tile([128, TP], F32)
        nc.vector.tensor_sub(num[:], gs12[:], sprod[:])
        d2 = col.tile([128, TP], F32)
        nc.vector.tensor_mul(d2[:], ss[:], ssS[:])
        rsq = col.tile([128, TP], F32)
        nc.scalar.activation(out=rsq[:], in_=d2[:], func=AF.Sqrt)
        nc.vector.reciprocal(rsq[:], rsq[:])
        rv = col.tile([128, TP], F32)
        nc.vector.tensor_mul(rv[:], num[:], rsq[:])
        rr2 = col.tile([128, TP], F32)
        nc.vector.tensor_mul(rr2[:], rv[:], rv[:])
        det = col.tile([128, TP], F32)
        nc.vector.tensor_scalar(det[:], rr2[:], -1.0, 1.0, OP.mult, OP.add)
        degen = col.tile([128, TP], F32)
        nc.vector.tensor_scalar(degen[:], det[:], 0.01, None, OP.is_lt)
        ndeg = col.tile([128, TP], F32)
        nc.vector.tensor_scalar(ndeg[:], degen[:], -1.0, 1.0, OP.mult, OP.add)
        # safe_det = det where det >= 0.01 else 1.0 (masks are exactly 0/1)
        sd = col.tile([128, TP], F32)
        nc.vector.tensor_mul(sd[:], det[:], ndeg[:])
        nc.vector.tensor_add(sd[:], sd[:], degen[:])
        invsd = col.tile([128, TP], F32)
        nc.vector.reciprocal(invsd[:], sd[:])
        c1 = col.tile([128, TP], F32)
        nc.vector.tensor_mul(c1[:], rv[:], invsd[:])
        c2 = col.tile([128, TP], F32)
        nc.vector.scalar_tensor_tensor(c2[:], c1[:], 0.5, rv[:], OP.mult, OP.mult)
        ccoef = col.tile([128, TP], F32)
        nc.scalar.activation(out=ccoef[:], in_=sd[:], func=AF.Sqrt, scale=TWO_PI)
        nc.vector.reciprocal(ccoef[:], ccoef[:])

        # coef rows -> DRAM -> [BC, T-2] broadcasts
        for k, srct in enumerate((c1, c2, ccoef, ndeg)):
            nc.sync.dma_start(out=_col_ap(scr_flat[(k + 1) * T:(k + 2) * T]), in_=srct[:])
        c1_8 = big.tile([BC, T - 2], F32)
        nc.gpsimd.dma_start(out=c1_8[:], in_=scr[1:2, 0:T - 2].to_broadcast([BC, T - 2]))
        c2_8 = big.tile([BC, T - 2], F32)
        nc.gpsimd.dma_start(out=c2_8[:], in_=scr[2:3, 0:T - 2].to_broadcast([BC, T - 2]))
        ccoef8 = big.tile([BC, T - 2], F32)
        nc.gpsimd.dma_start(out=ccoef8[:], in_=scr[3:4, 0:T - 2].to_broadcast([BC, T - 2]))
        ndeg8 = big.tile([BC, T - 2], F32)
        nc.gpsimd.dma_start(out=ndeg8[:], in_=scr[4:5, 0:T - 2].to_broadcast([BC, T - 2]))

        # ---------------- fallback flag: any all-zero energy column ------------
        gs2row = big.tile([1, T], F32, tag="shared_a")
        nc.sync.dma_start(out=gs2row[:], in_=cc_out[1:2, :])
        nc.vector.tensor_scalar(gs2row[:], gs2row[:], 0.0, None, OP.is_le)
        uo1 = wp.tile([1, 1], F32)
        nc.vector.reduce_max(uo1[:], gs2row[:], axis=AxisListType.X)
        nc.sync.dma_start(out=flg_d[:], in_=uo1[:])
        nc.sync.dma_start(out=scr[5:6, 0:1], in_=uo1[:])
        uo8 = wp.tile([BC, 1], F32)
        nc.gpsimd.dma_start(out=uo8[:], in_=scr[5:6, 0:1].to_broadcast([BC, 1]))

        # ---------------- copula pdf grid [BC, T-2] ----------------
        # att column i in 1..T-2 uses x1 = e[:, i-1], x2 = e[:, i], r = rr[i-1]
        x2 = e_sb[:, 1:T - 1]
        tA = mz[:, 0:T - 2]                  # mz is free after the marg mask

        expo = big.tile([BC, T - 2], F32)
        nc.vector.tensor_mul(expo[:], prod[:, 0:T - 2], c1_8[:])      # c1 * x1 x2
        nc.vector.tensor_mul(tA, sqs[:], c2_8[:])                     # c2 * (x1^2+x2^2)
        nc.vector.tensor_sub(expo[:], expo[:], tA)                    # the copula exponent

        # NaN positions: fp32 exp(expo) overflows AND marginal underflowed to 0,
        # in a non-degenerate column.
        nc.vector.tensor_scalar(tA, expo[:], EXP_OVF, None, OP.is_gt)
        nc.vector.scalar_tensor_tensor(tA, x2, E_BIG, tA, OP.is_gt, OP.mult)
        nc.vector.tensor_mul(tA, tA, ndeg8[:])                        # and not degen
        nan_row = wp.tile([BC, 1], F32)
        nc.vector.reduce_max(nan_row[:], tA, axis=AxisListType.X)

        # cop (finite branch) and energy_f1
        nc.vector.tensor_scalar(expo[:], expo[:], CLAMP_HI, CLAMP_LO, OP.min, OP.max)
        nc.scalar.activation(out=expo[:], in_=expo[:], func=AF.Exp)
        ef1 = big.tile([BC, T], F32, tag="shared_a")
        nc.vector.tensor_mul(ef1[:, 1:T - 1], marg[:, 1:T - 1], expo[:])
        nc.vector.tensor_mul(ef1[:, 1:T - 1], ef1[:, 1:T - 1], ccoef8[:])
        # degenerate columns: cop = 10.0; exact 0/1 blend
        nc.vector.tensor_mul(ef1[:, 1:T - 1], ef1[:, 1:T - 1], ndeg8[:])
        nc.vector.scalar_tensor_tensor(tA, ndeg8[:], -10.0, marg[:, 1:T - 1],
                                       OP.mult, OP.mult)              # -10*ndeg*marg
        nc.vector.scalar_tensor_tensor(tA, marg[:, 1:T - 1], 10.0, tA,
                                       OP.mult, OP.add)               # 10*marg*degen
        nc.vector.tensor_add(ef1[:, 1:T - 1], ef1[:, 1:T - 1], tA)
        nc.vector.tensor_copy(out=ef1[:, 0:1], in_=marg[:, 0:1])
        nc.vector.tensor_copy(out=ef1[:, T - 1:T], in_=marg[:, T - 1:T])

        acs = big.tile([BC, T], F32)         # copula softmax
        softmax_rows(acs, ef1, "cop")

        # blend: att_safe = uo * softmax(e) + (1-uo) * copula_score
        uo8inv = wp.tile([BC, 1], F32)
        nc.vector.tensor_scalar(uo8inv[:], uo8[:], -1.0, 1.0, OP.mult, OP.add)
        nc.vector.tensor_scalar(acs[:], acs[:], uo8inv[:], None, OP.mult)
        nc.vector.tensor_scalar(sme[:], sme[:], uo8[:], None, OP.mult)
        nc.vector.tensor_add(acs[:], acs[:], sme[:])

        # NaN rows (copula branch only)
        mask2 = wp.tile([BC, 1], F32)
        nc.vector.tensor_mul(mask2[:], nan_row[:], uo8inv[:])
        mask2u = wp.tile([BC, 1], mybir.dt.uint8)
        nc.vector.tensor_copy(out=mask2u[:], in_=mask2[:])
        nan_col = wp.tile([BC, 1], F32)
        nc.vector.select(nan_col[:], mask2u[:], nan1[0:BC, :], zero1[0:BC, :])

        att_out = big.tile([BC, T], F32, tag="shared_b")
        nc.vector.tensor_scalar(att_out[:], acs[:], nan_col[:], None, OP.add)
        nc.sync.dma_start(out=att_d[:], in_=att_out[:])

        # ---------------- context = sum_t att[b,t] * lis[b,t,:] ----------------
        attT = wp.tile([128, NCH, BC], BF16)
        for ch in range(NCH):
            ps_t = pp.tile([128, BC], F32, tag="ps_small", name="ps_tp")
            nc.tensor.transpose(ps_t[:, 0:BC], acs[0:BC, ts(ch, 128)], ident[0:BC, 0:BC])
            nc.vector.tensor_copy(out=attT[:, ch, :], in_=ps_t[:, 0:BC])

        ctx_sb = big.tile([BC, D], F32)
        for b in range(BC):
            ps_c = pp.tile([1, D], F32, tag="ps_small", name="ps_ctx")
            for q in range(NCH // 4):
                ln = ln_pool.tile([128, 4, D], BF16)
                nc.sync.dma_start(
                    out=ln[:],
                    in_=lisN_d[b].rearrange("(ch p) d -> p ch d", p=128)[:, 4 * q:4 * q + 4, :],
                )
                for j in range(4):
                    ch = 4 * q + j
                    nc.tensor.matmul(
                        ps_c[:],
                        attT[:, ch, b:b + 1],
                        ln[:, j, :],
                        start=(ch == 0),
                        stop=(ch == NCH - 1),
                    )
            c_stage = st_pool.tile([1, D], F32, name="c_stage")
            if b % 2 == 0:
                nc.scalar.copy(out=c_stage[:], in_=ps_c[0:1, :])
            else:
                nc.vector.tensor_copy(out=c_stage[:], in_=ps_c[0:1, :])
            nc.sync.dma_start(out=ctx_sb[b:b + 1, :], in_=c_stage[:])

        nc.vector.tensor_scalar(ctx_sb[:], ctx_sb[:], nan_col[:], None, OP.add)
        nc.sync.dma_start(out=ctx_d[:], in_=ctx_sb[:])


_NC_CACHE = {}
LAST_RESULTS = None


def _install_trace_shim():
    """The agent container's antenv stub lacks axon_hooks; register the NTFF
    profile hook ourselves so run_bass_kernel_spmd(trace=True) works."""
    import sys
    import types

    try:
        from antenv.axon_hooks import get_axon_ntff_profile_hook  # noqa: F401
    except ImportError:
        import antenv

        mod = types.ModuleType("antenv.axon_hooks")
        mod._hook = None
        mod.set_axon_ntff_profile_hook = lambda h: setattr(mod, "_hook", h)
        mod.get_axon_ntff_profile_hook = lambda: mod._hook
        sys.modules["antenv.axon_hooks"] = mod
        antenv.axon_hooks = mod
        try:
            from trn_agent_boot.trn_boot import _ntff_profile_via_ctypes
            mod._hook = _ntff_profile_via_ctypes("/opt/axon/libaxon_pjrt.so")
        except Exception:
            pass
    import concourse.bass_utils as bu
    bu.upload_artifacts = lambda tmpdir: tmpdir


def _get_nc():
    if "nc" not in _NC_CACHE:
        _NC_CACHE["nc"] = _build_nc()
    return _NC_CACHE["nc"]


def kernel(decoder_state, listener_feature, phi_w, phi_b, psi_w, psi_b):
    global LAST_RESULTS
    bf16 = ml_dtypes.bfloat16

    dec = np.asarray(decoder_state, np.float32)[:, 0, :]          # [B, D]
    lis_bf = np.asarray(listener_feature, np.float32).astype(bf16)  # [B, T, D]
    psiT = np.ascontiguousarray(np.asarray(psi_w, np.float32).T).astype(bf16)
    phiT = np.ascontiguousarray(np.asarray(phi_w, np.float32).T).astype(bf16)
    psib = np.asarray(psi_b, np.float32)
    phib = np.asarray(phi_b, np.float32)

    in_maps = []
    for c in range(NCORES):
        bs = slice(c * BC, (c + 1) * BC)
        in_maps.append({
            "lis_t": np.ascontiguousarray(lis_bf[bs].transpose(0, 2, 1)),
            "lis_n": np.ascontiguousarray(lis_bf[bs]),
            "dec_t": np.ascontiguousarray(dec[bs].T).astype(bf16),
            "psi_t": psiT,
            "phi_t": phiT,
            "psi_bias": psib,
            "phi_bias": phib,
        })

    trace = bool(os.environ.get("KERNEL_TRACE"))
    if trace:
        _install_trace_shim()
    nc = _get_nc()
    res = run_bass_kernel_spmd(
        nc,
        in_maps,
        core_ids=list(range(NCORES)),
        trace=trace,
    )
    LAST_RESULTS = res

    att = np.concatenate([res.results[c]["att"] for c in range(NCORES)], axis=0)
    ctx = np.concatenate([res.results[c]["ctx"] for c in range(NCORES)], axis=0)
    flag = float(res.results[0]["flg"][0, 0])
    count_original = np.int32(1 if flag > 0.5 else 0)
    count_copula = np.int32(1 - count_original)
    return att, ctx, count_original, count_copula


# revision 40
# speedup vs baseline: 1.0878x; 1.0769x over previous
"""Trainium2 Bass kernel for nn_Attention_37074157699663.

Copula attention: MLP preprocess (phi/psi + ReLU), energy = comp_dec . comp_lis,
adjacent-column Pearson correlation over the batch, bivariate Gaussian copula
pdf, softmax, context. Data-parallel over batch across 8 NeuronCores; the three
per-column stat vectors (sum e, sum e^2, sum e[t]e[t+1]) are AllReduced.

The reference computes norm.pdf(e) (underflows fp32 to 0) times exp(copula
exponent) (overflows fp32 to inf), so reference rows are NaN wherever any
column overflows. We reproduce that exactly by computing the overflow mask
explicitly (expo > ln(float32 max)) instead of trusting device exp() inf
semantics, then injecting NaN per row.
"""

import os

import numpy as np
import ml_dtypes

import concourse.bacc as bacc
import concourse.bass as bass
import concourse.tile as tile
from concourse import mybir
from concourse.bass_utils import run_bass_kernel_spmd
from concourse.masks import make_identity
from bass_rust import AxisListType

BF16 = mybir.dt.bfloat16
FP8 = mybir.dt.float8e4
F32 = mybir.dt.float32
ATT_SCALE = 128.0   # att probs (~5e-4) scaled into fp8e4m3's normal range
AF = mybir.ActivationFunctionType
OP = mybir.AluOpType
ts = bass.ts

B, T, D, M = 64, 2048, 512, 512
NCORES = 8
BC = B // NCORES        # 8 batch rows per core
TC = 512                # t-chunk for MM1/MM2 (one PSUM bank)
NTC = T // TC           # 4
NDT = D // 128          # 4 contraction tiles
NMT = M // 128          # 4 m tiles
NCH = T // 128          # 16 chunks for the context matmul
TP = T // 128           # 16: free size of [128, TP] column-stat tiles

INV_SQRT_2PI = 0.3989422804014327
TWO_PI = 6.283185307179586   # Rsqrt(2*pi*sd) == INV_SQRT_2PI * rsqrt(sd)
EXP_OVF = 88.722839          # ln(float32 max): fp32 exp(x) == inf for x above this
E_BIG = 20.0                 # e > 20  =>  INV_SQRT_2PI * exp(-e^2/2) is exactly 0 in fp32
CLAMP_LO = -103.0
CLAMP_HI = 88.0
BINV = 1.0 / B


def _col_ap(flat_ap):
    """[2048]-flat AP -> [128, 16] (t = p*16 + f)."""
    return flat_ap.rearrange("(p f) -> p f", p=128)


def _build_nc():
    nc = bacc.Bacc("TRN2", target_bir_lowering=False)

    lisT_d = nc.dram_tensor("lis_t", [BC, D, T], BF16, kind="ExternalInput")
    lisN_d = nc.dram_tensor("lis_n", [BC, T, D], FP8, kind="ExternalInput")
    decT_d = nc.dram_tensor("dec_t", [D, BC], BF16, kind="ExternalInput")
    psiT_d = nc.dram_tensor("psi_t", [D, M], BF16, kind="ExternalInput")
    phiT_d = nc.dram_tensor("phi_t", [D, M], BF16, kind="ExternalInput")
    psib_d = nc.dram_tensor("psi_bias", [M], F32, kind="ExternalInput")
    phib_d = nc.dram_tensor("phi_bias", [M], F32, kind="ExternalInput")

    att_d = nc.dram_tensor("att", [BC, T], F32, kind="ExternalOutput")
    ctx_d = nc.dram_tensor("ctx", [BC, D], F32, kind="ExternalOutput")
    flg_d = nc.dram_tensor("flg", [1, 1], F32, kind="ExternalOutput")

    with tile.TileContext(nc) as tc:
        _body(nc, tc, lisT_d, lisN_d, decT_d, psiT_d, phiT_d, psib_d, phib_d,
              att_d, ctx_d, flg_d)
    nc.compile()
    return nc


def _body(nc, tc, lisT_d, lisN_d, decT_d, psiT_d, phiT_d, psib_d, phib_d,
          att_d, ctx_d, flg_d):
    with (
        tc.tile_pool(name="wp", bufs=1) as wp,
        tc.tile_pool(name="lt_pool", bufs=4) as lt_pool,
        tc.tile_pool(name="ln_pool", bufs=6) as ln_pool,
        tc.tile_pool(name="cl_pool", bufs=2) as cl_pool,
        tc.tile_pool(name="big", bufs=1) as big,
        tc.tile_pool(name="st_pool", bufs=2) as st_pool,
        tc.tile_pool(name="col", bufs=1) as col,
        tc.tile_pool(name="pp", bufs=3, space="PSUM") as pp,
        tc.tile_pool(name="ep", bufs=2, space="PSUM") as ep,
        tc.tile_pool(name="dram", bufs=1, space="DRAM") as dram,
    ):
        # ---------------- constants / weights ----------------
        psiT = wp.tile([128, NDT, M], BF16)
        nc.sync.dma_start(out=psiT[:], in_=psiT_d[:].rearrange("(dt p) m -> p dt m", p=128))
        phiT = wp.tile([128, NDT, M], BF16)
        nc.sync.dma_start(out=phiT[:], in_=phiT_d[:].rearrange("(dt p) m -> p dt m", p=128))
        decT = wp.tile([128, NDT, BC], BF16)
        nc.sync.dma_start(out=decT[:], in_=decT_d[:].rearrange("(dt p) b -> p dt b", p=128))
        psib = wp.tile([128, NMT], F32)
        nc.sync.dma_start(out=psib[:], in_=psib_d[:].rearrange("(mt p) -> p mt", p=128))
        phib = wp.tile([128, NMT], F32)
        nc.sync.dma_start(out=phib[:], in_=phib_d[:].rearrange("(mt p) -> p mt", p=128))

        ident = wp.tile([128, 128], F32)
        make_identity(nc, ident)
        ones8 = wp.tile([BC, 1], F32)
        nc.vector.memset(ones8, 1.0)
        ones16 = wp.tile([128, TP], F32)
        nc.vector.memset(ones16, 1.0)
        nan1 = wp.tile([128, 1], F32)
        nc.vector.memset(nan1, float("nan"))
        zero1 = wp.tile([128, 1], F32)
        nc.vector.memset(zero1, 0.0)

        # ---------------- comp_dec = relu(dec @ phi_w.T + phi_b) ----------------
        comp_decT = wp.tile([128, NMT, BC], BF16)   # [m-part, mt, b]
        for mt in range(NMT):
            ps_dec = pp.tile([128, BC], F32, tag="ps_small")
            for dt in range(NDT):
                nc.tensor.matmul(
                    ps_dec[:],
                    phiT[:, dt, ts(mt, 128)],
                    decT[:, dt, :],
                    start=(dt == 0),
                    stop=(dt == NDT - 1),
                )
            nc.scalar.activation(out=comp_decT[:, mt, :], in_=ps_dec[:],
                                 func=AF.Relu, bias=phib[:, mt:mt + 1], scale=1.0)

        # ---------------- main loop: comp_lis + energy ----------------
        e_sb = big.tile([BC, T], F32)
        for b in range(BC):
            for tci in range(NTC):
                lt = lt_pool.tile([128, NDT, TC], BF16)
                nc.sync.dma_start(
                    out=lt[:],
                    in_=lisT_d[b].rearrange("(dt p) t -> p dt t", p=128)[:, :, ts(tci, TC)],
                )
                clis = cl_pool.tile([128, NMT, TC], BF16)
                for mt in range(NMT):
                    ps_l = pp.tile([128, TC], F32, tag="ps_mm1")
                    for dt in range(NDT):
                        nc.tensor.matmul(
                            ps_l[:],
                            psiT[:, dt, ts(mt, 128)],
                            lt[:, dt, :],
                            start=(dt == 0),
                            stop=(dt == NDT - 1),
                        )
                    if mt % 2 == 0:
                        nc.scalar.activation(out=clis[:, mt, :], in_=ps_l[:],
                                             func=AF.Relu, bias=psib[:, mt:mt + 1], scale=1.0)
                    else:
                        nc.vector.tensor_scalar(clis[:, mt, :], ps_l[:],
                                                psib[:, mt:mt + 1], 0.0, OP.add, OP.max)
                ps_e = ep.tile([1, TC], F32, tag="eps")
                for mt in range(NMT):
                    nc.tensor.matmul(
                        ps_e[:],
                        comp_decT[:, mt, b:b + 1],
                        clis[:, mt, :],
                        start=(mt == 0),
                        stop=(mt == NMT - 1),
                    )
                # engine accesses must start at a 32-aligned partition: copy
                # the M=1 PSUM row to an SBUF stage at partition 0, then
                # DMA-scatter into row b
                e_stage = st_pool.tile([1, TC], F32, name="e_stage")
                if (b + tci) % 2 == 0:
                    nc.scalar.copy(out=e_stage[:], in_=ps_e[0:1, :])
                else:
                    nc.vector.tensor_copy(out=e_stage[:], in_=ps_e[0:1, :])
                nc.sync.dma_start(out=e_sb[b:b + 1, ts(tci, TC)], in_=e_stage[:])

        # ---------------- softmax helper ----------------
        def softmax_rows(dst, srcv, tag):
            rmax = wp.tile([BC, 1], F32, name=f"rmax_{tag}")
            nc.vector.reduce_max(rmax[:], srcv[:], axis=AxisListType.X)
            nc.vector.tensor_scalar(rmax[:], rmax[:], -1.0, None, OP.mult)
            nc.vector.tensor_scalar(dst[:], srcv[:], rmax[:], CLAMP_LO, OP.add, OP.max)
            nc.scalar.activation(out=dst[:], in_=dst[:], func=AF.Exp)
            rsum = wp.tile([BC, 1], F32, name=f"rsum_{tag}")
            nc.vector.reduce_sum(rsum[:], dst[:], axis=AxisListType.X)
            nc.vector.reciprocal(rsum[:], rsum[:])
            nc.vector.tensor_scalar(dst[:], dst[:], rsum[:], None, OP.mult)

        # ---------------- batch stats + AllReduce ----------------
        cc_in = dram.tile([3, T], F32)
        cc_out = dram.tile([3, T], F32)
        scr = dram.tile([8, T], F32)   # scratch rows: shifted-ss + coef rows + uo

        sq = big.tile([BC, T], F32, tag="shared_c")
        nc.vector.tensor_mul(sq[:], e_sb[:], e_sb[:])
        prod = big.tile([BC, T], F32)
        nc.vector.memset(prod[:, T - 1:T], 0.0)
        nc.vector.tensor_mul(prod[:, 0:T - 1], e_sb[:, 0:T - 1], e_sb[:, 1:T])

        for k, src in enumerate((e_sb, sq, prod)):
            st_row = st_pool.tile([1, T], F32)
            for tci in range(NTC):
                ps_s = ep.tile([1, TC], F32, tag="eps", name="ps_stat")
                nc.tensor.matmul(
                    ps_s[:],
                    ones8[:],
                    src[0:BC, ts(tci, TC)],
                    start=True,
                    stop=True,
                )
                if (k + tci) % 2 == 0:
                    nc.scalar.copy(out=st_row[:, ts(tci, TC)], in_=ps_s[0:1, :])
                else:
                    nc.vector.tensor_copy(out=st_row[:, ts(tci, TC)], in_=ps_s[0:1, :])
            nc.sync.dma_start(out=cc_in[k:k + 1, :], in_=st_row[:])

        nc.gpsimd.collective_compute(
            "AllReduce",
            OP.add,
            replica_groups=[list(range(NCORES))],
            ins=[cc_in.opt()],
            outs=[cc_out.opt()],
        )

        # ---------------- local-only work overlapping the AllReduce ------------
        # marginal = norm.pdf(e) with exact zeros where it underflows in fp32
        marg = big.tile([BC, T], F32, tag="shared_b")
        nc.vector.tensor_scalar(marg[:], sq[:], -0.5, CLAMP_LO, OP.mult, OP.max)
        nc.scalar.activation(out=marg[:], in_=marg[:], func=AF.Exp)
        mz = big.tile([BC, T], F32)
        nc.gpsimd.tensor_scalar(mz[:], e_sb[:], E_BIG, None, OP.is_le)
        nc.vector.tensor_mul(marg[:], marg[:], mz[:])
        sqs = big.tile([BC, T - 2], F32)
        nc.gpsimd.tensor_add(sqs[:], sq[:, 0:T - 2], sq[:, 1:T - 1])   # x1^2 + x2^2
        sme = big.tile([BC, T], F32)
        softmax_rows(sme, e_sb, "eng")   # plain-softmax fallback branch

        # ---------------- column stats in [128, 16] layout ----------------
        cc_flat = cc_out.rearrange("a b -> (a b)")
        gs1 = col.tile([128, TP], F32)
        nc.sync.dma_start(out=gs1[:], in_=_col_ap(cc_flat[0:T]))
        gs1S = col.tile([128, TP], F32)
        nc.sync.dma_start(out=gs1S[:], in_=_col_ap(cc_flat[1:T + 1]))
        gs2 = col.tile([128, TP], F32)
        nc.sync.dma_start(out=gs2[:], in_=_col_ap(cc_flat[T:2 * T]))
        gs12 = col.tile([128, TP], F32)
        nc.sync.dma_start(out=gs12[:], in_=_col_ap(cc_flat[2 * T:3 * T]))

        m2 = col.tile([128, TP], F32)
        nc.vector.scalar_tensor_tensor(m2[:], gs1[:], BINV, gs1[:], OP.mult, OP.mult)
        ss = col.tile([128, TP], F32)
        nc.vector.tensor_sub(ss[:], gs2[:], m2[:])
        scr_flat = scr.rearrange("a b -> (a b)")
        nc.sync.dma_start(out=_col_ap(scr_flat[0:T]), in_=ss[:])
        ssS = col.tile([128, TP], F32)
        nc.sync.dma_start(out=ssS[:], in_=_col_ap(scr_flat[1:T + 1]))

        sprod = col.tile([128, TP], F32)
        nc.vector.scalar_tensor_tensor(sprod[:], gs1[:], BINV, gs1S[:], OP.mult, OP.mult)
        num = col.tile([128, TP], F32)
        nc.vector.tensor_sub(num[:], gs12[:], sprod[:])
        d2 = col.tile([128, TP], F32)
        nc.vector.tensor_mul(d2[:], ss[:], ssS[:])
        rsq = col.tile([128, TP], F32)
        nc.scalar.activation(out=rsq[:], in_=d2[:], func=AF.Sqrt)
        nc.vector.reciprocal(rsq[:], rsq[:])
        rv = col.tile([128, TP], F32)
        nc.vector.tensor_mul(rv[:], num[:], rsq[:])
        rr2 = col.tile([128, TP], F32)
        nc.vector.tensor_mul(rr2[:], rv[:], rv[:])
        det = col.tile([128, TP], F32)
        nc.vector.tensor_scalar(det[:], rr2[:], -1.0, 1.0, OP.mult, OP.add)
        degen = col.tile([128, TP], F32)
        nc.vector.tensor_scalar(degen[:], det[:], 0.01, None, OP.is_lt)
        ndeg = col.tile([128, TP], F32)
        nc.vector.tensor_scalar(ndeg[:], degen[:], -1.0, 1.0, OP.mult, OP.add)
        # safe_det = det where det >= 0.01 else 1.0 (masks are exactly 0/1)
        sd = col.tile([128, TP], F32)
        nc.vector.tensor_mul(sd[:], det[:], ndeg[:])
        nc.vector.tensor_add(sd[:], sd[:], degen[:])
        invsd = col.tile([128, TP], F32)
        nc.vector.reciprocal(invsd[:], sd[:])
        c1 = col.tile([128, TP], F32)
        nc.vector.tensor_mul(c1[:], rv[:], invsd[:])
        c2 = col.tile([128, TP], F32)
        nc.vector.scalar_tensor_tensor(c2[:], c1[:], 0.5, rv[:], OP.mult, OP.mult)
        ccoef = col.tile([128, TP], F32)
        nc.scalar.activation(out=ccoef[:], in_=sd[:], func=AF.Sqrt, scale=TWO_PI)
        nc.vector.reciprocal(ccoef[:], ccoef[:])

        # coef rows -> DRAM -> [BC, T-2] broadcasts
        for k, srct in enumerate((c1, c2, ccoef, ndeg)):
            nc.sync.dma_start(out=_col_ap(scr_flat[(k + 1) * T:(k + 2) * T]), in_=srct[:])
        c1_8 = big.tile([BC, T - 2], F32)
        nc.gpsimd.dma_start(out=c1_8[:], in_=scr[1:2, 0:T - 2].to_broadcast([BC, T - 2]))
        c2_8 = big.tile([BC, T - 2], F32)
        nc.gpsimd.dma_start(out=c2_8[:], in_=scr[2:3, 0:T - 2].to_broadcast([BC, T - 2]))
        ccoef8 = big.tile([BC, T - 2], F32)
        nc.gpsimd.dma_start(out=ccoef8[:], in_=scr[3:4, 0:T - 2].to_broadcast([BC, T - 2]))
        ndeg8 = big.tile([BC, T - 2], F32)
        nc.gpsimd.dma_start(out=ndeg8[:], in_=scr[4:5, 0:T - 2].to_broadcast([BC, T - 2]))

        # ---------------- fallback flag: any all-zero energy column ------------
        gs2row = big.tile([1, T], F32, tag="shared_a")
        nc.sync.dma_start(out=gs2row[:], in_=cc_out[1:2, :])
        nc.vector.tensor_scalar(gs2row[:], gs2row[:], 0.0, None, OP.is_le)
        uo1 = wp.tile([1, 1], F32)
        nc.vector.reduce_max(uo1[:], gs2row[:], axis=AxisListType.X)
        nc.sync.dma_start(out=flg_d[:], in_=uo1[:])
        nc.sync.dma_start(out=scr[5:6, 0:1], in_=uo1[:])
        uo8 = wp.tile([BC, 1], F32)
        nc.gpsimd.dma_start(out=uo8[:], in_=scr[5:6, 0:1].to_broadcast([BC, 1]))

        # ---------------- copula pdf grid [BC, T-2] ----------------
        # att column i in 1..T-2 uses x1 = e[:, i-1], x2 = e[:, i], r = rr[i-1].
        # Column-split every elementwise op: DVE takes 2/3, GpSimd (half DVE's
        # rate) takes 1/3, so both halves finish together.
        NT2 = T - 2
        SPL = 1364

        def split2(fn, n=NT2):
            fn(nc.vector, slice(0, SPL))
            fn(nc.gpsimd, slice(SPL, n))

        x2 = e_sb[:, 1:T - 1]
        tA = mz[:, 0:T - 2]                  # mz is free after the marg mask

        expo = big.tile([BC, T - 2], F32)
        split2(lambda g, s: g.tensor_mul(expo[:, s], prod[:, 0:T - 2][:, s], c1_8[:, s]))
        split2(lambda g, s: g.tensor_mul(tA[:, s], sqs[:, s], c2_8[:, s]))
        split2(lambda g, s: g.tensor_sub(expo[:, s], expo[:, s], tA[:, s]))

        # NaN positions: fp32 exp(expo) overflows AND marginal underflowed to 0,
        # in a non-degenerate column.
        nc.vector.tensor_scalar(tA, expo[:], EXP_OVF, None, OP.is_gt)
        nc.vector.scalar_tensor_tensor(tA, x2, E_BIG, tA, OP.is_gt, OP.mult)
        split2(lambda g, s: g.tensor_mul(tA[:, s], tA[:, s], ndeg8[:, s]))
        nan_row = wp.tile([BC, 1], F32)
        nc.vector.reduce_max(nan_row[:], tA, axis=AxisListType.X)

        # cop (finite branch) and energy_f1
        nc.vector.tensor_scalar(expo[:], expo[:], CLAMP_HI, CLAMP_LO, OP.min, OP.max)
        nc.scalar.activation(out=expo[:], in_=expo[:], func=AF.Exp)
        ef1 = big.tile([BC, T], F32, tag="shared_a")
        ef1c = ef1[:, 1:T - 1]
        mgc = marg[:, 1:T - 1]
        split2(lambda g, s: g.tensor_mul(ef1c[:, s], mgc[:, s], expo[:, s]))
        split2(lambda g, s: g.tensor_mul(ef1c[:, s], ef1c[:, s], ccoef8[:, s]))
        # degenerate columns: cop = 10.0; exact 0/1 blend
        split2(lambda g, s: g.tensor_mul(ef1c[:, s], ef1c[:, s], ndeg8[:, s]))
        nc.vector.scalar_tensor_tensor(tA, ndeg8[:], -10.0, mgc, OP.mult, OP.mult)
        nc.vector.scalar_tensor_tensor(tA, mgc, 10.0, tA, OP.mult, OP.add)
        split2(lambda g, s: g.tensor_add(ef1c[:, s], ef1c[:, s], tA[:, s]))
        nc.vector.tensor_copy(out=ef1[:, 0:1], in_=marg[:, 0:1])
        nc.vector.tensor_copy(out=ef1[:, T - 1:T], in_=marg[:, T - 1:T])

        acs = big.tile([BC, T], F32, tag="shared_c")   # copula softmax
        softmax_rows(acs, ef1, "cop")

        # blend: att_safe = uo * softmax(e) + (1-uo) * copula_score
        uo8inv = wp.tile([BC, 1], F32)
        nc.vector.tensor_scalar(uo8inv[:], uo8[:], -1.0, 1.0, OP.mult, OP.add)
        nc.vector.tensor_scalar(acs[:], acs[:], uo8inv[:], None, OP.mult)
        nc.vector.tensor_scalar(sme[:], sme[:], uo8[:], None, OP.mult)
        split2(lambda g, s: g.tensor_add(acs[:, s], acs[:, s], sme[:, s]), n=T)

        # NaN rows (copula branch only)
        mask2 = wp.tile([BC, 1], F32)
        nc.vector.tensor_mul(mask2[:], nan_row[:], uo8inv[:])
        mask2u = wp.tile([BC, 1], mybir.dt.uint8)
        nc.vector.tensor_copy(out=mask2u[:], in_=mask2[:])
        nan_col = wp.tile([BC, 1], F32)
        nc.vector.select(nan_col[:], mask2u[:], nan1[0:BC, :], zero1[0:BC, :])

        att_out = big.tile([BC, T], F32, tag="shared_b")
        nc.vector.tensor_scalar(att_out[:], acs[:], nan_col[:], None, OP.add)
        nc.sync.dma_start(out=att_d[:], in_=att_out[:])

        # ---------------- context = sum_t att[b,t] * lis[b,t,:] ----------------
        attT = wp.tile([128, NCH, BC], FP8)
        for ch in range(NCH):
            ps_t = pp.tile([128, BC], F32, tag="ps_small", name="ps_tp")
            nc.tensor.transpose(ps_t[:, 0:BC], acs[0:BC, ts(ch, 128)], ident[0:BC, 0:BC])
            nc.scalar.activation(out=attT[:, ch, :], in_=ps_t[:, 0:BC],
                                 func=AF.Copy, scale=ATT_SCALE)

        ctx_sb = big.tile([BC, D], F32)
        for b in range(BC):
            ps_c = pp.tile([1, D], F32, tag="ps_small", name="ps_ctx")
            for q in range(NCH // 4):
                ln = ln_pool.tile([128, 4, D], FP8)
                nc.sync.dma_start(
                    out=ln[:],
                    in_=lisN_d[b].rearrange("(ch p) d -> p ch d", p=128)[:, 4 * q:4 * q + 4, :],
                )
                for j in range(4):
                    ch = 4 * q + j
                    nc.tensor.matmul(
                        ps_c[:],
                        attT[:, ch, b:b + 1],
                        ln[:, j, :],
                        start=(ch == 0),
                        stop=(ch == NCH - 1),
                    )
            c_stage = st_pool.tile([1, D], F32, name="c_stage")
            if b % 2 == 0:
                nc.scalar.activation(out=c_stage[:], in_=ps_c[0:1, :],
                                     func=AF.Copy, scale=1.0 / ATT_SCALE)
            else:
                nc.vector.tensor_scalar(c_stage[:], ps_c[0:1, :], 1.0 / ATT_SCALE,
                                        None, OP.mult)
            nc.sync.dma_start(out=ctx_sb[b:b + 1, :], in_=c_stage[:])

        nc.vector.tensor_scalar(ctx_sb[:], ctx_sb[:], nan_col[:], None, OP.add)
        nc.sync.dma_start(out=ctx_d[:], in_=ctx_sb[:])


_NC_CACHE = {}
LAST_RESULTS = None


def _install_trace_shim():
    """The agent container's antenv stub lacks axon_hooks; register the NTFF
    profile hook ourselves so run_bass_kernel_spmd(trace=True) works."""
    import sys
    import types

    try:
        from antenv.axon_hooks import get_axon_ntff_profile_hook  # noqa: F401
    except ImportError:
        import antenv

        mod = types.ModuleType("antenv.axon_hooks")
        mod._hook = None
        mod.set_axon_ntff_profile_hook = lambda h: setattr(mod, "_hook", h)
        mod.get_axon_ntff_profile_hook = lambda: mod._hook
        sys.modules["antenv.axon_hooks"] = mod
        antenv.axon_hooks = mod
        try:
            from trn_agent_boot.trn_boot import _ntff_profile_via_ctypes
            mod._hook = _ntff_profile_via_ctypes("/opt/axon/libaxon_pjrt.so")
        except Exception:
            pass
    import concourse.bass_utils as bu
    bu.upload_artifacts = lambda tmpdir: tmpdir


def _get_nc():
    if "nc" not in _NC_CACHE:
        _NC_CACHE["nc"] = _build_nc()
    return _NC_CACHE["nc"]


def kernel(decoder_state, listener_feature, phi_w, phi_b, psi_w, psi_b):
    global LAST_RESULTS
    bf16 = ml_dtypes.bfloat16

    dec = np.asarray(decoder_state, np.float32)[:, 0, :]          # [B, D]
    lis_f32 = np.asarray(listener_feature, np.float32)            # [B, T, D]
    lis_bf = lis_f32.astype(bf16)
    lis_f8 = lis_f32.astype(ml_dtypes.float8_e4m3)
    psiT = np.ascontiguousarray(np.asarray(psi_w, np.float32).T).astype(bf16)
    phiT = np.ascontiguousarray(np.asarray(phi_w, np.float32).T).astype(bf16)
    psib = np.asarray(psi_b, np.float32)
    phib = np.asarray(phi_b, np.float32)

    in_maps = []
    for c in range(NCORES):
        bs = slice(c * BC, (c + 1) * BC)
        in_maps.append({
            "lis_t": np.ascontiguousarray(lis_bf[bs].transpose(0, 2, 1)),
            "lis_n": np.ascontiguousarray(lis_f8[bs]),
            "dec_t": np.ascontiguousarray(dec[bs].T).astype(bf16),
            "psi_t": psiT,
            "phi_t": phiT,
            "psi_bias": psib,
            "phi_bias": phib,
        })

    trace = bool(os.environ.get("KERNEL_TRACE"))
    if trace:
        _install_trace_shim()
    nc = _get_nc()
    res = run_bass_kernel_spmd(
        nc,
        in_maps,
        core_ids=list(range(NCORES)),
        trace=trace,
    )
    LAST_RESULTS = res

    att = np.concatenate([res.results[c]["att"] for c in range(NCORES)], axis=0)
    ctx = np.concatenate([res.results[c]["ctx"] for c in range(NCORES)], axis=0)
    flag = float(res.results[0]["flg"][0, 0])
    count_original = np.int32(1 if flag > 0.5 else 0)
    count_copula = np.int32(1 - count_original)
    return att, ctx, count_original, count_copula


# revision 41
# speedup vs baseline: 1.1497x; 1.0569x over previous
"""Trainium2 Bass kernel for nn_Attention_37074157699663.

Copula attention: MLP preprocess (phi/psi + ReLU), energy = comp_dec . comp_lis,
adjacent-column Pearson correlation over the batch, bivariate Gaussian copula
pdf, softmax, context. Data-parallel over batch across 8 NeuronCores; the three
per-column stat vectors (sum e, sum e^2, sum e[t]e[t+1]) are AllReduced.

The reference computes norm.pdf(e) (underflows fp32 to 0) times exp(copula
exponent) (overflows fp32 to inf), so reference rows are NaN wherever any
column overflows. We reproduce that exactly by computing the overflow mask
explicitly (expo > ln(float32 max)) instead of trusting device exp() inf
semantics, then injecting NaN per row.
"""

import os

import numpy as np
import ml_dtypes

import concourse.bacc as bacc
import concourse.bass as bass
import concourse.tile as tile
from concourse import mybir
from concourse.bass_utils import run_bass_kernel_spmd
from concourse.masks import make_identity
from bass_rust import AxisListType

BF16 = mybir.dt.bfloat16
FP8 = mybir.dt.float8e4
F32 = mybir.dt.float32
ATT_SCALE = 128.0   # att probs (~5e-4) scaled into fp8e4m3's normal range
AF = mybir.ActivationFunctionType
OP = mybir.AluOpType
ts = bass.ts

B, T, D, M = 64, 2048, 512, 512
NCORES = 8
BC = B // NCORES        # 8 batch rows per core
TC = 512                # t-chunk for MM1/MM2 (one PSUM bank)
NTC = T // TC           # 4
NDT = D // 128          # 4 contraction tiles
NMT = M // 128          # 4 m tiles
NCH = T // 128          # 16 chunks for the context matmul
TP = T // 128           # 16: free size of [128, TP] column-stat tiles

INV_SQRT_2PI = 0.3989422804014327
TWO_PI = 6.283185307179586   # Rsqrt(2*pi*sd) == INV_SQRT_2PI * rsqrt(sd)
EXP_OVF = 88.722839          # ln(float32 max): fp32 exp(x) == inf for x above this
E_BIG = 20.0                 # e > 20  =>  INV_SQRT_2PI * exp(-e^2/2) is exactly 0 in fp32
CLAMP_LO = -103.0
CLAMP_HI = 88.0
BINV = 1.0 / B


def _col_ap(flat_ap):
    """[2048]-flat AP -> [128, 16] (t = p*16 + f)."""
    return flat_ap.rearrange("(p f) -> p f", p=128)


def _build_nc():
    nc = bacc.Bacc("TRN2", target_bir_lowering=False)

    lisT_d = nc.dram_tensor("lis_t", [BC, D, T], BF16, kind="ExternalInput")
    lisN_d = nc.dram_tensor("lis_n", [BC, T, D], FP8, kind="ExternalInput")
    decT_d = nc.dram_tensor("dec_t", [D, BC], BF16, kind="ExternalInput")
    psiT_d = nc.dram_tensor("psi_t", [D, M], BF16, kind="ExternalInput")
    phiT_d = nc.dram_tensor("phi_t", [D, M], BF16, kind="ExternalInput")
    psib_d = nc.dram_tensor("psi_bias", [M], F32, kind="ExternalInput")
    phib_d = nc.dram_tensor("phi_bias", [M], F32, kind="ExternalInput")

    att_d = nc.dram_tensor("att", [BC, T], F32, kind="ExternalOutput")
    ctx_d = nc.dram_tensor("ctx", [BC, D], F32, kind="ExternalOutput")
    flg_d = nc.dram_tensor("flg", [1, 1], F32, kind="ExternalOutput")

    with tile.TileContext(nc) as tc:
        _body(nc, tc, lisT_d, lisN_d, decT_d, psiT_d, phiT_d, psib_d, phib_d,
              att_d, ctx_d, flg_d)
    nc.compile()
    return nc


def _body(nc, tc, lisT_d, lisN_d, decT_d, psiT_d, phiT_d, psib_d, phib_d,
          att_d, ctx_d, flg_d):
    with (
        tc.tile_pool(name="wp", bufs=1) as wp,
        tc.tile_pool(name="lt_pool", bufs=4) as lt_pool,
        tc.tile_pool(name="ln_pool", bufs=6) as ln_pool,
        tc.tile_pool(name="cl_pool", bufs=2) as cl_pool,
        tc.tile_pool(name="big", bufs=1) as big,
        tc.tile_pool(name="st_pool", bufs=2) as st_pool,
        tc.tile_pool(name="col", bufs=1) as col,
        tc.tile_pool(name="pp", bufs=3, space="PSUM") as pp,
        tc.tile_pool(name="ep", bufs=2, space="PSUM") as ep,
        tc.tile_pool(name="dram", bufs=1, space="DRAM") as dram,
    ):
        # ---------------- constants / weights ----------------
        psiT = wp.tile([128, NDT, M], BF16)
        nc.sync.dma_start(out=psiT[:], in_=psiT_d[:].rearrange("(dt p) m -> p dt m", p=128))
        phiT = wp.tile([128, NDT, M], BF16)
        nc.sync.dma_start(out=phiT[:], in_=phiT_d[:].rearrange("(dt p) m -> p dt m", p=128))
        decT = wp.tile([128, NDT, BC], BF16)
        nc.sync.dma_start(out=decT[:], in_=decT_d[:].rearrange("(dt p) b -> p dt b", p=128))
        psib = wp.tile([128, NMT], F32)
        nc.sync.dma_start(out=psib[:], in_=psib_d[:].rearrange("(mt p) -> p mt", p=128))
        phib = wp.tile([128, NMT], F32)
        nc.sync.dma_start(out=phib[:], in_=phib_d[:].rearrange("(mt p) -> p mt", p=128))

        ident = wp.tile([128, 128], F32)
        make_identity(nc, ident)
        ones8 = wp.tile([BC, 1], F32)
        nc.vector.memset(ones8, 1.0)
        ones16 = wp.tile([128, TP], F32)
        nc.vector.memset(ones16, 1.0)
        nan1 = wp.tile([128, 1], F32)
        nc.vector.memset(nan1, float("nan"))
        zero1 = wp.tile([128, 1], F32)
        nc.vector.memset(zero1, 0.0)

        # ---------------- comp_dec = relu(dec @ phi_w.T + phi_b) ----------------
        comp_decT = wp.tile([128, NMT, BC], BF16)   # [m-part, mt, b]
        for mt in range(NMT):
            ps_dec = pp.tile([128, BC], F32, tag="ps_small")
            for dt in range(NDT):
                nc.tensor.matmul(
                    ps_dec[:],
                    phiT[:, dt, ts(mt, 128)],
                    decT[:, dt, :],
                    start=(dt == 0),
                    stop=(dt == NDT - 1),
                )
            nc.scalar.activation(out=comp_decT[:, mt, :], in_=ps_dec[:],
                                 func=AF.Relu, bias=phib[:, mt:mt + 1], scale=1.0)

        # ---------------- main loop: comp_lis + energy ----------------
        e_sb = big.tile([BC, T], F32)
        for b in range(BC):
            for tci in range(NTC):
                lt = lt_pool.tile([128, NDT, TC], BF16)
                nc.sync.dma_start(
                    out=lt[:],
                    in_=lisT_d[b].rearrange("(dt p) t -> p dt t", p=128)[:, :, ts(tci, TC)],
                )
                clis = cl_pool.tile([128, NMT, TC], BF16)
                for mt in range(NMT):
                    ps_l = pp.tile([128, TC], F32, tag="ps_mm1")
                    for dt in range(NDT):
                        nc.tensor.matmul(
                            ps_l[:],
                            psiT[:, dt, ts(mt, 128)],
                            lt[:, dt, :],
                            start=(dt == 0),
                            stop=(dt == NDT - 1),
                        )
                    if mt % 2 == 0:
                        nc.scalar.activation(out=clis[:, mt, :], in_=ps_l[:],
                                             func=AF.Relu, bias=psib[:, mt:mt + 1], scale=1.0)
                    else:
                        nc.vector.tensor_scalar(clis[:, mt, :], ps_l[:],
                                                psib[:, mt:mt + 1], 0.0, OP.add, OP.max)
                ps_e = ep.tile([1, TC], F32, tag="eps")
                for mt in range(NMT):
                    nc.tensor.matmul(
                        ps_e[:],
                        comp_decT[:, mt, b:b + 1],
                        clis[:, mt, :],
                        start=(mt == 0),
                        stop=(mt == NMT - 1),
                    )
                # engine accesses must start at a 32-aligned partition: copy
                # the M=1 PSUM row to an SBUF stage at partition 0, then
                # DMA-scatter into row b
                e_stage = st_pool.tile([1, TC], F32, name="e_stage")
                if (b + tci) % 2 == 0:
                    nc.scalar.copy(out=e_stage[:], in_=ps_e[0:1, :])
                else:
                    nc.vector.tensor_copy(out=e_stage[:], in_=ps_e[0:1, :])
                nc.sync.dma_start(out=e_sb[b:b + 1, ts(tci, TC)], in_=e_stage[:])

        # ---------------- softmax helper ----------------
        def softmax_rows(dst, srcv, tag):
            rmax = wp.tile([BC, 1], F32, name=f"rmax_{tag}")
            nc.vector.reduce_max(rmax[:], srcv[:], axis=AxisListType.X)
            nc.vector.tensor_scalar(rmax[:], rmax[:], -1.0, None, OP.mult)
            nc.vector.tensor_scalar(dst[:], srcv[:], rmax[:], CLAMP_LO, OP.add, OP.max)
            nc.scalar.activation(out=dst[:], in_=dst[:], func=AF.Exp)
            rsum = wp.tile([BC, 1], F32, name=f"rsum_{tag}")
            nc.vector.reduce_sum(rsum[:], dst[:], axis=AxisListType.X)
            nc.vector.reciprocal(rsum[:], rsum[:])
            nc.vector.tensor_scalar(dst[:], dst[:], rsum[:], None, OP.mult)

        # ---------------- batch stats + AllReduce ----------------
        cc_in = dram.tile([3, T], F32)
        cc_out = dram.tile([3, T], F32)
        scr = dram.tile([8, T], F32)   # scratch rows: shifted-ss + coef rows + uo

        sq = big.tile([BC, T], F32, tag="shared_c")
        nc.vector.tensor_mul(sq[:], e_sb[:], e_sb[:])
        prod = big.tile([BC, T], F32)
        nc.vector.memset(prod[:, T - 1:T], 0.0)
        nc.vector.tensor_mul(prod[:, 0:T - 1], e_sb[:, 0:T - 1], e_sb[:, 1:T])

        for k, src in enumerate((e_sb, sq, prod)):
            st_row = st_pool.tile([1, T], F32)
            for tci in range(NTC):
                ps_s = ep.tile([1, TC], F32, tag="eps", name="ps_stat")
                nc.tensor.matmul(
                    ps_s[:],
                    ones8[:],
                    src[0:BC, ts(tci, TC)],
                    start=True,
                    stop=True,
                )
                if (k + tci) % 2 == 0:
                    nc.scalar.copy(out=st_row[:, ts(tci, TC)], in_=ps_s[0:1, :])
                else:
                    nc.vector.tensor_copy(out=st_row[:, ts(tci, TC)], in_=ps_s[0:1, :])
            nc.sync.dma_start(out=cc_in[k:k + 1, :], in_=st_row[:])

        nc.gpsimd.collective_compute(
            "AllReduce",
            OP.add,
            replica_groups=[list(range(NCORES))],
            ins=[cc_in.opt()],
            outs=[cc_out.opt()],
        )

        # ---------------- local-only work overlapping the AllReduce ------------
        # marginal = norm.pdf(e) with exact zeros where it underflows in fp32
        marg = big.tile([BC, T], F32, tag="shared_b")
        nc.vector.tensor_scalar(marg[:], sq[:], -0.5, CLAMP_LO, OP.mult, OP.max)
        nc.scalar.activation(out=marg[:], in_=marg[:], func=AF.Exp)
        mz = big.tile([BC, T], F32)
        nc.vector.tensor_scalar(mz[:], e_sb[:], E_BIG, None, OP.is_le)
        nc.vector.tensor_mul(marg[:], marg[:], mz[:])
        sqs = big.tile([BC, T - 2], F32)
        nc.vector.tensor_add(sqs[:], sq[:, 0:T - 2], sq[:, 1:T - 1])   # x1^2 + x2^2
        sme = big.tile([BC, T], F32)
        softmax_rows(sme, e_sb, "eng")   # plain-softmax fallback branch

        # ---------------- column stats in [128, 16] layout ----------------
        cc_flat = cc_out.rearrange("a b -> (a b)")
        gs1 = col.tile([128, TP], F32)
        nc.sync.dma_start(out=gs1[:], in_=_col_ap(cc_flat[0:T]))
        gs1S = col.tile([128, TP], F32)
        nc.sync.dma_start(out=gs1S[:], in_=_col_ap(cc_flat[1:T + 1]))
        gs2 = col.tile([128, TP], F32)
        nc.sync.dma_start(out=gs2[:], in_=_col_ap(cc_flat[T:2 * T]))
        gs12 = col.tile([128, TP], F32)
        nc.sync.dma_start(out=gs12[:], in_=_col_ap(cc_flat[2 * T:3 * T]))

        m2 = col.tile([128, TP], F32)
        nc.vector.scalar_tensor_tensor(m2[:], gs1[:], BINV, gs1[:], OP.mult, OP.mult)
        ss = col.tile([128, TP], F32)
        nc.vector.tensor_sub(ss[:], gs2[:], m2[:])
        scr_flat = scr.rearrange("a b -> (a b)")
        nc.sync.dma_start(out=_col_ap(scr_flat[0:T]), in_=ss[:])
        ssS = col.tile([128, TP], F32)
        nc.sync.dma_start(out=ssS[:], in_=_col_ap(scr_flat[1:T + 1]))

        sprod = col.tile([128, TP], F32)
        nc.vector.scalar_tensor_tensor(sprod[:], gs1[:], BINV, gs1S[:], OP.mult, OP.mult)
        num = col.tile([128, TP], F32)
        nc.vector.tensor_sub(num[:], gs12[:], sprod[:])
        d2 = col.tile([128, TP], F32)
        nc.vector.tensor_mul(d2[:], ss[:], ssS[:])
        rsq = col.tile([128, TP], F32)
        nc.scalar.activation(out=rsq[:], in_=d2[:], func=AF.Sqrt)
        nc.vector.reciprocal(rsq[:], rsq[:])
        rv = col.tile([128, TP], F32)
        nc.vector.tensor_mul(rv[:], num[:], rsq[:])
        rr2 = col.tile([128, TP], F32)
        nc.vector.tensor_mul(rr2[:], rv[:], rv[:])
        det = col.tile([128, TP], F32)
        nc.vector.tensor_scalar(det[:], rr2[:], -1.0, 1.0, OP.mult, OP.add)
        degen = col.tile([128, TP], F32)
        nc.vector.tensor_scalar(degen[:], det[:], 0.01, None, OP.is_lt)
        ndeg = col.tile([128, TP], F32)
        nc.vector.tensor_scalar(ndeg[:], degen[:], -1.0, 1.0, OP.mult, OP.add)
        # safe_det = det where det >= 0.01 else 1.0 (masks are exactly 0/1)
        sd = col.tile([128, TP], F32)
        nc.vector.tensor_mul(sd[:], det[:], ndeg[:])
        nc.vector.tensor_add(sd[:], sd[:], degen[:])
        invsd = col.tile([128, TP], F32)
        nc.vector.reciprocal(invsd[:], sd[:])
        c1 = col.tile([128, TP], F32)
        nc.vector.tensor_mul(c1[:], rv[:], invsd[:])
        c2 = col.tile([128, TP], F32)
        nc.vector.scalar_tensor_tensor(c2[:], c1[:], 0.5, rv[:], OP.mult, OP.mult)
        ccoef = col.tile([128, TP], F32)
        nc.scalar.activation(out=ccoef[:], in_=sd[:], func=AF.Sqrt, scale=TWO_PI)
        nc.vector.reciprocal(ccoef[:], ccoef[:])

        # coef rows -> DRAM -> [BC, T-2] broadcasts
        for k, srct in enumerate((c1, c2, ccoef, ndeg)):
            nc.sync.dma_start(out=_col_ap(scr_flat[(k + 1) * T:(k + 2) * T]), in_=srct[:])
        c1_8 = big.tile([BC, T - 2], F32)
        nc.gpsimd.dma_start(out=c1_8[:], in_=scr[1:2, 0:T - 2].to_broadcast([BC, T - 2]))
        c2_8 = big.tile([BC, T - 2], F32)
        nc.gpsimd.dma_start(out=c2_8[:], in_=scr[2:3, 0:T - 2].to_broadcast([BC, T - 2]))
        ccoef8 = big.tile([BC, T - 2], F32)
        nc.gpsimd.dma_start(out=ccoef8[:], in_=scr[3:4, 0:T - 2].to_broadcast([BC, T - 2]))
        ndeg8 = big.tile([BC, T - 2], F32)
        nc.gpsimd.dma_start(out=ndeg8[:], in_=scr[4:5, 0:T - 2].to_broadcast([BC, T - 2]))

        # ---------------- fallback flag: any all-zero energy column ------------
        gs2row = big.tile([1, T], F32, tag="shared_a")
        nc.sync.dma_start(out=gs2row[:], in_=cc_out[1:2, :])
        nc.vector.tensor_scalar(gs2row[:], gs2row[:], 0.0, None, OP.is_le)
        uo1 = wp.tile([1, 1], F32)
        nc.vector.reduce_max(uo1[:], gs2row[:], axis=AxisListType.X)
        nc.sync.dma_start(out=flg_d[:], in_=uo1[:])
        nc.sync.dma_start(out=scr[5:6, 0:1], in_=uo1[:])
        uo8 = wp.tile([BC, 1], F32)
        nc.gpsimd.dma_start(out=uo8[:], in_=scr[5:6, 0:1].to_broadcast([BC, 1]))

        # ---------------- copula pdf grid [BC, T-2] ----------------
        # att column i in 1..T-2 uses x1 = e[:, i-1], x2 = e[:, i], r = rr[i-1].
        # Column-split every elementwise op: DVE takes 2/3, GpSimd (half DVE's
        # rate) takes 1/3, so both halves finish together.
        NT2 = T - 2
        SPL = 1364

        def split2(fn, n=NT2):
            fn(nc.vector, slice(0, n))

        x2 = e_sb[:, 1:T - 1]
        tA = mz[:, 0:T - 2]                  # mz is free after the marg mask

        expo = big.tile([BC, T - 2], F32)
        split2(lambda g, s: g.tensor_mul(expo[:, s], prod[:, 0:T - 2][:, s], c1_8[:, s]))
        split2(lambda g, s: g.tensor_mul(tA[:, s], sqs[:, s], c2_8[:, s]))
        split2(lambda g, s: g.tensor_sub(expo[:, s], expo[:, s], tA[:, s]))

        # NaN positions: fp32 exp(expo) overflows AND marginal underflowed to 0,
        # in a non-degenerate column.
        nc.vector.tensor_scalar(tA, expo[:], EXP_OVF, None, OP.is_gt)
        nc.vector.scalar_tensor_tensor(tA, x2, E_BIG, tA, OP.is_gt, OP.mult)
        split2(lambda g, s: g.tensor_mul(tA[:, s], tA[:, s], ndeg8[:, s]))
        nan_row = wp.tile([BC, 1], F32)
        nc.vector.reduce_max(nan_row[:], tA, axis=AxisListType.X)

        # cop (finite branch) and energy_f1
        nc.vector.tensor_scalar(expo[:], expo[:], CLAMP_HI, CLAMP_LO, OP.min, OP.max)
        nc.scalar.activation(out=expo[:], in_=expo[:], func=AF.Exp)
        ef1 = big.tile([BC, T], F32, tag="shared_a")
        ef1c = ef1[:, 1:T - 1]
        mgc = marg[:, 1:T - 1]
        split2(lambda g, s: g.tensor_mul(ef1c[:, s], mgc[:, s], expo[:, s]))
        split2(lambda g, s: g.tensor_mul(ef1c[:, s], ef1c[:, s], ccoef8[:, s]))
        # degenerate columns: cop = 10.0; exact 0/1 blend
        split2(lambda g, s: g.tensor_mul(ef1c[:, s], ef1c[:, s], ndeg8[:, s]))
        nc.vector.scalar_tensor_tensor(tA, ndeg8[:], -10.0, mgc, OP.mult, OP.mult)
        nc.vector.scalar_tensor_tensor(tA, mgc, 10.0, tA, OP.mult, OP.add)
        split2(lambda g, s: g.tensor_add(ef1c[:, s], ef1c[:, s], tA[:, s]))
        nc.vector.tensor_copy(out=ef1[:, 0:1], in_=marg[:, 0:1])
        nc.vector.tensor_copy(out=ef1[:, T - 1:T], in_=marg[:, T - 1:T])

        acs = big.tile([BC, T], F32, tag="shared_c")   # copula softmax
        softmax_rows(acs, ef1, "cop")

        # blend: att_safe = uo * softmax(e) + (1-uo) * copula_score
        uo8inv = wp.tile([BC, 1], F32)
        nc.vector.tensor_scalar(uo8inv[:], uo8[:], -1.0, 1.0, OP.mult, OP.add)
        nc.vector.tensor_scalar(acs[:], acs[:], uo8inv[:], None, OP.mult)
        nc.vector.tensor_scalar(sme[:], sme[:], uo8[:], None, OP.mult)
        split2(lambda g, s: g.tensor_add(acs[:, s], acs[:, s], sme[:, s]), n=T)

        # NaN rows (copula branch only)
        mask2 = wp.tile([BC, 1], F32)
        nc.vector.tensor_mul(mask2[:], nan_row[:], uo8inv[:])
        mask2u = wp.tile([BC, 1], mybir.dt.uint8)
        nc.vector.tensor_copy(out=mask2u[:], in_=mask2[:])
        nan_col = wp.tile([BC, 1], F32)
        nc.vector.select(nan_col[:], mask2u[:], nan1[0:BC, :], zero1[0:BC, :])

        att_out = big.tile([BC, T], F32, tag="shared_b")
        nc.vector.tensor_scalar(att_out[:], acs[:], nan_col[:], None, OP.add)
        nc.sync.dma_start(out=att_d[:], in_=att_out[:])

        # ---------------- context = sum_t att[b,t] * lis[b,t,:] ----------------
        attT = wp.tile([128, NCH, BC], FP8)
        for ch in range(NCH):
            ps_t = pp.tile([128, BC], F32, tag="ps_small", name="ps_tp")
            nc.tensor.transpose(ps_t[:, 0:BC], acs[0:BC, ts(ch, 128)], ident[0:BC, 0:BC])
            nc.scalar.activation(out=attT[:, ch, :], in_=ps_t[:, 0:BC],
                                 func=AF.Copy, scale=ATT_SCALE)

        ctx_sb = big.tile([BC, D], F32)
        for b in range(BC):
            ps_c = pp.tile([1, D], F32, tag="ps_small", name="ps_ctx")
            for q in range(NCH // 4):
                ln = ln_pool.tile([128, 4, D], FP8)
                nc.sync.dma_start(
                    out=ln[:],
                    in_=lisN_d[b].rearrange("(ch p) d -> p ch d", p=128)[:, 4 * q:4 * q + 4, :],
                )
                for j in range(4):
                    ch = 4 * q + j
                    nc.tensor.matmul(
                        ps_c[:],
                        attT[:, ch, b:b + 1],
                        ln[:, j, :],
                        start=(ch == 0),
                        stop=(ch == NCH - 1),
                    )
            c_stage = st_pool.tile([1, D], F32, name="c_stage")
            if b % 2 == 0:
                nc.scalar.activation(out=c_stage[:], in_=ps_c[0:1, :],
                                     func=AF.Copy, scale=1.0 / ATT_SCALE)
            else:
                nc.vector.tensor_scalar(c_stage[:], ps_c[0:1, :], 1.0 / ATT_SCALE,
                                        None, OP.mult)
            nc.sync.dma_start(out=ctx_sb[b:b + 1, :], in_=c_stage[:])

        nc.vector.tensor_scalar(ctx_sb[:], ctx_sb[:], nan_col[:], None, OP.add)
        nc.sync.dma_start(out=ctx_d[:], in_=ctx_sb[:])


_NC_CACHE = {}
LAST_RESULTS = None


def _install_trace_shim():
    """The agent container's antenv stub lacks axon_hooks; register the NTFF
    profile hook ourselves so run_bass_kernel_spmd(trace=True) works."""
    import sys
    import types

    try:
        from antenv.axon_hooks import get_axon_ntff_profile_hook  # noqa: F401
    except ImportError:
        import antenv

        mod = types.ModuleType("antenv.axon_hooks")
        mod._hook = None
        mod.set_axon_ntff_profile_hook = lambda h: setattr(mod, "_hook", h)
        mod.get_axon_ntff_profile_hook = lambda: mod._hook
        sys.modules["antenv.axon_hooks"] = mod
        antenv.axon_hooks = mod
        try:
            from trn_agent_boot.trn_boot import _ntff_profile_via_ctypes
            mod._hook = _ntff_profile_via_ctypes("/opt/axon/libaxon_pjrt.so")
        except Exception:
            pass
    import concourse.bass_utils as bu
    bu.upload_artifacts = lambda tmpdir: tmpdir


def _get_nc():
    if "nc" not in _NC_CACHE:
        _NC_CACHE["nc"] = _build_nc()
    return _NC_CACHE["nc"]


def kernel(decoder_state, listener_feature, phi_w, phi_b, psi_w, psi_b):
    global LAST_RESULTS
    bf16 = ml_dtypes.bfloat16

    dec = np.asarray(decoder_state, np.float32)[:, 0, :]          # [B, D]
    lis_f32 = np.asarray(listener_feature, np.float32)            # [B, T, D]
    lis_bf = lis_f32.astype(bf16)
    lis_f8 = lis_f32.astype(ml_dtypes.float8_e4m3)
    psiT = np.ascontiguousarray(np.asarray(psi_w, np.float32).T).astype(bf16)
    phiT = np.ascontiguousarray(np.asarray(phi_w, np.float32).T).astype(bf16)
    psib = np.asarray(psi_b, np.float32)
    phib = np.asarray(phi_b, np.float32)

    in_maps = []
    for c in range(NCORES):
        bs = slice(c * BC, (c + 1) * BC)
        in_maps.append({
            "lis_t": np.ascontiguousarray(lis_bf[bs].transpose(0, 2, 1)),
            "lis_n": np.ascontiguousarray(lis_f8[bs]),
            "dec_t": np.ascontiguousarray(dec[bs].T).astype(bf16),
            "psi_t": psiT,
            "phi_t": phiT,
            "psi_bias": psib,
            "phi_bias": phib,
        })

    trace = bool(os.environ.get("KERNEL_TRACE"))
    if trace:
        _install_trace_shim()
    nc = _get_nc()
    res = run_bass_kernel_spmd(
        nc,
        in_maps,
        core_ids=list(range(NCORES)),
        trace=trace,
    )
    LAST_RESULTS = res

    att = np.concatenate([res.results[c]["att"] for c in range(NCORES)], axis=0)
    ctx = np.concatenate([res.results[c]["ctx"] for c in range(NCORES)], axis=0)
    flag = float(res.results[0]["flg"][0, 0])
    count_original = np.int32(1 if flag > 0.5 else 0)
    count_copula = np.int32(1 - count_original)
    return att, ctx, count_original, count_copula


# revision 48
# speedup vs baseline: 1.2364x; 1.0754x over previous
"""Trainium2 Bass kernel for nn_Attention_37074157699663.

Copula attention: MLP preprocess (phi/psi + ReLU), energy = comp_dec . comp_lis,
adjacent-column Pearson correlation over the batch, bivariate Gaussian copula
pdf, softmax, context. Data-parallel over batch across 8 NeuronCores; the three
per-column stat vectors (sum e, sum e^2, sum e[t]e[t+1]) are AllReduced.

The reference computes norm.pdf(e) (underflows fp32 to 0) times exp(copula
exponent) (overflows fp32 to inf), so reference rows are NaN wherever any
column overflows. We reproduce that exactly by computing the overflow mask
explicitly (expo > ln(float32 max)) instead of trusting device exp() inf
semantics, then injecting NaN per row.
"""

import os

import numpy as np
import ml_dtypes

import concourse.bacc as bacc
import concourse.bass as bass
import concourse.tile as tile
from concourse import mybir
from concourse.bass_utils import run_bass_kernel_spmd
from concourse.masks import make_identity
from bass_rust import AxisListType

BF16 = mybir.dt.bfloat16
FP8 = mybir.dt.float8e4
F32 = mybir.dt.float32
ATT_SCALE = 128.0   # att probs (~5e-4) scaled into fp8e4m3's normal range
AF = mybir.ActivationFunctionType
OP = mybir.AluOpType
ts = bass.ts

B, T, D, M = 64, 2048, 512, 512
NCORES = 8
BC = B // NCORES        # 8 batch rows per core
TC = 512                # t-chunk for MM1/MM2 (one PSUM bank)
NTC = T // TC           # 4
NDT = D // 128          # 4 contraction tiles
NMT = M // 128          # 4 m tiles
NCH = T // 128          # 16 chunks for the context matmul
TP = T // 128           # 16: free size of [128, TP] column-stat tiles

INV_SQRT_2PI = 0.3989422804014327
TWO_PI = 6.283185307179586   # Rsqrt(2*pi*sd) == INV_SQRT_2PI * rsqrt(sd)
EXP_OVF = 88.722839          # ln(float32 max): fp32 exp(x) == inf for x above this
E_BIG = 20.0                 # e > 20  =>  INV_SQRT_2PI * exp(-e^2/2) is exactly 0 in fp32
CLAMP_LO = -103.0
CLAMP_HI = 88.0
BINV = 1.0 / B


def _col_ap(flat_ap):
    """[2048]-flat AP -> [128, 16] (t = p*16 + f)."""
    return flat_ap.rearrange("(p f) -> p f", p=128)


def _build_nc():
    nc = bacc.Bacc("TRN2", target_bir_lowering=False)

    # lis_t / psi_t are pre-interleaved for DoubleRow: d = 256*g + 128*j + ki
    lisT_d = nc.dram_tensor("lis_t", [BC, 2, 128, 2, T], FP8, kind="ExternalInput")
    lisN_d = nc.dram_tensor("lis_n", [BC, T, D], FP8, kind="ExternalInput")
    decT_d = nc.dram_tensor("dec_t", [D, BC], BF16, kind="ExternalInput")
    psiT_d = nc.dram_tensor("psi_t", [2, 128, 2, M], FP8, kind="ExternalInput")
    phiT_d = nc.dram_tensor("phi_t", [D, M], BF16, kind="ExternalInput")
    psib_d = nc.dram_tensor("psi_bias", [M], F32, kind="ExternalInput")
    phib_d = nc.dram_tensor("phi_bias", [M], F32, kind="ExternalInput")

    att_d = nc.dram_tensor("att", [BC, T], F32, kind="ExternalOutput")
    ctx_d = nc.dram_tensor("ctx", [BC, D], F32, kind="ExternalOutput")
    flg_d = nc.dram_tensor("flg", [1, 1], F32, kind="ExternalOutput")

    with tile.TileContext(nc) as tc:
        _body(nc, tc, lisT_d, lisN_d, decT_d, psiT_d, phiT_d, psib_d, phib_d,
              att_d, ctx_d, flg_d)
    nc.compile()
    return nc


def _body(nc, tc, lisT_d, lisN_d, decT_d, psiT_d, phiT_d, psib_d, phib_d,
          att_d, ctx_d, flg_d):
    with (
        tc.tile_pool(name="wp", bufs=1) as wp,
        tc.tile_pool(name="lt_pool", bufs=4) as lt_pool,
        tc.tile_pool(name="ln_pool", bufs=6) as ln_pool,
        tc.tile_pool(name="cl_pool", bufs=2) as cl_pool,
        tc.tile_pool(name="big", bufs=1) as big,
        tc.tile_pool(name="st_pool", bufs=2) as st_pool,
        tc.tile_pool(name="col", bufs=1) as col,
        tc.tile_pool(name="pp", bufs=3, space="PSUM") as pp,
        tc.tile_pool(name="ep", bufs=2, space="PSUM") as ep,
        tc.tile_pool(name="dram", bufs=1, space="DRAM") as dram,
    ):
        # ---------------- constants / weights ----------------
        psiT = wp.tile([128, 2, 2, M], FP8)
        nc.sync.dma_start(out=psiT[:], in_=psiT_d[:].rearrange("g p j m -> p g j m"))
        phiT = wp.tile([128, NDT, M], BF16)
        nc.sync.dma_start(out=phiT[:], in_=phiT_d[:].rearrange("(dt p) m -> p dt m", p=128))
        decT = wp.tile([128, NDT, BC], BF16)
        nc.sync.dma_start(out=decT[:], in_=decT_d[:].rearrange("(dt p) b -> p dt b", p=128))
        psib = wp.tile([128, NMT], F32)
        nc.sync.dma_start(out=psib[:], in_=psib_d[:].rearrange("(mt p) -> p mt", p=128))
        phib = wp.tile([128, NMT], F32)
        nc.sync.dma_start(out=phib[:], in_=phib_d[:].rearrange("(mt p) -> p mt", p=128))

        ident = wp.tile([128, 128], F32)
        make_identity(nc, ident)
        ones8 = wp.tile([BC, 1], F32)
        nc.vector.memset(ones8, 1.0)
        ones16 = wp.tile([128, TP], F32)
        nc.vector.memset(ones16, 1.0)
        nan1 = wp.tile([128, 1], F32)
        nc.vector.memset(nan1, float("nan"))
        zero1 = wp.tile([128, 1], F32)
        nc.vector.memset(zero1, 0.0)

        # ---------------- comp_dec = relu(dec @ phi_w.T + phi_b) ----------------
        comp_decT = wp.tile([128, NMT, BC], BF16)   # [m-part, mt, b]
        for mt in range(NMT):
            ps_dec = pp.tile([128, BC], F32, tag="ps_small")
            for dt in range(NDT):
                nc.tensor.matmul(
                    ps_dec[:],
                    phiT[:, dt, ts(mt, 128)],
                    decT[:, dt, :],
                    start=(dt == 0),
                    stop=(dt == NDT - 1),
                )
            nc.scalar.activation(out=comp_decT[:, mt, :], in_=ps_dec[:],
                                 func=AF.Relu, bias=phib[:, mt:mt + 1], scale=1.0)

        # ---------------- main loop: comp_lis + energy ----------------
        e_sb = big.tile([BC, T], F32)
        for b in range(BC):
            for tci in range(NTC):
                lt = lt_pool.tile([128, 2, 2, TC], FP8)
                for g in range(2):
                    nc.sync.dma_start(
                        out=lt[:, g, :, :],
                        in_=lisT_d[b, g][:, :, ts(tci, TC)],
                    )
                clis = cl_pool.tile([128, NMT, TC], BF16)
                for mt in range(NMT):
                    ps_l = pp.tile([128, TC], F32, tag="ps_mm1")
                    for g in range(2):
                        nc.tensor.matmul(
                            ps_l[:],
                            psiT[:, g, :, ts(mt, 128)],
                            lt[:, g, :, :],
                            start=(g == 0),
                            stop=(g == 1),
                            perf_mode=mybir.MatmulPerfMode.DoubleRow,
                        )
                    if mt % 2 == 0:
                        nc.scalar.activation(out=clis[:, mt, :], in_=ps_l[:],
                                             func=AF.Relu, bias=psib[:, mt:mt + 1], scale=1.0)
                    else:
                        nc.vector.tensor_scalar(clis[:, mt, :], ps_l[:],
                                                psib[:, mt:mt + 1], 0.0, OP.add, OP.max)
                ps_e = ep.tile([1, TC], F32, tag="eps")
                for mt in range(NMT):
                    nc.tensor.matmul(
                        ps_e[:],
                        comp_decT[:, mt, b:b + 1],
                        clis[:, mt, :],
                        start=(mt == 0),
                        stop=(mt == NMT - 1),
                    )
                # engine accesses must start at a 32-aligned partition: copy
                # the M=1 PSUM row to an SBUF stage at partition 0, then
                # DMA-scatter into row b
                e_stage = st_pool.tile([1, TC], F32, name="e_stage")
                if (b + tci) % 2 == 0:
                    nc.scalar.copy(out=e_stage[:], in_=ps_e[0:1, :])
                else:
                    nc.vector.tensor_copy(out=e_stage[:], in_=ps_e[0:1, :])
                nc.sync.dma_start(out=e_sb[b:b + 1, ts(tci, TC)], in_=e_stage[:])

        # ---------------- softmax helper ----------------
        def softmax_rows(dst, srcv, tag):
            rmax = wp.tile([BC, 1], F32, name=f"rmax_{tag}")
            nc.vector.reduce_max(rmax[:], srcv[:], axis=AxisListType.X)
            nc.vector.tensor_scalar(rmax[:], rmax[:], -1.0, None, OP.mult)
            nc.vector.tensor_scalar(dst[:], srcv[:], rmax[:], CLAMP_LO, OP.add, OP.max)
            nc.scalar.activation(out=dst[:], in_=dst[:], func=AF.Exp)
            rsum = wp.tile([BC, 1], F32, name=f"rsum_{tag}")
            nc.vector.reduce_sum(rsum[:], dst[:], axis=AxisListType.X)
            nc.vector.reciprocal(rsum[:], rsum[:])
            nc.vector.tensor_scalar(dst[:], dst[:], rsum[:], None, OP.mult)

        # ---------------- batch stats + AllReduce ----------------
        cc_in = dram.tile([3, T], F32)
        cc_out = dram.tile([3, T], F32)
        scr = dram.tile([8, T], F32)   # scratch rows: shifted-ss + coef rows + uo

        sq = big.tile([BC, T], F32, tag="shared_c")
        nc.vector.tensor_mul(sq[:], e_sb[:], e_sb[:])
        prod = big.tile([BC, T], F32)
        nc.vector.memset(prod[:, T - 1:T], 0.0)
        nc.vector.tensor_mul(prod[:, 0:T - 1], e_sb[:, 0:T - 1], e_sb[:, 1:T])

        for k, src in enumerate((e_sb, sq, prod)):
            st_row = st_pool.tile([1, T], F32)
            for tci in range(NTC):
                ps_s = ep.tile([1, TC], F32, tag="eps", name="ps_stat")
                nc.tensor.matmul(
                    ps_s[:],
                    ones8[:],
                    src[0:BC, ts(tci, TC)],
                    start=True,
                    stop=True,
                )
                if (k + tci) % 2 == 0:
                    nc.scalar.copy(out=st_row[:, ts(tci, TC)], in_=ps_s[0:1, :])
                else:
                    nc.vector.tensor_copy(out=st_row[:, ts(tci, TC)], in_=ps_s[0:1, :])
            nc.sync.dma_start(out=cc_in[k:k + 1, :], in_=st_row[:])

        nc.gpsimd.collective_compute(
            "AllReduce",
            OP.add,
            replica_groups=[list(range(NCORES))],
            ins=[cc_in.opt()],
            outs=[cc_out.opt()],
        )

        # ---------------- local-only work overlapping the AllReduce ------------
        # marginal = norm.pdf(e) with exact zeros where it underflows in fp32
        marg = big.tile([BC, T], F32, tag="shared_b")
        nc.vector.tensor_scalar(marg[:], sq[:], -0.5, CLAMP_LO, OP.mult, OP.max)
        nc.scalar.activation(out=marg[:], in_=marg[:], func=AF.Exp)
        mz = big.tile([BC, T], F32)
        nc.vector.tensor_scalar(mz[:], e_sb[:], E_BIG, None, OP.is_le)
        nc.vector.tensor_mul(marg[:], marg[:], mz[:])
        sqs = big.tile([BC, T - 2], F32)
        nc.vector.tensor_add(sqs[:], sq[:, 0:T - 2], sq[:, 1:T - 1])   # x1^2 + x2^2
        sme = big.tile([BC, T], F32)
        softmax_rows(sme, e_sb, "eng")   # plain-softmax fallback branch

        # ---------------- column stats in [128, 16] layout ----------------
        cc_flat = cc_out.rearrange("a b -> (a b)")
        gs1 = col.tile([128, TP], F32)
        nc.sync.dma_start(out=gs1[:], in_=_col_ap(cc_flat[0:T]))
        gs1S = col.tile([128, TP], F32)
        nc.sync.dma_start(out=gs1S[:], in_=_col_ap(cc_flat[1:T + 1]))
        gs2 = col.tile([128, TP], F32)
        nc.sync.dma_start(out=gs2[:], in_=_col_ap(cc_flat[T:2 * T]))
        gs12 = col.tile([128, TP], F32)
        nc.sync.dma_start(out=gs12[:], in_=_col_ap(cc_flat[2 * T:3 * T]))

        m2 = col.tile([128, TP], F32)
        nc.vector.scalar_tensor_tensor(m2[:], gs1[:], BINV, gs1[:], OP.mult, OP.mult)
        ss = col.tile([128, TP], F32)
        nc.vector.tensor_sub(ss[:], gs2[:], m2[:])
        scr_flat = scr.rearrange("a b -> (a b)")
        nc.sync.dma_start(out=_col_ap(scr_flat[0:T]), in_=ss[:])
        ssS = col.tile([128, TP], F32)
        nc.sync.dma_start(out=ssS[:], in_=_col_ap(scr_flat[1:T + 1]))

        sprod = col.tile([128, TP], F32)
        nc.vector.scalar_tensor_tensor(sprod[:], gs1[:], BINV, gs1S[:], OP.mult, OP.mult)
        num = col.tile([128, TP], F32)
        nc.vector.tensor_sub(num[:], gs12[:], sprod[:])
        d2 = col.tile([128, TP], F32)
        nc.vector.tensor_mul(d2[:], ss[:], ssS[:])
        rsq = col.tile([128, TP], F32)
        nc.scalar.activation(out=rsq[:], in_=d2[:], func=AF.Sqrt)
        nc.vector.reciprocal(rsq[:], rsq[:])
        rv = col.tile([128, TP], F32)
        nc.vector.tensor_mul(rv[:], num[:], rsq[:])
        rr2 = col.tile([128, TP], F32)
        nc.vector.tensor_mul(rr2[:], rv[:], rv[:])
        det = col.tile([128, TP], F32)
        nc.vector.tensor_scalar(det[:], rr2[:], -1.0, 1.0, OP.mult, OP.add)
        degen = col.tile([128, TP], F32)
        nc.vector.tensor_scalar(degen[:], det[:], 0.01, None, OP.is_lt)
        ndeg = col.tile([128, TP], F32)
        nc.vector.tensor_scalar(ndeg[:], degen[:], -1.0, 1.0, OP.mult, OP.add)
        # safe_det = det where det >= 0.01 else 1.0 (masks are exactly 0/1)
        sd = col.tile([128, TP], F32)
        nc.vector.tensor_mul(sd[:], det[:], ndeg[:])
        nc.vector.tensor_add(sd[:], sd[:], degen[:])
        invsd = col.tile([128, TP], F32)
        nc.vector.reciprocal(invsd[:], sd[:])
        c1 = col.tile([128, TP], F32)
        nc.vector.tensor_mul(c1[:], rv[:], invsd[:])
        c2 = col.tile([128, TP], F32)
        nc.vector.scalar_tensor_tensor(c2[:], c1[:], 0.5, rv[:], OP.mult, OP.mult)
        ccoef = col.tile([128, TP], F32)
        nc.scalar.activation(out=ccoef[:], in_=sd[:], func=AF.Sqrt, scale=TWO_PI)
        nc.vector.reciprocal(ccoef[:], ccoef[:])

        # coef rows -> DRAM -> [BC, T-2] broadcasts
        for k, srct in enumerate((c1, c2, ccoef, ndeg)):
            nc.sync.dma_start(out=_col_ap(scr_flat[(k + 1) * T:(k + 2) * T]), in_=srct[:])
        c1_8 = big.tile([BC, T - 2], F32)
        nc.gpsimd.dma_start(out=c1_8[:], in_=scr[1:2, 0:T - 2].to_broadcast([BC, T - 2]))
        c2_8 = big.tile([BC, T - 2], F32)
        nc.gpsimd.dma_start(out=c2_8[:], in_=scr[2:3, 0:T - 2].to_broadcast([BC, T - 2]))
        ccoef8 = big.tile([BC, T - 2], F32)
        nc.gpsimd.dma_start(out=ccoef8[:], in_=scr[3:4, 0:T - 2].to_broadcast([BC, T - 2]))
        ndeg8 = big.tile([BC, T - 2], F32)
        nc.gpsimd.dma_start(out=ndeg8[:], in_=scr[4:5, 0:T - 2].to_broadcast([BC, T - 2]))

        # ---------------- fallback flag: any all-zero energy column ------------
        gs2row = big.tile([1, T], F32, tag="shared_a")
        nc.sync.dma_start(out=gs2row[:], in_=cc_out[1:2, :])
        nc.vector.tensor_scalar(gs2row[:], gs2row[:], 0.0, None, OP.is_le)
        uo1 = wp.tile([1, 1], F32)
        nc.vector.reduce_max(uo1[:], gs2row[:], axis=AxisListType.X)
        nc.sync.dma_start(out=flg_d[:], in_=uo1[:])
        nc.sync.dma_start(out=scr[5:6, 0:1], in_=uo1[:])
        uo8 = wp.tile([BC, 1], F32)
        nc.gpsimd.dma_start(out=uo8[:], in_=scr[5:6, 0:1].to_broadcast([BC, 1]))

        # ---------------- copula pdf grid [BC, T-2] ----------------
        # att column i in 1..T-2 uses x1 = e[:, i-1], x2 = e[:, i], r = rr[i-1].
        # Column-split every elementwise op: DVE takes 2/3, GpSimd (half DVE's
        # rate) takes 1/3, so both halves finish together.
        NT2 = T - 2
        SPL = 1364

        def split2(fn, n=NT2):
            fn(nc.vector, slice(0, n))

        x2 = e_sb[:, 1:T - 1]
        tA = mz[:, 0:T - 2]                  # mz is free after the marg mask

        expo = big.tile([BC, T - 2], F32)
        split2(lambda g, s: g.tensor_mul(expo[:, s], prod[:, 0:T - 2][:, s], c1_8[:, s]))
        split2(lambda g, s: g.tensor_mul(tA[:, s], sqs[:, s], c2_8[:, s]))
        split2(lambda g, s: g.tensor_sub(expo[:, s], expo[:, s], tA[:, s]))

        # NaN positions: fp32 exp(expo) overflows AND marginal underflowed to 0,
        # in a non-degenerate column.
        nc.vector.tensor_scalar(tA, expo[:], EXP_OVF, None, OP.is_gt)
        nc.vector.scalar_tensor_tensor(tA, x2, E_BIG, tA, OP.is_gt, OP.mult)
        split2(lambda g, s: g.tensor_mul(tA[:, s], tA[:, s], ndeg8[:, s]))
        nan_row = wp.tile([BC, 1], F32)
        nc.vector.reduce_max(nan_row[:], tA, axis=AxisListType.X)

        # cop (finite branch) and energy_f1
        nc.vector.tensor_scalar(expo[:], expo[:], CLAMP_HI, CLAMP_LO, OP.min, OP.max)
        nc.scalar.activation(out=expo[:], in_=expo[:], func=AF.Exp)
        ef1 = big.tile([BC, T], F32, tag="shared_a")
        ef1c = ef1[:, 1:T - 1]
        mgc = marg[:, 1:T - 1]
        split2(lambda g, s: g.tensor_mul(ef1c[:, s], mgc[:, s], expo[:, s]))
        split2(lambda g, s: g.tensor_mul(ef1c[:, s], ef1c[:, s], ccoef8[:, s]))
        # degenerate columns: cop = 10.0; exact 0/1 blend
        split2(lambda g, s: g.tensor_mul(ef1c[:, s], ef1c[:, s], ndeg8[:, s]))
        nc.vector.scalar_tensor_tensor(tA, ndeg8[:], -10.0, mgc, OP.mult, OP.mult)
        nc.vector.scalar_tensor_tensor(tA, mgc, 10.0, tA, OP.mult, OP.add)
        split2(lambda g, s: g.tensor_add(ef1c[:, s], ef1c[:, s], tA[:, s]))
        nc.vector.tensor_copy(out=ef1[:, 0:1], in_=marg[:, 0:1])
        nc.vector.tensor_copy(out=ef1[:, T - 1:T], in_=marg[:, T - 1:T])

        acs = big.tile([BC, T], F32, tag="shared_c")   # copula softmax
        softmax_rows(acs, ef1, "cop")

        # blend: att_safe = uo * softmax(e) + (1-uo) * copula_score
        uo8inv = wp.tile([BC, 1], F32)
        nc.vector.tensor_scalar(uo8inv[:], uo8[:], -1.0, 1.0, OP.mult, OP.add)
        nc.vector.tensor_scalar(acs[:], acs[:], uo8inv[:], None, OP.mult)
        nc.vector.tensor_scalar(sme[:], sme[:], uo8[:], None, OP.mult)
        split2(lambda g, s: g.tensor_add(acs[:, s], acs[:, s], sme[:, s]), n=T)

        # NaN rows (copula branch only)
        mask2 = wp.tile([BC, 1], F32)
        nc.vector.tensor_mul(mask2[:], nan_row[:], uo8inv[:])
        mask2u = wp.tile([BC, 1], mybir.dt.uint8)
        nc.vector.tensor_copy(out=mask2u[:], in_=mask2[:])
        nan_col = wp.tile([BC, 1], F32)
        nc.vector.select(nan_col[:], mask2u[:], nan1[0:BC, :], zero1[0:BC, :])

        att_out = big.tile([BC, T], F32, tag="shared_b")
        nc.vector.tensor_scalar(att_out[:], acs[:], nan_col[:], None, OP.add)
        nc.sync.dma_start(out=att_d[:], in_=att_out[:])

        # ---------------- context = sum_t att[b,t] * lis[b,t,:] ----------------
        attT = wp.tile([128, NCH, BC], FP8)
        for ch in range(NCH):
            ps_t = pp.tile([128, BC], F32, tag="ps_small", name="ps_tp")
            nc.tensor.transpose(ps_t[:, 0:BC], acs[0:BC, ts(ch, 128)], ident[0:BC, 0:BC])
            nc.scalar.activation(out=attT[:, ch, :], in_=ps_t[:, 0:BC],
                                 func=AF.Copy, scale=ATT_SCALE)

        ctx_sb = big.tile([BC, D], F32)
        for b in range(BC):
            ps_c = pp.tile([1, D], F32, tag="ps_small", name="ps_ctx")
            for q in range(NCH // 4):
                ln = ln_pool.tile([128, 4, D], FP8)
                nc.sync.dma_start(
                    out=ln[:],
                    in_=lisN_d[b].rearrange("(ch p) d -> p ch d", p=128)[:, 4 * q:4 * q + 4, :],
                )
                for j in range(4):
                    ch = 4 * q + j
                    nc.tensor.matmul(
                        ps_c[:],
                        attT[:, ch, b:b + 1],
                        ln[:, j, :],
                        start=(ch == 0),
                        stop=(ch == NCH - 1),
                    )
            c_stage = st_pool.tile([1, D], F32, name="c_stage")
            if b % 2 == 0:
                nc.scalar.activation(out=c_stage[:], in_=ps_c[0:1, :],
                                     func=AF.Copy, scale=1.0 / ATT_SCALE)
            else:
                nc.vector.tensor_scalar(c_stage[:], ps_c[0:1, :], 1.0 / ATT_SCALE,
                                        None, OP.mult)
            nc.sync.dma_start(out=ctx_sb[b:b + 1, :], in_=c_stage[:])

        nc.vector.tensor_scalar(ctx_sb[:], ctx_sb[:], nan_col[:], None, OP.add)
        nc.sync.dma_start(out=ctx_d[:], in_=ctx_sb[:])


_NC_CACHE = {}
LAST_RESULTS = None


def _install_trace_shim():
    """The agent container's antenv stub lacks axon_hooks; register the NTFF
    profile hook ourselves so run_bass_kernel_spmd(trace=True) works."""
    import sys
    import types

    try:
        from antenv.axon_hooks import get_axon_ntff_profile_hook  # noqa: F401
    except ImportError:
        import antenv

        mod = types.ModuleType("antenv.axon_hooks")
        mod._hook = None
        mod.set_axon_ntff_profile_hook = lambda h: setattr(mod, "_hook", h)
        mod.get_axon_ntff_profile_hook = lambda: mod._hook
        sys.modules["antenv.axon_hooks"] = mod
        antenv.axon_hooks = mod
        try:
            from trn_agent_boot.trn_boot import _ntff_profile_via_ctypes
            mod._hook = _ntff_profile_via_ctypes("/opt/axon/libaxon_pjrt.so")
        except Exception:
            pass
    import concourse.bass_utils as bu
    bu.upload_artifacts = lambda tmpdir: tmpdir


def _get_nc():
    if "nc" not in _NC_CACHE:
        _NC_CACHE["nc"] = _build_nc()
    return _NC_CACHE["nc"]


def kernel(decoder_state, listener_feature, phi_w, phi_b, psi_w, psi_b):
    global LAST_RESULTS
    bf16 = ml_dtypes.bfloat16

    dec = np.asarray(decoder_state, np.float32)[:, 0, :]          # [B, D]
    lis_f32 = np.asarray(listener_feature, np.float32)            # [B, T, D]
    lis_f8 = lis_f32.astype(ml_dtypes.float8_e4m3)
    fp8 = ml_dtypes.float8_e4m3
    # DoubleRow interleave: index [g, ki, j, m] = psi_w.T[256g + 128j + ki, m]
    psiT = np.ascontiguousarray(
        np.asarray(psi_w, np.float32).T.reshape(2, 2, 128, M)
        .transpose(0, 2, 1, 3)).astype(fp8)
    phiT = np.ascontiguousarray(np.asarray(phi_w, np.float32).T).astype(bf16)
    psib = np.asarray(psi_b, np.float32)
    phib = np.asarray(phi_b, np.float32)

    in_maps = []
    for c in range(NCORES):
        bs = slice(c * BC, (c + 1) * BC)
        lis_t8 = (lis_f8[bs].transpose(0, 2, 1)           # [BC, D, T]
                  .reshape(BC, 2, 2, 128, T)              # [BC, g, j, ki, T]
                  .transpose(0, 1, 3, 2, 4))              # [BC, g, ki, j, T]
        in_maps.append({
            "lis_t": np.ascontiguousarray(lis_t8),
            "lis_n": np.ascontiguousarray(lis_f8[bs]),
            "dec_t": np.ascontiguousarray(dec[bs].T).astype(bf16),
            "psi_t": psiT,
            "phi_t": phiT,
            "psi_bias": psib,
            "phi_bias": phib,
        })

    trace = bool(os.environ.get("KERNEL_TRACE"))
    if trace:
        _install_trace_shim()
    nc = _get_nc()
    res = run_bass_kernel_spmd(
        nc,
        in_maps,
        core_ids=list(range(NCORES)),
        trace=trace,
    )
    LAST_RESULTS = res

    att = np.concatenate([res.results[c]["att"] for c in range(NCORES)], axis=0)
    ctx = np.concatenate([res.results[c]["ctx"] for c in range(NCORES)], axis=0)
    flag = float(res.results[0]["flg"][0, 0])
    count_original = np.int32(1 if flag > 0.5 else 0)
    count_copula = np.int32(1 - count_original)
    return att, ctx, count_original, count_copula


# revision 55
# speedup vs baseline: 1.4633x; 1.1835x over previous
"""Trainium2 Bass kernel for nn_Attention_37074157699663.

Copula attention: MLP preprocess (phi/psi + ReLU), energy = comp_dec . comp_lis,
adjacent-column Pearson correlation over the batch, bivariate Gaussian copula
pdf, softmax, context. Data-parallel over batch across 8 NeuronCores; the three
per-column stat vectors (sum e, sum e^2, sum e[t]e[t+1]) are AllReduced.

The reference computes norm.pdf(e) (underflows fp32 to 0) times exp(copula
exponent) (overflows fp32 to inf), so reference rows are NaN wherever any
column overflows. We reproduce that exactly by computing the overflow mask
explicitly (expo > ln(float32 max)) instead of trusting device exp() inf
semantics, then injecting NaN per row.
"""

import os

import numpy as np
import ml_dtypes

import concourse.bacc as bacc
import concourse.bass as bass
import concourse.tile as tile
from concourse import mybir
from concourse.bass_utils import run_bass_kernel_spmd
from concourse.masks import make_identity
from bass_rust import AxisListType

BF16 = mybir.dt.bfloat16
FP8 = mybir.dt.float8e4
F32 = mybir.dt.float32
ATT_SCALE = 128.0   # att probs (~5e-4) scaled into fp8e4m3's normal range
AF = mybir.ActivationFunctionType
OP = mybir.AluOpType
ts = bass.ts

B, T, D, M = 64, 2048, 512, 512
NCORES = 8
BC = B // NCORES        # 8 batch rows per core
TC = 512                # t-chunk for MM1/MM2 (one PSUM bank)
NTC = T // TC           # 4
NDT = D // 128          # 4 contraction tiles
NMT = M // 128          # 4 m tiles
NCH = T // 128          # 16 chunks for the context matmul
TP = T // 128           # 16: free size of [128, TP] column-stat tiles

INV_SQRT_2PI = 0.3989422804014327
TWO_PI = 6.283185307179586   # Rsqrt(2*pi*sd) == INV_SQRT_2PI * rsqrt(sd)
EXP_OVF = 88.722839          # ln(float32 max): fp32 exp(x) == inf for x above this
E_BIG = 20.0                 # e > 20  =>  INV_SQRT_2PI * exp(-e^2/2) is exactly 0 in fp32
CLAMP_LO = -103.0
CLAMP_HI = 88.0
BINV = 1.0 / B


def _col_ap(flat_ap):
    """[2048]-flat AP -> [128, 16] (t = p*16 + f)."""
    return flat_ap.rearrange("(p f) -> p f", p=128)


def _build_nc():
    nc = bacc.Bacc("TRN2", target_bir_lowering=False)

    # lis_t / psi_t are pre-interleaved for DoubleRow: d = 256*g + 128*j + ki
    lisT_d = nc.dram_tensor("lis_t", [BC, 2, 128, 2, T], FP8, kind="ExternalInput")
    lisN_d = nc.dram_tensor("lis_n", [BC, T, D], FP8, kind="ExternalInput")
    decT_d = nc.dram_tensor("dec_t", [D, BC], BF16, kind="ExternalInput")
    psiT_d = nc.dram_tensor("psi_t", [2, 128, 2, M], FP8, kind="ExternalInput")
    phiT_d = nc.dram_tensor("phi_t", [D, M], BF16, kind="ExternalInput")
    psib_d = nc.dram_tensor("psi_bias", [M], F32, kind="ExternalInput")
    phib_d = nc.dram_tensor("phi_bias", [M], F32, kind="ExternalInput")

    att_d = nc.dram_tensor("att", [BC, T], F32, kind="ExternalOutput")
    ctx_d = nc.dram_tensor("ctx", [BC, D], F32, kind="ExternalOutput")
    flg_d = nc.dram_tensor("flg", [1, 1], F32, kind="ExternalOutput")

    with tile.TileContext(nc) as tc:
        _body(nc, tc, lisT_d, lisN_d, decT_d, psiT_d, phiT_d, psib_d, phib_d,
              att_d, ctx_d, flg_d)
    nc.compile()
    return nc


def _body(nc, tc, lisT_d, lisN_d, decT_d, psiT_d, phiT_d, psib_d, phib_d,
          att_d, ctx_d, flg_d):
    with (
        tc.tile_pool(name="wp", bufs=1) as wp,
        tc.tile_pool(name="lt_pool", bufs=4) as lt_pool,
        tc.tile_pool(name="ln_pool", bufs=6) as ln_pool,
        tc.tile_pool(name="cl_pool", bufs=2) as cl_pool,
        tc.tile_pool(name="big", bufs=1) as big,
        tc.tile_pool(name="st_pool", bufs=2) as st_pool,
        tc.tile_pool(name="col", bufs=1) as col,
        tc.tile_pool(name="pp", bufs=3, space="PSUM") as pp,
        tc.tile_pool(name="ep", bufs=2, space="PSUM") as ep,
        tc.tile_pool(name="dram", bufs=1, space="DRAM") as dram,
    ):
        # ---------------- constants / weights ----------------
        psiT = wp.tile([128, 2, 2, M], FP8)
        nc.sync.dma_start(out=psiT[:], in_=psiT_d[:].rearrange("g p j m -> p g j m"))
        phiT = wp.tile([128, NDT, M], BF16)
        nc.sync.dma_start(out=phiT[:], in_=phiT_d[:].rearrange("(dt p) m -> p dt m", p=128))
        decT = wp.tile([128, NDT, BC], BF16)
        nc.sync.dma_start(out=decT[:], in_=decT_d[:].rearrange("(dt p) b -> p dt b", p=128))
        psib = wp.tile([128, NMT], F32)
        nc.sync.dma_start(out=psib[:], in_=psib_d[:].rearrange("(mt p) -> p mt", p=128))
        phib = wp.tile([128, NMT], F32)
        nc.sync.dma_start(out=phib[:], in_=phib_d[:].rearrange("(mt p) -> p mt", p=128))

        ident = wp.tile([128, 128], F32)
        make_identity(nc, ident)
        ones8 = wp.tile([BC, 1], F32)
        nc.vector.memset(ones8, 1.0)
        ones16 = wp.tile([128, TP], F32)
        nc.vector.memset(ones16, 1.0)
        nan1 = wp.tile([128, 1], F32)
        nc.vector.memset(nan1, float("nan"))
        zero1 = wp.tile([128, 1], F32)
        nc.vector.memset(zero1, 0.0)

        # ---------------- comp_dec = relu(dec @ phi_w.T + phi_b) ----------------
        comp_decT = wp.tile([128, NMT, 16], FP8)    # [m-part, mt, b (pad 16)]
        for mt in range(NMT):
            ps_dec = pp.tile([128, BC], F32, tag="ps_small")
            for dt in range(NDT):
                nc.tensor.matmul(
                    ps_dec[:],
                    phiT[:, dt, ts(mt, 128)],
                    decT[:, dt, :],
                    start=(dt == 0),
                    stop=(dt == NDT - 1),
                )
            nc.scalar.activation(out=comp_decT[:, mt, 0:BC], in_=ps_dec[:],
                                 func=AF.Relu, bias=phib[:, mt:mt + 1], scale=1.0)

        # ---------------- main loop: comp_lis + energy ----------------
        e_sb = big.tile([BC, T], F32)
        for b in range(BC):
            for tci in range(NTC):
                lt = lt_pool.tile([128, 2, 2, TC], FP8)
                for g in range(2):
                    nc.sync.dma_start(
                        out=lt[:, g, :, :],
                        in_=lisT_d[b, g][:, :, ts(tci, TC)],
                    )
                clis = cl_pool.tile([128, NMT, TC], FP8)
                for mt in range(NMT):
                    ps_l = pp.tile([128, TC], F32, tag="ps_mm1")
                    for g in range(2):
                        nc.tensor.matmul(
                            ps_l[:],
                            psiT[:, g, :, ts(mt, 128)],
                            lt[:, g, :, :],
                            start=(g == 0),
                            stop=(g == 1),
                            perf_mode=mybir.MatmulPerfMode.DoubleRow,
                        )
                    if mt % 2 == 0:
                        nc.scalar.activation(out=clis[:, mt, :], in_=ps_l[:],
                                             func=AF.Relu, bias=psib[:, mt:mt + 1], scale=1.0)
                    else:
                        nc.vector.tensor_scalar(clis[:, mt, :], ps_l[:],
                                                psib[:, mt:mt + 1], 0.0, OP.add, OP.max)
                ps_e = ep.tile([1, TC], F32, tag="eps")
                for g2 in range(2):
                    nc.tensor.matmul(
                        ps_e[:],
                        comp_decT[:, 2 * g2:2 * g2 + 2, b:b + 1],
                        clis[:, 2 * g2:2 * g2 + 2, :],
                        start=(g2 == 0),
                        stop=(g2 == 1),
                        perf_mode=mybir.MatmulPerfMode.DoubleRow,
                    )
                # engine accesses must start at a 32-aligned partition: copy
                # the M=1 PSUM row to an SBUF stage at partition 0, then
                # DMA-scatter into row b
                e_stage = st_pool.tile([1, TC], F32, name="e_stage")
                if (b + tci) % 2 == 0:
                    nc.scalar.copy(out=e_stage[:], in_=ps_e[0:1, :])
                else:
                    nc.vector.tensor_copy(out=e_stage[:], in_=ps_e[0:1, :])
                nc.sync.dma_start(out=e_sb[b:b + 1, ts(tci, TC)], in_=e_stage[:])

        # ---------------- softmax helper ----------------
        def softmax_rows(dst, srcv, tag):
            rmax = wp.tile([BC, 1], F32, name=f"rmax_{tag}")
            nc.vector.reduce_max(rmax[:], srcv[:], axis=AxisListType.X)
            nc.vector.tensor_scalar(rmax[:], rmax[:], -1.0, None, OP.mult)
            nc.vector.tensor_scalar(dst[:], srcv[:], rmax[:], CLAMP_LO, OP.add, OP.max)
            nc.scalar.activation(out=dst[:], in_=dst[:], func=AF.Exp)
            rsum = wp.tile([BC, 1], F32, name=f"rsum_{tag}")
            nc.vector.reduce_sum(rsum[:], dst[:], axis=AxisListType.X)
            nc.vector.reciprocal(rsum[:], rsum[:])
            nc.vector.tensor_scalar(dst[:], dst[:], rsum[:], None, OP.mult)

        # ---------------- batch stats + AllReduce ----------------
        cc_in = dram.tile([3, T], F32)
        cc_out = dram.tile([3, T], F32)
        scr = dram.tile([8, T], F32)   # scratch rows: shifted-ss + coef rows + uo

        sq = big.tile([BC, T], F32, tag="shared_c")
        nc.vector.tensor_mul(sq[:], e_sb[:], e_sb[:])
        prod = big.tile([BC, T], F32)
        nc.vector.memset(prod[:, T - 1:T], 0.0)
        nc.vector.tensor_mul(prod[:, 0:T - 1], e_sb[:, 0:T - 1], e_sb[:, 1:T])

        for k, src in enumerate((e_sb, sq, prod)):
            st_row = st_pool.tile([1, T], F32)
            for tci in range(NTC):
                ps_s = ep.tile([1, TC], F32, tag="eps", name="ps_stat")
                nc.tensor.matmul(
                    ps_s[:],
                    ones8[:],
                    src[0:BC, ts(tci, TC)],
                    start=True,
                    stop=True,
                )
                if (k + tci) % 2 == 0:
                    nc.scalar.copy(out=st_row[:, ts(tci, TC)], in_=ps_s[0:1, :])
                else:
                    nc.vector.tensor_copy(out=st_row[:, ts(tci, TC)], in_=ps_s[0:1, :])
            nc.sync.dma_start(out=cc_in[k:k + 1, :], in_=st_row[:])

        nc.gpsimd.collective_compute(
            "AllReduce",
            OP.add,
            replica_groups=[list(range(NCORES))],
            ins=[cc_in.opt()],
            outs=[cc_out.opt()],
        )

        # ---------------- local-only work overlapping the AllReduce ------------
        # marginal = norm.pdf(e) with exact zeros where it underflows in fp32
        marg = big.tile([BC, T], F32, tag="shared_b")
        nc.vector.tensor_scalar(marg[:], sq[:], -0.5, CLAMP_LO, OP.mult, OP.max)
        nc.scalar.activation(out=marg[:], in_=marg[:], func=AF.Exp)
        mz = big.tile([BC, T], F32)
        nc.vector.tensor_scalar(mz[:], e_sb[:], E_BIG, None, OP.is_le)
        nc.vector.tensor_mul(marg[:], marg[:], mz[:])
        sqs = big.tile([BC, T - 2], F32)
        nc.vector.tensor_add(sqs[:], sq[:, 0:T - 2], sq[:, 1:T - 1])   # x1^2 + x2^2
        sme = big.tile([BC, T], F32)
        softmax_rows(sme, e_sb, "eng")   # plain-softmax fallback branch

        # ---------------- column stats in [128, 16] layout ----------------
        cc_flat = cc_out.rearrange("a b -> (a b)")
        gs1 = col.tile([128, TP], F32)
        nc.sync.dma_start(out=gs1[:], in_=_col_ap(cc_flat[0:T]))
        gs1S = col.tile([128, TP], F32)
        nc.sync.dma_start(out=gs1S[:], in_=_col_ap(cc_flat[1:T + 1]))
        gs2 = col.tile([128, TP], F32)
        nc.sync.dma_start(out=gs2[:], in_=_col_ap(cc_flat[T:2 * T]))
        gs12 = col.tile([128, TP], F32)
        nc.sync.dma_start(out=gs12[:], in_=_col_ap(cc_flat[2 * T:3 * T]))

        m2 = col.tile([128, TP], F32)
        nc.vector.scalar_tensor_tensor(m2[:], gs1[:], BINV, gs1[:], OP.mult, OP.mult)
        ss = col.tile([128, TP], F32)
        nc.vector.tensor_sub(ss[:], gs2[:], m2[:])
        scr_flat = scr.rearrange("a b -> (a b)")
        nc.sync.dma_start(out=_col_ap(scr_flat[0:T]), in_=ss[:])
        ssS = col.tile([128, TP], F32)
        nc.sync.dma_start(out=ssS[:], in_=_col_ap(scr_flat[1:T + 1]))

        sprod = col.tile([128, TP], F32)
        nc.vector.scalar_tensor_tensor(sprod[:], gs1[:], BINV, gs1S[:], OP.mult, OP.mult)
        num = col.tile([128, TP], F32)
        nc.vector.tensor_sub(num[:], gs12[:], sprod[:])
        d2 = col.tile([128, TP], F32)
        nc.vector.tensor_mul(d2[:], ss[:], ssS[:])
        rsq = col.tile([128, TP], F32)
        nc.scalar.activation(out=rsq[:], in_=d2[:], func=AF.Sqrt)
        nc.vector.reciprocal(rsq[:], rsq[:])
        rv = col.tile([128, TP], F32)
        nc.vector.tensor_mul(rv[:], num[:], rsq[:])
        rr2 = col.tile([128, TP], F32)
        nc.vector.tensor_mul(rr2[:], rv[:], rv[:])
        det = col.tile([128, TP], F32)
        nc.vector.tensor_scalar(det[:], rr2[:], -1.0, 1.0, OP.mult, OP.add)
        degen = col.tile([128, TP], F32)
        nc.vector.tensor_scalar(degen[:], det[:], 0.01, None, OP.is_lt)
        ndeg = col.tile([128, TP], F32)
        nc.vector.tensor_scalar(ndeg[:], degen[:], -1.0, 1.0, OP.mult, OP.add)
        # safe_det = det where det >= 0.01 else 1.0 (masks are exactly 0/1)
        sd = col.tile([128, TP], F32)
        nc.vector.tensor_mul(sd[:], det[:], ndeg[:])
        nc.vector.tensor_add(sd[:], sd[:], degen[:])
        invsd = col.tile([128, TP], F32)
        nc.vector.reciprocal(invsd[:], sd[:])
        c1 = col.tile([128, TP], F32)
        nc.vector.tensor_mul(c1[:], rv[:], invsd[:])
        c2 = col.tile([128, TP], F32)
        nc.vector.scalar_tensor_tensor(c2[:], c1[:], 0.5, rv[:], OP.mult, OP.mult)
        ccoef = col.tile([128, TP], F32)
        nc.scalar.activation(out=ccoef[:], in_=sd[:], func=AF.Sqrt, scale=TWO_PI)
        nc.vector.reciprocal(ccoef[:], ccoef[:])

        # coef rows -> DRAM -> [BC, T-2] broadcasts
        for k, srct in enumerate((c1, c2, ccoef, ndeg)):
            nc.sync.dma_start(out=_col_ap(scr_flat[(k + 1) * T:(k + 2) * T]), in_=srct[:])
        c1_8 = big.tile([BC, T - 2], F32)
        nc.gpsimd.dma_start(out=c1_8[:], in_=scr[1:2, 0:T - 2].to_broadcast([BC, T - 2]))
        c2_8 = big.tile([BC, T - 2], F32)
        nc.gpsimd.dma_start(out=c2_8[:], in_=scr[2:3, 0:T - 2].to_broadcast([BC, T - 2]))
        ccoef8 = big.tile([BC, T - 2], F32)
        nc.gpsimd.dma_start(out=ccoef8[:], in_=scr[3:4, 0:T - 2].to_broadcast([BC, T - 2]))
        ndeg8 = big.tile([BC, T - 2], F32)
        nc.gpsimd.dma_start(out=ndeg8[:], in_=scr[4:5, 0:T - 2].to_broadcast([BC, T - 2]))

        # ---------------- fallback flag: any all-zero energy column ------------
        gs2row = big.tile([1, T], F32, tag="shared_a")
        nc.sync.dma_start(out=gs2row[:], in_=cc_out[1:2, :])
        nc.vector.tensor_scalar(gs2row[:], gs2row[:], 0.0, None, OP.is_le)
        uo1 = wp.tile([1, 1], F32)
        nc.vector.reduce_max(uo1[:], gs2row[:], axis=AxisListType.X)
        nc.sync.dma_start(out=flg_d[:], in_=uo1[:])
        nc.sync.dma_start(out=scr[5:6, 0:1], in_=uo1[:])
        uo8 = wp.tile([BC, 1], F32)
        nc.gpsimd.dma_start(out=uo8[:], in_=scr[5:6, 0:1].to_broadcast([BC, 1]))

        # ---------------- copula pdf grid [BC, T-2] ----------------
        # att column i in 1..T-2 uses x1 = e[:, i-1], x2 = e[:, i], r = rr[i-1].
        # Column-split every elementwise op: DVE takes 2/3, GpSimd (half DVE's
        # rate) takes 1/3, so both halves finish together.
        NT2 = T - 2
        SPL = 1364

        def split2(fn, n=NT2):
            fn(nc.vector, slice(0, n))

        x2 = e_sb[:, 1:T - 1]
        tA = mz[:, 0:T - 2]                  # mz is free after the marg mask

        expo = big.tile([BC, T - 2], F32)
        split2(lambda g, s: g.tensor_mul(expo[:, s], prod[:, 0:T - 2][:, s], c1_8[:, s]))
        split2(lambda g, s: g.tensor_mul(tA[:, s], sqs[:, s], c2_8[:, s]))
        split2(lambda g, s: g.tensor_sub(expo[:, s], expo[:, s], tA[:, s]))

        # NaN positions: fp32 exp(expo) overflows AND marginal underflowed to 0,
        # in a non-degenerate column.
        nc.vector.tensor_scalar(tA, expo[:], EXP_OVF, None, OP.is_gt)
        nc.vector.scalar_tensor_tensor(tA, x2, E_BIG, tA, OP.is_gt, OP.mult)
        split2(lambda g, s: g.tensor_mul(tA[:, s], tA[:, s], ndeg8[:, s]))
        nan_row = wp.tile([BC, 1], F32)
        nc.vector.reduce_max(nan_row[:], tA, axis=AxisListType.X)

        # cop (finite branch) and energy_f1
        nc.vector.tensor_scalar(expo[:], expo[:], CLAMP_HI, CLAMP_LO, OP.min, OP.max)
        nc.scalar.activation(out=expo[:], in_=expo[:], func=AF.Exp)
        ef1 = big.tile([BC, T], F32, tag="shared_a")
        ef1c = ef1[:, 1:T - 1]
        mgc = marg[:, 1:T - 1]
        split2(lambda g, s: g.tensor_mul(ef1c[:, s], mgc[:, s], expo[:, s]))
        split2(lambda g, s: g.tensor_mul(ef1c[:, s], ef1c[:, s], ccoef8[:, s]))
        # degenerate columns: cop = 10.0; exact 0/1 blend
        split2(lambda g, s: g.tensor_mul(ef1c[:, s], ef1c[:, s], ndeg8[:, s]))
        nc.vector.scalar_tensor_tensor(tA, ndeg8[:], -10.0, mgc, OP.mult, OP.mult)
        nc.vector.scalar_tensor_tensor(tA, mgc, 10.0, tA, OP.mult, OP.add)
        split2(lambda g, s: g.tensor_add(ef1c[:, s], ef1c[:, s], tA[:, s]))
        nc.vector.tensor_copy(out=ef1[:, 0:1], in_=marg[:, 0:1])
        nc.vector.tensor_copy(out=ef1[:, T - 1:T], in_=marg[:, T - 1:T])

        acs = big.tile([BC, T], F32, tag="shared_c")   # copula softmax
        softmax_rows(acs, ef1, "cop")

        # blend: att_safe = uo * softmax(e) + (1-uo) * copula_score
        uo8inv = wp.tile([BC, 1], F32)
        nc.vector.tensor_scalar(uo8inv[:], uo8[:], -1.0, 1.0, OP.mult, OP.add)
        nc.vector.tensor_scalar(acs[:], acs[:], uo8inv[:], None, OP.mult)
        nc.vector.tensor_scalar(sme[:], sme[:], uo8[:], None, OP.mult)
        split2(lambda g, s: g.tensor_add(acs[:, s], acs[:, s], sme[:, s]), n=T)

        # NaN rows (copula branch only)
        mask2 = wp.tile([BC, 1], F32)
        nc.vector.tensor_mul(mask2[:], nan_row[:], uo8inv[:])
        mask2u = wp.tile([BC, 1], mybir.dt.uint8)
        nc.vector.tensor_copy(out=mask2u[:], in_=mask2[:])
        nan_col = wp.tile([BC, 1], F32)
        nc.vector.select(nan_col[:], mask2u[:], nan1[0:BC, :], zero1[0:BC, :])

        att_out = big.tile([BC, T], F32, tag="shared_b")
        nc.vector.tensor_scalar(att_out[:], acs[:], nan_col[:], None, OP.add)
        nc.sync.dma_start(out=att_d[:], in_=att_out[:])

        # ---------------- context = sum_t att[b,t] * lis[b,t,:] ----------------
        attT = wp.tile([128, NCH, 16], FP8)
        for ch in range(NCH):
            ps_t = pp.tile([128, BC], F32, tag="ps_small", name="ps_tp")
            nc.tensor.transpose(ps_t[:, 0:BC], acs[0:BC, ts(ch, 128)], ident[0:BC, 0:BC])
            nc.scalar.activation(out=attT[:, ch, 0:BC], in_=ps_t[:, 0:BC],
                                 func=AF.Copy, scale=ATT_SCALE)

        ctx_sb = big.tile([BC, D], F32)
        for b in range(BC):
            ps_c = pp.tile([1, D], F32, tag="ps_small", name="ps_ctx")
            for q in range(NCH // 4):
                ln = ln_pool.tile([128, 4, D], FP8)
                nc.sync.dma_start(
                    out=ln[:],
                    in_=lisN_d[b].rearrange("(ch p) d -> p ch d", p=128)[:, 4 * q:4 * q + 4, :],
                )
                for j in range(2):
                    gq = 2 * q + j
                    nc.tensor.matmul(
                        ps_c[:],
                        attT[:, 2 * gq:2 * gq + 2, b:b + 1],
                        ln[:, 2 * j:2 * j + 2, :],
                        start=(gq == 0),
                        stop=(gq == NCH // 2 - 1),
                        perf_mode=mybir.MatmulPerfMode.DoubleRow,
                    )
            c_stage = st_pool.tile([1, D], F32, name="c_stage")
            if b % 2 == 0:
                nc.scalar.activation(out=c_stage[:], in_=ps_c[0:1, :],
                                     func=AF.Copy, scale=1.0 / ATT_SCALE)
            else:
                nc.vector.tensor_scalar(c_stage[:], ps_c[0:1, :], 1.0 / ATT_SCALE,
                                        None, OP.mult)
            nc.sync.dma_start(out=ctx_sb[b:b + 1, :], in_=c_stage[:])

        nc.vector.tensor_scalar(ctx_sb[:], ctx_sb[:], nan_col[:], None, OP.add)
        nc.sync.dma_start(out=ctx_d[:], in_=ctx_sb[:])


_NC_CACHE = {}
LAST_RESULTS = None


def _install_trace_shim():
    """The agent container's antenv stub lacks axon_hooks; register the NTFF
    profile hook ourselves so run_bass_kernel_spmd(trace=True) works."""
    import sys
    import types

    try:
        from antenv.axon_hooks import get_axon_ntff_profile_hook  # noqa: F401
    except ImportError:
        import antenv

        mod = types.ModuleType("antenv.axon_hooks")
        mod._hook = None
        mod.set_axon_ntff_profile_hook = lambda h: setattr(mod, "_hook", h)
        mod.get_axon_ntff_profile_hook = lambda: mod._hook
        sys.modules["antenv.axon_hooks"] = mod
        antenv.axon_hooks = mod
        try:
            from trn_agent_boot.trn_boot import _ntff_profile_via_ctypes
            mod._hook = _ntff_profile_via_ctypes("/opt/axon/libaxon_pjrt.so")
        except Exception:
            pass
    import concourse.bass_utils as bu
    bu.upload_artifacts = lambda tmpdir: tmpdir


def _get_nc():
    if "nc" not in _NC_CACHE:
        _NC_CACHE["nc"] = _build_nc()
    return _NC_CACHE["nc"]


def kernel(decoder_state, listener_feature, phi_w, phi_b, psi_w, psi_b):
    global LAST_RESULTS
    bf16 = ml_dtypes.bfloat16

    dec = np.asarray(decoder_state, np.float32)[:, 0, :]          # [B, D]
    lis_f32 = np.asarray(listener_feature, np.float32)            # [B, T, D]
    lis_f8 = lis_f32.astype(ml_dtypes.float8_e4m3)
    fp8 = ml_dtypes.float8_e4m3
    # DoubleRow interleave: index [g, ki, j, m] = psi_w.T[256g + 128j + ki, m]
    psiT = np.ascontiguousarray(
        np.asarray(psi_w, np.float32).T.reshape(2, 2, 128, M)
        .transpose(0, 2, 1, 3)).astype(fp8)
    phiT = np.ascontiguousarray(np.asarray(phi_w, np.float32).T).astype(bf16)
    psib = np.asarray(psi_b, np.float32)
    phib = np.asarray(phi_b, np.float32)

    in_maps = []
    for c in range(NCORES):
        bs = slice(c * BC, (c + 1) * BC)
        lis_t8 = (lis_f8[bs].transpose(0, 2, 1)           # [BC, D, T]
                  .reshape(BC, 2, 2, 128, T)              # [BC, g, j, ki, T]
                  .transpose(0, 1, 3, 2, 4))              # [BC, g, ki, j, T]
        in_maps.append({
            "lis_t": np.ascontiguousarray(lis_t8),
            "lis_n": np.ascontiguousarray(lis_f8[bs]),
            "dec_t": np.ascontiguousarray(dec[bs].T).astype(bf16),
            "psi_t": psiT,
            "phi_t": phiT,
            "psi_bias": psib,
            "phi_bias": phib,
        })

    trace = bool(os.environ.get("KERNEL_TRACE"))
    if trace:
        _install_trace_shim()
    nc = _get_nc()
    res = run_bass_kernel_spmd(
        nc,
        in_maps,
        core_ids=list(range(NCORES)),
        trace=trace,
    )
    LAST_RESULTS = res

    att = np.concatenate([res.results[c]["att"] for c in range(NCORES)], axis=0)
    ctx = np.concatenate([res.results[c]["ctx"] for c in range(NCORES)], axis=0)
    flag = float(res.results[0]["flg"][0, 0])
    count_original = np.int32(1 if flag > 0.5 else 0)
    count_copula = np.int32(1 - count_original)
    return att, ctx, count_original, count_copula


# revision 58
# speedup vs baseline: 1.4968x; 1.0229x over previous
"""Trainium2 Bass kernel for nn_Attention_37074157699663.

Copula attention: MLP preprocess (phi/psi + ReLU), energy = comp_dec . comp_lis,
adjacent-column Pearson correlation over the batch, bivariate Gaussian copula
pdf, softmax, context. Data-parallel over batch across 8 NeuronCores; the three
per-column stat vectors (sum e, sum e^2, sum e[t]e[t+1]) are AllReduced.

The reference computes norm.pdf(e) (underflows fp32 to 0) times exp(copula
exponent) (overflows fp32 to inf), so reference rows are NaN wherever any
column overflows. We reproduce that exactly by computing the overflow mask
explicitly (expo > ln(float32 max)) instead of trusting device exp() inf
semantics, then injecting NaN per row.
"""

import os

import numpy as np
import ml_dtypes

import concourse.bacc as bacc
import concourse.bass as bass
import concourse.tile as tile
from concourse import mybir
from concourse.bass_utils import run_bass_kernel_spmd
from concourse.masks import make_identity
from bass_rust import AxisListType

BF16 = mybir.dt.bfloat16
FP8 = mybir.dt.float8e4
F32 = mybir.dt.float32
ATT_SCALE = 128.0   # att probs (~5e-4) scaled into fp8e4m3's normal range
AF = mybir.ActivationFunctionType
OP = mybir.AluOpType
ts = bass.ts

B, T, D, M = 64, 2048, 512, 512
NCORES = 8
BC = B // NCORES        # 8 batch rows per core
TC = 512                # t-chunk for MM1/MM2 (one PSUM bank)
NTC = T // TC           # 4
NDT = D // 128          # 4 contraction tiles
NMT = M // 128          # 4 m tiles
NCH = T // 128          # 16 chunks for the context matmul
TP = T // 128           # 16: free size of [128, TP] column-stat tiles

INV_SQRT_2PI = 0.3989422804014327
TWO_PI = 6.283185307179586   # Rsqrt(2*pi*sd) == INV_SQRT_2PI * rsqrt(sd)
EXP_OVF = 88.722839          # ln(float32 max): fp32 exp(x) == inf for x above this
E_BIG = 20.0                 # e > 20  =>  INV_SQRT_2PI * exp(-e^2/2) is exactly 0 in fp32
CLAMP_LO = -103.0
CLAMP_HI = 88.0
BINV = 1.0 / B


def _col_ap(flat_ap):
    """[2048]-flat AP -> [128, 16] (t = p*16 + f)."""
    return flat_ap.rearrange("(p f) -> p f", p=128)


def _build_nc():
    nc = bacc.Bacc("TRN2", target_bir_lowering=False)

    # lis_t / psi_t are pre-interleaved for DoubleRow: d = 256*g + 128*j + ki
    lisT_d = nc.dram_tensor("lis_t", [BC, 2, 128, 2, T], FP8, kind="ExternalInput")
    lisN_d = nc.dram_tensor("lis_n", [BC, T, D], FP8, kind="ExternalInput")
    decT_d = nc.dram_tensor("dec_t", [D, BC], BF16, kind="ExternalInput")
    psiT_d = nc.dram_tensor("psi_t", [2, 128, 2, M], FP8, kind="ExternalInput")
    phiT_d = nc.dram_tensor("phi_t", [D, M], BF16, kind="ExternalInput")
    psib_d = nc.dram_tensor("psi_bias", [M], F32, kind="ExternalInput")
    phib_d = nc.dram_tensor("phi_bias", [M], F32, kind="ExternalInput")

    att_d = nc.dram_tensor("att", [BC, T], F32, kind="ExternalOutput")
    ctx_d = nc.dram_tensor("ctx", [BC, D], F32, kind="ExternalOutput")
    flg_d = nc.dram_tensor("flg", [1, 1], F32, kind="ExternalOutput")

    with tile.TileContext(nc) as tc:
        _body(nc, tc, lisT_d, lisN_d, decT_d, psiT_d, phiT_d, psib_d, phib_d,
              att_d, ctx_d, flg_d)
    nc.compile()
    return nc


def _body(nc, tc, lisT_d, lisN_d, decT_d, psiT_d, phiT_d, psib_d, phib_d,
          att_d, ctx_d, flg_d):
    with (
        tc.tile_pool(name="wp", bufs=1) as wp,
        tc.tile_pool(name="lt_pool", bufs=4) as lt_pool,
        tc.tile_pool(name="ln_pool", bufs=14) as ln_pool,
        tc.tile_pool(name="cl_pool", bufs=2) as cl_pool,
        tc.tile_pool(name="big", bufs=1) as big,
        tc.tile_pool(name="st_pool", bufs=2) as st_pool,
        tc.tile_pool(name="col", bufs=1) as col,
        tc.tile_pool(name="pp", bufs=3, space="PSUM") as pp,
        tc.tile_pool(name="ep", bufs=2, space="PSUM") as ep,
        tc.tile_pool(name="dram", bufs=1, space="DRAM") as dram,
    ):
        # ---------------- constants / weights ----------------
        psiT = wp.tile([128, 2, 2, M], FP8)
        nc.sync.dma_start(out=psiT[:], in_=psiT_d[:].rearrange("g p j m -> p g j m"))
        phiT = wp.tile([128, NDT, M], BF16)
        nc.sync.dma_start(out=phiT[:], in_=phiT_d[:].rearrange("(dt p) m -> p dt m", p=128))
        decT = wp.tile([128, NDT, BC], BF16)
        nc.sync.dma_start(out=decT[:], in_=decT_d[:].rearrange("(dt p) b -> p dt b", p=128))
        psib = wp.tile([128, NMT], F32)
        nc.sync.dma_start(out=psib[:], in_=psib_d[:].rearrange("(mt p) -> p mt", p=128))
        phib = wp.tile([128, NMT], F32)
        nc.sync.dma_start(out=phib[:], in_=phib_d[:].rearrange("(mt p) -> p mt", p=128))

        ident = wp.tile([128, 128], F32)
        make_identity(nc, ident)
        ones8 = wp.tile([BC, 1], F32)
        nc.vector.memset(ones8, 1.0)
        ones16 = wp.tile([128, TP], F32)
        nc.vector.memset(ones16, 1.0)
        nan1 = wp.tile([128, 1], F32)
        nc.vector.memset(nan1, float("nan"))
        zero1 = wp.tile([128, 1], F32)
        nc.vector.memset(zero1, 0.0)

        # ---------------- comp_dec = relu(dec @ phi_w.T + phi_b) ----------------
        comp_decT = wp.tile([128, NMT, 16], FP8)    # [m-part, mt, b (pad 16)]
        for mt in range(NMT):
            ps_dec = pp.tile([128, BC], F32, tag="ps_small")
            for dt in range(NDT):
                nc.tensor.matmul(
                    ps_dec[:],
                    phiT[:, dt, ts(mt, 128)],
                    decT[:, dt, :],
                    start=(dt == 0),
                    stop=(dt == NDT - 1),
                )
            nc.scalar.activation(out=comp_decT[:, mt, 0:BC], in_=ps_dec[:],
                                 func=AF.Relu, bias=phib[:, mt:mt + 1], scale=1.0)

        # ---------------- main loop: comp_lis + energy ----------------
        e_sb = big.tile([BC, T], F32)
        for b in range(BC):
            for tci in range(NTC):
                lt = lt_pool.tile([128, 2, 2, TC], FP8)
                for g in range(2):
                    nc.sync.dma_start(
                        out=lt[:, g, :, :],
                        in_=lisT_d[b, g][:, :, ts(tci, TC)],
                    )
                clis = cl_pool.tile([128, NMT, TC], FP8)
                for mt in range(NMT):
                    ps_l = pp.tile([128, TC], F32, tag="ps_mm1")
                    for g in range(2):
                        nc.tensor.matmul(
                            ps_l[:],
                            psiT[:, g, :, ts(mt, 128)],
                            lt[:, g, :, :],
                            start=(g == 0),
                            stop=(g == 1),
                            perf_mode=mybir.MatmulPerfMode.DoubleRow,
                        )
                    if mt % 2 == 0:
                        nc.scalar.activation(out=clis[:, mt, :], in_=ps_l[:],
                                             func=AF.Relu, bias=psib[:, mt:mt + 1], scale=1.0)
                    else:
                        nc.vector.tensor_scalar(clis[:, mt, :], ps_l[:],
                                                psib[:, mt:mt + 1], 0.0, OP.add, OP.max)
                ps_e = ep.tile([1, TC], F32, tag="eps")
                for g2 in range(2):
                    nc.tensor.matmul(
                        ps_e[:],
                        comp_decT[:, 2 * g2:2 * g2 + 2, b:b + 1],
                        clis[:, 2 * g2:2 * g2 + 2, :],
                        start=(g2 == 0),
                        stop=(g2 == 1),
                        perf_mode=mybir.MatmulPerfMode.DoubleRow,
                    )
                # engine accesses must start at a 32-aligned partition: copy
                # the M=1 PSUM row to an SBUF stage at partition 0, then
                # DMA-scatter into row b
                e_stage = st_pool.tile([1, TC], F32, name="e_stage")
                if (b + tci) % 2 == 0:
                    nc.scalar.copy(out=e_stage[:], in_=ps_e[0:1, :])
                else:
                    nc.vector.tensor_copy(out=e_stage[:], in_=ps_e[0:1, :])
                nc.sync.dma_start(out=e_sb[b:b + 1, ts(tci, TC)], in_=e_stage[:])

        # ---------------- softmax helper ----------------
        def softmax_rows(dst, srcv, tag):
            rmax = wp.tile([BC, 1], F32, name=f"rmax_{tag}")
            nc.vector.reduce_max(rmax[:], srcv[:], axis=AxisListType.X)
            nc.vector.tensor_scalar(rmax[:], rmax[:], -1.0, None, OP.mult)
            nc.vector.tensor_scalar(dst[:], srcv[:], rmax[:], CLAMP_LO, OP.add, OP.max)
            nc.scalar.activation(out=dst[:], in_=dst[:], func=AF.Exp)
            rsum = wp.tile([BC, 1], F32, name=f"rsum_{tag}")
            nc.vector.reduce_sum(rsum[:], dst[:], axis=AxisListType.X)
            nc.vector.reciprocal(rsum[:], rsum[:])
            nc.vector.tensor_scalar(dst[:], dst[:], rsum[:], None, OP.mult)

        # ---------------- batch stats + AllReduce ----------------
        cc_in = dram.tile([3, T], F32)
        cc_out = dram.tile([3, T], F32)
        scr = dram.tile([8, T], F32)   # scratch rows: shifted-ss + coef rows + uo

        sq = big.tile([BC, T], F32, tag="shared_c")
        nc.vector.tensor_mul(sq[:], e_sb[:], e_sb[:])
        prod = big.tile([BC, T], F32)
        nc.vector.memset(prod[:, T - 1:T], 0.0)
        nc.vector.tensor_mul(prod[:, 0:T - 1], e_sb[:, 0:T - 1], e_sb[:, 1:T])

        for k, src in enumerate((e_sb, sq, prod)):
            st_row = st_pool.tile([1, T], F32)
            for tci in range(NTC):
                ps_s = ep.tile([1, TC], F32, tag="eps", name="ps_stat")
                nc.tensor.matmul(
                    ps_s[:],
                    ones8[:],
                    src[0:BC, ts(tci, TC)],
                    start=True,
                    stop=True,
                )
                if (k + tci) % 2 == 0:
                    nc.scalar.copy(out=st_row[:, ts(tci, TC)], in_=ps_s[0:1, :])
                else:
                    nc.vector.tensor_copy(out=st_row[:, ts(tci, TC)], in_=ps_s[0:1, :])
            nc.sync.dma_start(out=cc_in[k:k + 1, :], in_=st_row[:])

        nc.gpsimd.collective_compute(
            "AllReduce",
            OP.add,
            replica_groups=[list(range(NCORES))],
            ins=[cc_in.opt()],
            outs=[cc_out.opt()],
        )

        # ---------------- local-only work overlapping the AllReduce ------------
        # marginal = norm.pdf(e) with exact zeros where it underflows in fp32
        marg = big.tile([BC, T], F32, tag="shared_b")
        nc.vector.tensor_scalar(marg[:], sq[:], -0.5, CLAMP_LO, OP.mult, OP.max)
        nc.scalar.activation(out=marg[:], in_=marg[:], func=AF.Exp)
        mz = big.tile([BC, T], F32)
        nc.vector.tensor_scalar(mz[:], e_sb[:], E_BIG, None, OP.is_le)
        nc.vector.tensor_mul(marg[:], marg[:], mz[:])
        sqs = big.tile([BC, T - 2], F32)
        nc.vector.tensor_add(sqs[:], sq[:, 0:T - 2], sq[:, 1:T - 1])   # x1^2 + x2^2
        sme = big.tile([BC, T], F32)
        softmax_rows(sme, e_sb, "eng")   # plain-softmax fallback branch

        # ---------------- column stats in [128, 16] layout ----------------
        cc_flat = cc_out.rearrange("a b -> (a b)")
        gs1 = col.tile([128, TP], F32)
        nc.sync.dma_start(out=gs1[:], in_=_col_ap(cc_flat[0:T]))
        gs1S = col.tile([128, TP], F32)
        nc.sync.dma_start(out=gs1S[:], in_=_col_ap(cc_flat[1:T + 1]))
        gs2 = col.tile([128, TP], F32)
        nc.sync.dma_start(out=gs2[:], in_=_col_ap(cc_flat[T:2 * T]))
        gs12 = col.tile([128, TP], F32)
        nc.sync.dma_start(out=gs12[:], in_=_col_ap(cc_flat[2 * T:3 * T]))

        m2 = col.tile([128, TP], F32)
        nc.vector.scalar_tensor_tensor(m2[:], gs1[:], BINV, gs1[:], OP.mult, OP.mult)
        ss = col.tile([128, TP], F32)
        nc.vector.tensor_sub(ss[:], gs2[:], m2[:])
        scr_flat = scr.rearrange("a b -> (a b)")
        nc.sync.dma_start(out=_col_ap(scr_flat[0:T]), in_=ss[:])
        ssS = col.tile([128, TP], F32)
        nc.sync.dma_start(out=ssS[:], in_=_col_ap(scr_flat[1:T + 1]))

        sprod = col.tile([128, TP], F32)
        nc.vector.scalar_tensor_tensor(sprod[:], gs1[:], BINV, gs1S[:], OP.mult, OP.mult)
        num = col.tile([128, TP], F32)
        nc.vector.tensor_sub(num[:], gs12[:], sprod[:])
        d2 = col.tile([128, TP], F32)
        nc.vector.tensor_mul(d2[:], ss[:], ssS[:])
        rsq = col.tile([128, TP], F32)
        nc.scalar.activation(out=rsq[:], in_=d2[:], func=AF.Sqrt)
        nc.vector.reciprocal(rsq[:], rsq[:])
        rv = col.tile([128, TP], F32)
        nc.vector.tensor_mul(rv[:], num[:], rsq[:])
        rr2 = col.tile([128, TP], F32)
        nc.vector.tensor_mul(rr2[:], rv[:], rv[:])
        det = col.tile([128, TP], F32)
        nc.vector.tensor_scalar(det[:], rr2[:], -1.0, 1.0, OP.mult, OP.add)
        degen = col.tile([128, TP], F32)
        nc.vector.tensor_scalar(degen[:], det[:], 0.01, None, OP.is_lt)
        ndeg = col.tile([128, TP], F32)
        nc.vector.tensor_scalar(ndeg[:], degen[:], -1.0, 1.0, OP.mult, OP.add)
        # safe_det = det where det >= 0.01 else 1.0 (masks are exactly 0/1)
        sd = col.tile([128, TP], F32)
        nc.vector.tensor_mul(sd[:], det[:], ndeg[:])
        nc.vector.tensor_add(sd[:], sd[:], degen[:])
        invsd = col.tile([128, TP], F32)
        nc.vector.reciprocal(invsd[:], sd[:])
        c1 = col.tile([128, TP], F32)
        nc.vector.tensor_mul(c1[:], rv[:], invsd[:])
        c2 = col.tile([128, TP], F32)
        nc.vector.scalar_tensor_tensor(c2[:], c1[:], 0.5, rv[:], OP.mult, OP.mult)
        ccoef = col.tile([128, TP], F32)
        nc.scalar.activation(out=ccoef[:], in_=sd[:], func=AF.Sqrt, scale=TWO_PI)
        nc.vector.reciprocal(ccoef[:], ccoef[:])
        # fold the degenerate-column branch into the coefficients:
        # expo uses c1*ndeg, c2*ndeg (degen cols get expo = 0), and
        # energy_f1 = marg * (exp(expo)*A + Bv) with A = ccoef*ndeg, Bv = 10*degen
        nc.vector.tensor_mul(c1[:], c1[:], ndeg[:])
        nc.vector.tensor_mul(c2[:], c2[:], ndeg[:])
        av = col.tile([128, TP], F32)
        nc.vector.tensor_mul(av[:], ccoef[:], ndeg[:])
        bv = col.tile([128, TP], F32)
        nc.vector.tensor_scalar(bv[:], degen[:], 10.0, None, OP.mult)

        # coef rows -> DRAM -> [BC, T-2] broadcasts
        for k, srct in enumerate((c1, c2, av, bv)):
            nc.sync.dma_start(out=_col_ap(scr_flat[(k + 1) * T:(k + 2) * T]), in_=srct[:])
        c1_8 = big.tile([BC, T - 2], F32)
        nc.gpsimd.dma_start(out=c1_8[:], in_=scr[1:2, 0:T - 2].to_broadcast([BC, T - 2]))
        c2_8 = big.tile([BC, T - 2], F32)
        nc.gpsimd.dma_start(out=c2_8[:], in_=scr[2:3, 0:T - 2].to_broadcast([BC, T - 2]))
        a8 = big.tile([BC, T - 2], F32)
        nc.gpsimd.dma_start(out=a8[:], in_=scr[3:4, 0:T - 2].to_broadcast([BC, T - 2]))
        b8 = big.tile([BC, T - 2], F32)
        nc.gpsimd.dma_start(out=b8[:], in_=scr[4:5, 0:T - 2].to_broadcast([BC, T - 2]))

        # ---------------- copula pdf grid [BC, T-2] ----------------
        # att column i in 1..T-2 uses x1 = e[:, i-1], x2 = e[:, i], r = rr[i-1].
        # Column-split every elementwise op: DVE takes 2/3, GpSimd (half DVE's
        # rate) takes 1/3, so both halves finish together.
        NT2 = T - 2
        SPL = 1364

        def split2(fn, n=NT2):
            fn(nc.vector, slice(0, n))

        x2 = e_sb[:, 1:T - 1]
        tA = mz[:, 0:T - 2]                  # mz is free after the marg mask

        expo = big.tile([BC, T - 2], F32)
        split2(lambda g, s: g.tensor_mul(expo[:, s], prod[:, 0:T - 2][:, s], c1_8[:, s]))
        split2(lambda g, s: g.tensor_mul(tA[:, s], sqs[:, s], c2_8[:, s]))
        split2(lambda g, s: g.tensor_sub(expo[:, s], expo[:, s], tA[:, s]))

        # NaN positions: fp32 exp(expo) overflows AND marginal underflowed to 0,
        # in a non-degenerate column.
        nc.vector.tensor_scalar(tA, expo[:], EXP_OVF, None, OP.is_gt)
        nc.vector.scalar_tensor_tensor(tA, x2, E_BIG, tA, OP.is_gt, OP.mult)
        nan_row = wp.tile([BC, 1], F32)
        nc.vector.reduce_max(nan_row[:], tA, axis=AxisListType.X)

        # cop (finite branch) and energy_f1
        nc.vector.tensor_scalar(expo[:], expo[:], CLAMP_HI, CLAMP_LO, OP.min, OP.max)
        nc.scalar.activation(out=expo[:], in_=expo[:], func=AF.Exp)
        ef1 = big.tile([BC, T], F32, tag="shared_a")
        nc.vector.tensor_mul(expo[:], expo[:], a8[:])
        nc.vector.tensor_add(expo[:], expo[:], b8[:])
        nc.vector.tensor_mul(ef1[:, 1:T - 1], marg[:, 1:T - 1], expo[:])
        nc.vector.tensor_copy(out=ef1[:, 0:1], in_=marg[:, 0:1])
        nc.vector.tensor_copy(out=ef1[:, T - 1:T], in_=marg[:, T - 1:T])

        # ---------------- fallback flag: any all-zero energy column ------------
        gs2row = big.tile([1, T], F32, tag="e_sb")
        nc.sync.dma_start(out=gs2row[:], in_=cc_out[1:2, :])
        nc.vector.tensor_scalar(gs2row[:], gs2row[:], 0.0, None, OP.is_le)
        uo1 = wp.tile([1, 1], F32)
        nc.vector.reduce_max(uo1[:], gs2row[:], axis=AxisListType.X)
        nc.sync.dma_start(out=flg_d[:], in_=uo1[:])
        nc.sync.dma_start(out=scr[5:6, 0:1], in_=uo1[:])
        uo8 = wp.tile([BC, 1], F32)
        nc.gpsimd.dma_start(out=uo8[:], in_=scr[5:6, 0:1].to_broadcast([BC, 1]))

        acs = big.tile([BC, T], F32, tag="shared_c")   # copula softmax
        softmax_rows(acs, ef1, "cop")

        # blend: att_safe = uo * softmax(e) + (1-uo) * copula_score
        uo8inv = wp.tile([BC, 1], F32)
        nc.vector.tensor_scalar(uo8inv[:], uo8[:], -1.0, 1.0, OP.mult, OP.add)
        nc.vector.tensor_scalar(acs[:], acs[:], uo8inv[:], None, OP.mult)
        nc.vector.tensor_scalar(sme[:], sme[:], uo8[:], None, OP.mult)
        split2(lambda g, s: g.tensor_add(acs[:, s], acs[:, s], sme[:, s]), n=T)

        # NaN rows (copula branch only)
        mask2 = wp.tile([BC, 1], F32)
        nc.vector.tensor_mul(mask2[:], nan_row[:], uo8inv[:])
        mask2u = wp.tile([BC, 1], mybir.dt.uint8)
        nc.vector.tensor_copy(out=mask2u[:], in_=mask2[:])
        nan_col = wp.tile([BC, 1], F32)
        nc.vector.select(nan_col[:], mask2u[:], nan1[0:BC, :], zero1[0:BC, :])

        att_out = big.tile([BC, T], F32, tag="shared_b")
        nc.vector.tensor_scalar(att_out[:], acs[:], nan_col[:], None, OP.add)
        nc.sync.dma_start(out=att_d[:], in_=att_out[:])

        # ---------------- context = sum_t att[b,t] * lis[b,t,:] ----------------
        attT = wp.tile([128, NCH, 16], FP8)
        for ch in range(NCH):
            ps_t = pp.tile([128, BC], F32, tag="ps_small", name="ps_tp")
            nc.tensor.transpose(ps_t[:, 0:BC], acs[0:BC, ts(ch, 128)], ident[0:BC, 0:BC])
            nc.scalar.activation(out=attT[:, ch, 0:BC], in_=ps_t[:, 0:BC],
                                 func=AF.Copy, scale=ATT_SCALE)

        ctx_sb = big.tile([BC, D], F32)
        for b in range(BC):
            ps_c = pp.tile([1, D], F32, tag="ps_small", name="ps_ctx")
            for q in range(NCH // 4):
                ln = ln_pool.tile([128, 4, D], FP8)
                nc.sync.dma_start(
                    out=ln[:],
                    in_=lisN_d[b].rearrange("(ch p) d -> p ch d", p=128)[:, 4 * q:4 * q + 4, :],
                )
                for j in range(2):
                    gq = 2 * q + j
                    nc.tensor.matmul(
                        ps_c[:],
                        attT[:, 2 * gq:2 * gq + 2, b:b + 1],
                        ln[:, 2 * j:2 * j + 2, :],
                        start=(gq == 0),
                        stop=(gq == NCH // 2 - 1),
                        perf_mode=mybir.MatmulPerfMode.DoubleRow,
                    )
            c_stage = st_pool.tile([1, D], F32, name="c_stage")
            if b % 2 == 0:
                nc.scalar.activation(out=c_stage[:], in_=ps_c[0:1, :],
                                     func=AF.Copy, scale=1.0 / ATT_SCALE)
            else:
                nc.vector.tensor_scalar(c_stage[:], ps_c[0:1, :], 1.0 / ATT_SCALE,
                                        None, OP.mult)
            nc.sync.dma_start(out=ctx_sb[b:b + 1, :], in_=c_stage[:])

        nc.vector.tensor_scalar(ctx_sb[:], ctx_sb[:], nan_col[:], None, OP.add)
        nc.sync.dma_start(out=ctx_d[:], in_=ctx_sb[:])


_NC_CACHE = {}
LAST_RESULTS = None


def _install_trace_shim():
    """The agent container's antenv stub lacks axon_hooks; register the NTFF
    profile hook ourselves so run_bass_kernel_spmd(trace=True) works."""
    import sys
    import types

    try:
        from antenv.axon_hooks import get_axon_ntff_profile_hook  # noqa: F401
    except ImportError:
        import antenv

        mod = types.ModuleType("antenv.axon_hooks")
        mod._hook = None
        mod.set_axon_ntff_profile_hook = lambda h: setattr(mod, "_hook", h)
        mod.get_axon_ntff_profile_hook = lambda: mod._hook
        sys.modules["antenv.axon_hooks"] = mod
        antenv.axon_hooks = mod
        try:
            from trn_agent_boot.trn_boot import _ntff_profile_via_ctypes
            mod._hook = _ntff_profile_via_ctypes("/opt/axon/libaxon_pjrt.so")
        except Exception:
            pass
    import concourse.bass_utils as bu
    bu.upload_artifacts = lambda tmpdir: tmpdir


def _get_nc():
    if "nc" not in _NC_CACHE:
        _NC_CACHE["nc"] = _build_nc()
    return _NC_CACHE["nc"]


def kernel(decoder_state, listener_feature, phi_w, phi_b, psi_w, psi_b):
    global LAST_RESULTS
    bf16 = ml_dtypes.bfloat16

    dec = np.asarray(decoder_state, np.float32)[:, 0, :]          # [B, D]
    lis_f32 = np.asarray(listener_feature, np.float32)            # [B, T, D]
    lis_f8 = lis_f32.astype(ml_dtypes.float8_e4m3)
    fp8 = ml_dtypes.float8_e4m3
    # DoubleRow interleave: index [g, ki, j, m] = psi_w.T[256g + 128j + ki, m]
    psiT = np.ascontiguousarray(
        np.asarray(psi_w, np.float32).T.reshape(2, 2, 128, M)
        .transpose(0, 2, 1, 3)).astype(fp8)
    phiT = np.ascontiguousarray(np.asarray(phi_w, np.float32).T).astype(bf16)
    psib = np.asarray(psi_b, np.float32)
    phib = np.asarray(phi_b, np.float32)

    in_maps = []
    for c in range(NCORES):
        bs = slice(c * BC, (c + 1) * BC)
        lis_t8 = (lis_f8[bs].transpose(0, 2, 1)           # [BC, D, T]
                  .reshape(BC, 2, 2, 128, T)              # [BC, g, j, ki, T]
                  .transpose(0, 1, 3, 2, 4))              # [BC, g, ki, j, T]
        in_maps.append({
            "lis_t": np.ascontiguousarray(lis_t8),
            "lis_n": np.ascontiguousarray(lis_f8[bs]),
            "dec_t": np.ascontiguousarray(dec[bs].T).astype(bf16),
            "psi_t": psiT,
            "phi_t": phiT,
            "psi_bias": psib,
            "phi_bias": phib,
        })

    trace = bool(os.environ.get("KERNEL_TRACE"))
    if trace:
        _install_trace_shim()
    nc = _get_nc()
    res = run_bass_kernel_spmd(
        nc,
        in_maps,
        core_ids=list(range(NCORES)),
        trace=trace,
    )
    LAST_RESULTS = res

    att = np.concatenate([res.results[c]["att"] for c in range(NCORES)], axis=0)
    ctx = np.concatenate([res.results[c]["ctx"] for c in range(NCORES)], axis=0)
    flag = float(res.results[0]["flg"][0, 0])
    count_original = np.int32(1 if flag > 0.5 else 0)
    count_copula = np.int32(1 - count_original)
    return att, ctx, count_original, count_copula


# revision 60
# speedup vs baseline: 1.6109x; 1.0762x over previous
"""Trainium2 Bass kernel for nn_Attention_37074157699663.

Copula attention: MLP preprocess (phi/psi + ReLU), energy = comp_dec . comp_lis,
adjacent-column Pearson correlation over the batch, bivariate Gaussian copula
pdf, softmax, context. Data-parallel over batch across 8 NeuronCores; the three
per-column stat vectors (sum e, sum e^2, sum e[t]e[t+1]) are AllReduced.

The reference computes norm.pdf(e) (underflows fp32 to 0) times exp(copula
exponent) (overflows fp32 to inf), so reference rows are NaN wherever any
column overflows. We reproduce that exactly by computing the overflow mask
explicitly (expo > ln(float32 max)) instead of trusting device exp() inf
semantics, then injecting NaN per row.
"""

import os

import numpy as np
import ml_dtypes

import concourse.bacc as bacc
import concourse.bass as bass
import concourse.tile as tile
from concourse import mybir
from concourse.bass_utils import run_bass_kernel_spmd
from concourse.masks import make_identity
from bass_rust import AxisListType

BF16 = mybir.dt.bfloat16
FP8 = mybir.dt.float8e4
F32 = mybir.dt.float32
ATT_SCALE = 128.0   # att probs (~5e-4) scaled into fp8e4m3's normal range
AF = mybir.ActivationFunctionType
OP = mybir.AluOpType
ts = bass.ts

B, T, D, M = 64, 2048, 512, 512
NCORES = 8
BC = B // NCORES        # 8 batch rows per core
TC = 512                # t-chunk for MM1/MM2 (one PSUM bank)
NTC = T // TC           # 4
NDT = D // 128          # 4 contraction tiles
NMT = M // 128          # 4 m tiles
NCH = T // 128          # 16 chunks for the context matmul
TP = T // 128           # 16: free size of [128, TP] column-stat tiles

INV_SQRT_2PI = 0.3989422804014327
TWO_PI = 6.283185307179586   # Rsqrt(2*pi*sd) == INV_SQRT_2PI * rsqrt(sd)
EXP_OVF = 88.722839          # ln(float32 max): fp32 exp(x) == inf for x above this
E_BIG = 20.0                 # e > 20  =>  INV_SQRT_2PI * exp(-e^2/2) is exactly 0 in fp32
CLAMP_LO = -103.0
CLAMP_HI = 88.0
BINV = 1.0 / B


def _col_ap(flat_ap):
    """[2048]-flat AP -> [128, 16] (t = p*16 + f)."""
    return flat_ap.rearrange("(p f) -> p f", p=128)


def _build_nc():
    nc = bacc.Bacc("TRN2", target_bir_lowering=False)

    # lis_t / psi_t are pre-interleaved for DoubleRow: d = 256*g + 128*j + ki
    lisT_d = nc.dram_tensor("lis_t", [BC, 2, 128, 2, T], FP8, kind="ExternalInput")
    lisN_d = nc.dram_tensor("lis_n", [BC, T, D], FP8, kind="ExternalInput")
    decT_d = nc.dram_tensor("dec_t", [D, BC], BF16, kind="ExternalInput")
    psiT_d = nc.dram_tensor("psi_t", [2, 128, 2, M], FP8, kind="ExternalInput")
    phiT_d = nc.dram_tensor("phi_t", [D, M], BF16, kind="ExternalInput")
    psib_d = nc.dram_tensor("psi_bias", [M], F32, kind="ExternalInput")
    phib_d = nc.dram_tensor("phi_bias", [M], F32, kind="ExternalInput")

    att_d = nc.dram_tensor("att", [BC, T], F32, kind="ExternalOutput")
    ctx_d = nc.dram_tensor("ctx", [BC, D], F32, kind="ExternalOutput")
    flg_d = nc.dram_tensor("flg", [1, 1], F32, kind="ExternalOutput")

    with tile.TileContext(nc) as tc:
        _body(nc, tc, lisT_d, lisN_d, decT_d, psiT_d, phiT_d, psib_d, phib_d,
              att_d, ctx_d, flg_d)
    nc.compile()
    return nc


def _body(nc, tc, lisT_d, lisN_d, decT_d, psiT_d, phiT_d, psib_d, phib_d,
          att_d, ctx_d, flg_d):
    with (
        tc.tile_pool(name="wp", bufs=1) as wp,
        tc.tile_pool(name="lt_pool", bufs=4) as lt_pool,
        tc.tile_pool(name="ln_pool", bufs=14) as ln_pool,
        tc.tile_pool(name="cl_pool", bufs=2) as cl_pool,
        tc.tile_pool(name="big", bufs=1) as big,
        tc.tile_pool(name="st_pool", bufs=2) as st_pool,
        tc.tile_pool(name="col", bufs=1) as col,
        tc.tile_pool(name="pp", bufs=3, space="PSUM") as pp,
        tc.tile_pool(name="ep", bufs=2, space="PSUM") as ep,
        tc.tile_pool(name="dram", bufs=1, space="DRAM") as dram,
    ):
        # ---------------- constants / weights ----------------
        psiT = wp.tile([128, 2, 2, M], FP8)
        nc.sync.dma_start(out=psiT[:], in_=psiT_d[:].rearrange("g p j m -> p g j m"))
        phiT = wp.tile([128, NDT, M], BF16)
        nc.sync.dma_start(out=phiT[:], in_=phiT_d[:].rearrange("(dt p) m -> p dt m", p=128))
        decT = wp.tile([128, NDT, BC], BF16)
        nc.sync.dma_start(out=decT[:], in_=decT_d[:].rearrange("(dt p) b -> p dt b", p=128))
        psib = wp.tile([128, NMT], F32)
        nc.sync.dma_start(out=psib[:], in_=psib_d[:].rearrange("(mt p) -> p mt", p=128))
        phib = wp.tile([128, NMT], F32)
        nc.sync.dma_start(out=phib[:], in_=phib_d[:].rearrange("(mt p) -> p mt", p=128))

        ident = wp.tile([128, 128], F32)
        make_identity(nc, ident)
        ones8 = wp.tile([BC, 1], BF16)
        nc.vector.memset(ones8, 1.0)
        ones16 = wp.tile([128, TP], F32)
        nc.vector.memset(ones16, 1.0)
        nan1 = wp.tile([128, 1], F32)
        nc.vector.memset(nan1, float("nan"))
        zero1 = wp.tile([128, 1], F32)
        nc.vector.memset(zero1, 0.0)

        # ---------------- comp_dec = relu(dec @ phi_w.T + phi_b) ----------------
        comp_decT = wp.tile([128, NMT, 16], FP8)    # [m-part, mt, b (pad 16)]
        for mt in range(NMT):
            ps_dec = pp.tile([128, BC], F32, tag="ps_small")
            for dt in range(NDT):
                nc.tensor.matmul(
                    ps_dec[:],
                    phiT[:, dt, ts(mt, 128)],
                    decT[:, dt, :],
                    start=(dt == 0),
                    stop=(dt == NDT - 1),
                )
            nc.scalar.activation(out=comp_decT[:, mt, 0:BC], in_=ps_dec[:],
                                 func=AF.Relu, bias=phib[:, mt:mt + 1], scale=1.0)

        # ---------------- main loop: comp_lis + energy ----------------
        e_sb = big.tile([BC, T], BF16)
        for b in range(BC):
            for tci in range(NTC):
                lt = lt_pool.tile([128, 2, 2, TC], FP8)
                for g in range(2):
                    nc.sync.dma_start(
                        out=lt[:, g, :, :],
                        in_=lisT_d[b, g][:, :, ts(tci, TC)],
                    )
                clis = cl_pool.tile([128, NMT, TC], FP8)
                for mt in range(NMT):
                    ps_l = pp.tile([128, TC], F32, tag="ps_mm1")
                    for g in range(2):
                        nc.tensor.matmul(
                            ps_l[:],
                            psiT[:, g, :, ts(mt, 128)],
                            lt[:, g, :, :],
                            start=(g == 0),
                            stop=(g == 1),
                            perf_mode=mybir.MatmulPerfMode.DoubleRow,
                        )
                    if mt % 2 == 0:
                        nc.scalar.activation(out=clis[:, mt, :], in_=ps_l[:],
                                             func=AF.Relu, bias=psib[:, mt:mt + 1], scale=1.0)
                    else:
                        nc.vector.tensor_scalar(clis[:, mt, :], ps_l[:],
                                                psib[:, mt:mt + 1], 0.0, OP.add, OP.max)
                ps_e = ep.tile([1, TC], F32, tag="eps")
                for g2 in range(2):
                    nc.tensor.matmul(
                        ps_e[:],
                        comp_decT[:, 2 * g2:2 * g2 + 2, b:b + 1],
                        clis[:, 2 * g2:2 * g2 + 2, :],
                        start=(g2 == 0),
                        stop=(g2 == 1),
                        perf_mode=mybir.MatmulPerfMode.DoubleRow,
                    )
                # engine accesses must start at a 32-aligned partition: copy
                # the M=1 PSUM row to an SBUF stage at partition 0, then
                # DMA-scatter into row b
                e_stage = st_pool.tile([1, TC], BF16, name="e_stage")
                if (b + tci) % 2 == 0:
                    nc.scalar.copy(out=e_stage[:], in_=ps_e[0:1, :])
                else:
                    nc.vector.tensor_copy(out=e_stage[:], in_=ps_e[0:1, :])
                nc.sync.dma_start(out=e_sb[b:b + 1, ts(tci, TC)], in_=e_stage[:])

        # ---------------- softmax helper ----------------
        def softmax_rows(dst, srcv, tag):
            rmax = wp.tile([BC, 1], F32, name=f"rmax_{tag}")
            nc.vector.reduce_max(rmax[:], srcv[:], axis=AxisListType.X)
            nc.vector.tensor_scalar(rmax[:], rmax[:], -1.0, None, OP.mult)
            nc.vector.tensor_scalar(dst[:], srcv[:], rmax[:], CLAMP_LO, OP.add, OP.max)
            nc.scalar.activation(out=dst[:], in_=dst[:], func=AF.Exp)
            rsum = wp.tile([BC, 1], F32, name=f"rsum_{tag}")
            nc.vector.reduce_sum(rsum[:], dst[:], axis=AxisListType.X)
            nc.vector.reciprocal(rsum[:], rsum[:])
            nc.vector.tensor_scalar(dst[:], dst[:], rsum[:], None, OP.mult)

        # ---------------- batch stats + AllReduce ----------------
        cc_in = dram.tile([3, T], F32)
        cc_out = dram.tile([3, T], F32)
        scr = dram.tile([8, T], F32)   # scratch rows: shifted-ss + coef rows + uo

        sq = big.tile([BC, T], BF16, tag="sqtag")
        nc.vector.tensor_mul(sq[:], e_sb[:], e_sb[:])
        prod = big.tile([BC, T], BF16)
        nc.vector.memset(prod[:, T - 1:T], 0.0)
        nc.vector.tensor_mul(prod[:, 0:T - 1], e_sb[:, 0:T - 1], e_sb[:, 1:T])

        for k, src in enumerate((e_sb, sq, prod)):
            st_row = st_pool.tile([1, T], F32)
            for tci in range(NTC):
                ps_s = ep.tile([1, TC], F32, tag="eps", name="ps_stat")
                nc.tensor.matmul(
                    ps_s[:],
                    ones8[:],
                    src[0:BC, ts(tci, TC)],
                    start=True,
                    stop=True,
                )
                if (k + tci) % 2 == 0:
                    nc.scalar.copy(out=st_row[:, ts(tci, TC)], in_=ps_s[0:1, :])
                else:
                    nc.vector.tensor_copy(out=st_row[:, ts(tci, TC)], in_=ps_s[0:1, :])
            nc.sync.dma_start(out=cc_in[k:k + 1, :], in_=st_row[:])

        nc.gpsimd.collective_compute(
            "AllReduce",
            OP.add,
            replica_groups=[list(range(NCORES))],
            ins=[cc_in.opt()],
            outs=[cc_out.opt()],
        )

        # ---------------- local-only work overlapping the AllReduce ------------
        # marginal = norm.pdf(e) with exact zeros where it underflows in fp32
        marg = big.tile([BC, T], F32, tag="shared_b")
        nc.vector.tensor_scalar(marg[:], sq[:], -0.5, CLAMP_LO, OP.mult, OP.max)
        nc.scalar.activation(out=marg[:], in_=marg[:], func=AF.Exp)
        mz = big.tile([BC, T], F32)
        nc.vector.tensor_scalar(mz[:], e_sb[:], E_BIG, None, OP.is_le)
        nc.vector.tensor_mul(marg[:], marg[:], mz[:])
        sqs = big.tile([BC, T - 2], F32)
        nc.vector.tensor_add(sqs[:], sq[:, 0:T - 2], sq[:, 1:T - 1])   # x1^2 + x2^2
        sme = big.tile([BC, T], F32)
        softmax_rows(sme, e_sb, "eng")   # plain-softmax fallback branch

        # ---------------- column stats in [128, 16] layout ----------------
        cc_flat = cc_out.rearrange("a b -> (a b)")
        gs1 = col.tile([128, TP], F32)
        nc.sync.dma_start(out=gs1[:], in_=_col_ap(cc_flat[0:T]))
        gs1S = col.tile([128, TP], F32)
        nc.sync.dma_start(out=gs1S[:], in_=_col_ap(cc_flat[1:T + 1]))
        gs2 = col.tile([128, TP], F32)
        nc.sync.dma_start(out=gs2[:], in_=_col_ap(cc_flat[T:2 * T]))
        gs12 = col.tile([128, TP], F32)
        nc.sync.dma_start(out=gs12[:], in_=_col_ap(cc_flat[2 * T:3 * T]))

        gs2S = col.tile([128, TP], F32)
        nc.sync.dma_start(out=gs2S[:], in_=_col_ap(cc_flat[T + 1:2 * T + 1]))
        scr_flat = scr.rearrange("a b -> (a b)")
        m2 = col.tile([128, TP], F32)
        nc.vector.scalar_tensor_tensor(m2[:], gs1[:], BINV, gs1[:], OP.mult, OP.mult)
        ss = col.tile([128, TP], F32)
        nc.vector.tensor_sub(ss[:], gs2[:], m2[:])
        ssS = col.tile([128, TP], F32)
        nc.vector.scalar_tensor_tensor(ssS[:], gs1S[:], BINV, gs1S[:], OP.mult, OP.mult)
        nc.vector.tensor_sub(ssS[:], gs2S[:], ssS[:])

        sprod = col.tile([128, TP], F32)
        nc.vector.scalar_tensor_tensor(sprod[:], gs1[:], BINV, gs1S[:], OP.mult, OP.mult)
        num = col.tile([128, TP], F32)
        nc.vector.tensor_sub(num[:], gs12[:], sprod[:])
        d2 = col.tile([128, TP], F32)
        nc.vector.tensor_mul(d2[:], ss[:], ssS[:])
        rsq = col.tile([128, TP], F32)
        nc.scalar.activation(out=rsq[:], in_=d2[:], func=AF.Sqrt)
        nc.vector.reciprocal(rsq[:], rsq[:])
        rv = col.tile([128, TP], F32)
        nc.vector.tensor_mul(rv[:], num[:], rsq[:])
        rr2 = col.tile([128, TP], F32)
        nc.vector.tensor_mul(rr2[:], rv[:], rv[:])
        det = col.tile([128, TP], F32)
        nc.vector.tensor_scalar(det[:], rr2[:], -1.0, 1.0, OP.mult, OP.add)
        degen = col.tile([128, TP], F32)
        nc.vector.tensor_scalar(degen[:], det[:], 0.01, None, OP.is_lt)
        ndeg = col.tile([128, TP], F32)
        nc.vector.tensor_scalar(ndeg[:], degen[:], -1.0, 1.0, OP.mult, OP.add)
        # safe_det = det where det >= 0.01 else 1.0 (masks are exactly 0/1)
        sd = col.tile([128, TP], F32)
        nc.vector.tensor_mul(sd[:], det[:], ndeg[:])
        nc.vector.tensor_add(sd[:], sd[:], degen[:])
        invsd = col.tile([128, TP], F32)
        nc.vector.reciprocal(invsd[:], sd[:])
        c1 = col.tile([128, TP], F32)
        nc.vector.tensor_mul(c1[:], rv[:], invsd[:])
        c2 = col.tile([128, TP], F32)
        nc.vector.scalar_tensor_tensor(c2[:], c1[:], 0.5, rv[:], OP.mult, OP.mult)
        ccoef = col.tile([128, TP], F32)
        nc.scalar.activation(out=ccoef[:], in_=sd[:], func=AF.Sqrt, scale=TWO_PI)
        nc.vector.reciprocal(ccoef[:], ccoef[:])
        # fold the degenerate-column branch into the coefficients:
        # expo uses c1*ndeg, c2*ndeg (degen cols get expo = 0), and
        # energy_f1 = marg * (exp(expo)*A + Bv) with A = ccoef*ndeg, Bv = 10*degen
        nc.vector.tensor_mul(c1[:], c1[:], ndeg[:])
        nc.vector.tensor_mul(c2[:], c2[:], ndeg[:])
        av = col.tile([128, TP], F32)
        nc.vector.tensor_mul(av[:], ccoef[:], ndeg[:])
        bv = col.tile([128, TP], F32)
        nc.vector.tensor_scalar(bv[:], degen[:], 10.0, None, OP.mult)

        # coef rows -> DRAM -> [BC, T-2] broadcasts
        for k, srct in enumerate((c1, c2, av, bv)):
            nc.sync.dma_start(out=_col_ap(scr_flat[(k + 1) * T:(k + 2) * T]), in_=srct[:])
        c1_8 = big.tile([BC, T - 2], F32)
        nc.gpsimd.dma_start(out=c1_8[:], in_=scr[1:2, 0:T - 2].to_broadcast([BC, T - 2]))
        c2_8 = big.tile([BC, T - 2], F32)
        nc.gpsimd.dma_start(out=c2_8[:], in_=scr[2:3, 0:T - 2].to_broadcast([BC, T - 2]))
        a8 = big.tile([BC, T - 2], F32)
        nc.gpsimd.dma_start(out=a8[:], in_=scr[3:4, 0:T - 2].to_broadcast([BC, T - 2]))
        b8 = big.tile([BC, T - 2], F32)
        nc.gpsimd.dma_start(out=b8[:], in_=scr[4:5, 0:T - 2].to_broadcast([BC, T - 2]))

        # ---------------- copula pdf grid [BC, T-2] ----------------
        # att column i in 1..T-2 uses x1 = e[:, i-1], x2 = e[:, i], r = rr[i-1].
        # Column-split every elementwise op: DVE takes 2/3, GpSimd (half DVE's
        # rate) takes 1/3, so both halves finish together.
        NT2 = T - 2
        SPL = 1364

        def split2(fn, n=NT2):
            fn(nc.vector, slice(0, n))

        x2 = e_sb[:, 1:T - 1]
        tA = mz[:, 0:T - 2]                  # mz is free after the marg mask

        expo = big.tile([BC, T - 2], F32)
        split2(lambda g, s: g.tensor_mul(expo[:, s], prod[:, 0:T - 2][:, s], c1_8[:, s]))
        split2(lambda g, s: g.tensor_mul(tA[:, s], sqs[:, s], c2_8[:, s]))
        split2(lambda g, s: g.tensor_sub(expo[:, s], expo[:, s], tA[:, s]))

        # NaN positions: fp32 exp(expo) overflows AND marginal underflowed to 0,
        # in a non-degenerate column.
        nc.vector.tensor_scalar(tA, expo[:], EXP_OVF, None, OP.is_gt)
        nc.vector.scalar_tensor_tensor(tA, x2, E_BIG, tA, OP.is_gt, OP.mult)
        nan_row = wp.tile([BC, 1], F32)
        nc.vector.reduce_max(nan_row[:], tA, axis=AxisListType.X)

        # cop (finite branch) and energy_f1
        nc.vector.tensor_scalar(expo[:], expo[:], CLAMP_HI, CLAMP_LO, OP.min, OP.max)
        nc.scalar.activation(out=expo[:], in_=expo[:], func=AF.Exp)
        ef1 = big.tile([BC, T], F32, tag="shared_a")
        nc.vector.tensor_mul(expo[:], expo[:], a8[:])
        nc.vector.tensor_add(expo[:], expo[:], b8[:])
        nc.vector.tensor_mul(ef1[:, 1:T - 1], marg[:, 1:T - 1], expo[:])
        nc.vector.tensor_copy(out=ef1[:, 0:1], in_=marg[:, 0:1])
        nc.vector.tensor_copy(out=ef1[:, T - 1:T], in_=marg[:, T - 1:T])

        # ---------------- fallback flag: any all-zero energy column ------------
        zc = col.tile([128, TP], F32)
        nc.vector.tensor_scalar(zc[:], gs2[:], 0.0, None, OP.is_le)
        zcr = col.tile([128, 1], F32)
        nc.vector.reduce_max(zcr[:], zc[:], axis=AxisListType.X)
        ps_z = pp.tile([1, 128], F32, tag="ps_small", name="ps_z")
        nc.tensor.transpose(ps_z[0:1, :], zcr[:], ident[:])
        uo1 = wp.tile([1, 1], F32)
        nc.vector.reduce_max(uo1[:], ps_z[0:1, :], axis=AxisListType.X)
        nc.sync.dma_start(out=flg_d[:], in_=uo1[:])
        nc.sync.dma_start(out=scr[5:6, 0:1], in_=uo1[:])
        uo8 = wp.tile([BC, 1], F32)
        nc.gpsimd.dma_start(out=uo8[:], in_=scr[5:6, 0:1].to_broadcast([BC, 1]))

        acs = big.tile([BC, T], F32, tag="shared_c")   # copula softmax
        softmax_rows(acs, ef1, "cop")

        # blend: att_safe = uo * softmax(e) + (1-uo) * copula_score
        uo8inv = wp.tile([BC, 1], F32)
        nc.vector.tensor_scalar(uo8inv[:], uo8[:], -1.0, 1.0, OP.mult, OP.add)
        nc.vector.tensor_scalar(acs[:], acs[:], uo8inv[:], None, OP.mult)
        nc.vector.tensor_scalar(sme[:], sme[:], uo8[:], None, OP.mult)
        split2(lambda g, s: g.tensor_add(acs[:, s], acs[:, s], sme[:, s]), n=T)

        # NaN rows (copula branch only)
        mask2 = wp.tile([BC, 1], F32)
        nc.vector.tensor_mul(mask2[:], nan_row[:], uo8inv[:])
        mask2u = wp.tile([BC, 1], mybir.dt.uint8)
        nc.vector.tensor_copy(out=mask2u[:], in_=mask2[:])
        nan_col = wp.tile([BC, 1], F32)
        nc.vector.select(nan_col[:], mask2u[:], nan1[0:BC, :], zero1[0:BC, :])

        att_out = big.tile([BC, T], F32, tag="shared_b")
        nc.vector.tensor_scalar(att_out[:], acs[:], nan_col[:], None, OP.add)
        nc.sync.dma_start(out=att_d[:], in_=att_out[:])

        # ---------------- context = sum_t att[b,t] * lis[b,t,:] ----------------
        attT = wp.tile([128, NCH, 16], FP8)
        for ch in range(NCH):
            ps_t = pp.tile([128, BC], F32, tag="ps_small", name="ps_tp")
            nc.tensor.transpose(ps_t[:, 0:BC], acs[0:BC, ts(ch, 128)], ident[0:BC, 0:BC])
            nc.scalar.activation(out=attT[:, ch, 0:BC], in_=ps_t[:, 0:BC],
                                 func=AF.Copy, scale=ATT_SCALE)

        ctx_sb = big.tile([BC, D], F32)
        for b in range(BC):
            ps_c = pp.tile([1, D], F32, tag="ps_small", name="ps_ctx")
            for q in range(NCH // 4):
                ln = ln_pool.tile([128, 4, D], FP8)
                nc.sync.dma_start(
                    out=ln[:],
                    in_=lisN_d[b].rearrange("(ch p) d -> p ch d", p=128)[:, 4 * q:4 * q + 4, :],
                )
                for j in range(2):
                    gq = 2 * q + j
                    nc.tensor.matmul(
                        ps_c[:],
                        attT[:, 2 * gq:2 * gq + 2, b:b + 1],
                        ln[:, 2 * j:2 * j + 2, :],
                        start=(gq == 0),
                        stop=(gq == NCH // 2 - 1),
                        perf_mode=mybir.MatmulPerfMode.DoubleRow,
                    )
            c_stage = st_pool.tile([1, D], F32, name="c_stage")
            if b % 2 == 0:
                nc.scalar.activation(out=c_stage[:], in_=ps_c[0:1, :],
                                     func=AF.Copy, scale=1.0 / ATT_SCALE)
            else:
                nc.vector.tensor_scalar(c_stage[:], ps_c[0:1, :], 1.0 / ATT_SCALE,
                                        None, OP.mult)
            nc.sync.dma_start(out=ctx_sb[b:b + 1, :], in_=c_stage[:])

        nc.vector.tensor_scalar(ctx_sb[:], ctx_sb[:], nan_col[:], None, OP.add)
        nc.sync.dma_start(out=ctx_d[:], in_=ctx_sb[:])


_NC_CACHE = {}
LAST_RESULTS = None


def _install_trace_shim():
    """The agent container's antenv stub lacks axon_hooks; register the NTFF
    profile hook ourselves so run_bass_kernel_spmd(trace=True) works."""
    import sys
    import types

    try:
        from antenv.axon_hooks import get_axon_ntff_profile_hook  # noqa: F401
    except ImportError:
        import antenv

        mod = types.ModuleType("antenv.axon_hooks")
        mod._hook = None
        mod.set_axon_ntff_profile_hook = lambda h: setattr(mod, "_hook", h)
        mod.get_axon_ntff_profile_hook = lambda: mod._hook
        sys.modules["antenv.axon_hooks"] = mod
        antenv.axon_hooks = mod
        try:
            from trn_agent_boot.trn_boot import _ntff_profile_via_ctypes
            mod._hook = _ntff_profile_via_ctypes("/opt/axon/libaxon_pjrt.so")
        except Exception:
            pass
    import concourse.bass_utils as bu
    bu.upload_artifacts = lambda tmpdir: tmpdir


def _get_nc():
    if "nc" not in _NC_CACHE:
        _NC_CACHE["nc"] = _build_nc()
    return _NC_CACHE["nc"]


def kernel(decoder_state, listener_feature, phi_w, phi_b, psi_w, psi_b):
    global LAST_RESULTS
    bf16 = ml_dtypes.bfloat16

    dec = np.asarray(decoder_state, np.float32)[:, 0, :]          # [B, D]
    lis_f32 = np.asarray(listener_feature, np.float32)            # [B, T, D]
    lis_f8 = lis_f32.astype(ml_dtypes.float8_e4m3)
    fp8 = ml_dtypes.float8_e4m3
    # DoubleRow interleave: index [g, ki, j, m] = psi_w.T[256g + 128j + ki, m]
    psiT = np.ascontiguousarray(
        np.asarray(psi_w, np.float32).T.reshape(2, 2, 128, M)
        .transpose(0, 2, 1, 3)).astype(fp8)
    phiT = np.ascontiguousarray(np.asarray(phi_w, np.float32).T).astype(bf16)
    psib = np.asarray(psi_b, np.float32)
    phib = np.asarray(phi_b, np.float32)

    in_maps = []
    for c in range(NCORES):
        bs = slice(c * BC, (c + 1) * BC)
        lis_t8 = (lis_f8[bs].transpose(0, 2, 1)           # [BC, D, T]
                  .reshape(BC, 2, 2, 128, T)              # [BC, g, j, ki, T]
                  .transpose(0, 1, 3, 2, 4))              # [BC, g, ki, j, T]
        in_maps.append({
            "lis_t": np.ascontiguousarray(lis_t8),
            "lis_n": np.ascontiguousarray(lis_f8[bs]),
            "dec_t": np.ascontiguousarray(dec[bs].T).astype(bf16),
            "psi_t": psiT,
            "phi_t": phiT,
            "psi_bias": psib,
            "phi_bias": phib,
        })

    trace = bool(os.environ.get("KERNEL_TRACE"))
    if trace:
        _install_trace_shim()
    nc = _get_nc()
    res = run_bass_kernel_spmd(
        nc,
        in_maps,
        core_ids=list(range(NCORES)),
        trace=trace,
    )
    LAST_RESULTS = res

    att = np.concatenate([res.results[c]["att"] for c in range(NCORES)], axis=0)
    ctx = np.concatenate([res.results[c]["ctx"] for c in range(NCORES)], axis=0)
    flag = float(res.results[0]["flg"][0, 0])
    count_original = np.int32(1 if flag > 0.5 else 0)
    count_copula = np.int32(1 - count_original)
    return att, ctx, count_original, count_copula


# revision 61
# speedup vs baseline: 1.6525x; 1.0258x over previous
"""Trainium2 Bass kernel for nn_Attention_37074157699663.

Copula attention: MLP preprocess (phi/psi + ReLU), energy = comp_dec . comp_lis,
adjacent-column Pearson correlation over the batch, bivariate Gaussian copula
pdf, softmax, context. Data-parallel over batch across 8 NeuronCores; the three
per-column stat vectors (sum e, sum e^2, sum e[t]e[t+1]) are AllReduced.

The reference computes norm.pdf(e) (underflows fp32 to 0) times exp(copula
exponent) (overflows fp32 to inf), so reference rows are NaN wherever any
column overflows. We reproduce that exactly by computing the overflow mask
explicitly (expo > ln(float32 max)) instead of trusting device exp() inf
semantics, then injecting NaN per row.
"""

import os

import numpy as np
import ml_dtypes

import concourse.bacc as bacc
import concourse.bass as bass
import concourse.tile as tile
from concourse import mybir
from concourse.bass_utils import run_bass_kernel_spmd
from concourse.masks import make_identity
from bass_rust import AxisListType

BF16 = mybir.dt.bfloat16
FP8 = mybir.dt.float8e4
F32 = mybir.dt.float32
ATT_SCALE = 128.0   # att probs (~5e-4) scaled into fp8e4m3's normal range
AF = mybir.ActivationFunctionType
OP = mybir.AluOpType
ts = bass.ts

B, T, D, M = 64, 2048, 512, 512
NCORES = 8
BC = B // NCORES        # 8 batch rows per core
TC = 512                # t-chunk for MM1/MM2 (one PSUM bank)
NTC = T // TC           # 4
NDT = D // 128          # 4 contraction tiles
NMT = M // 128          # 4 m tiles
NCH = T // 128          # 16 chunks for the context matmul
TP = T // 128           # 16: free size of [128, TP] column-stat tiles

INV_SQRT_2PI = 0.3989422804014327
TWO_PI = 6.283185307179586   # Rsqrt(2*pi*sd) == INV_SQRT_2PI * rsqrt(sd)
EXP_OVF = 88.722839          # ln(float32 max): fp32 exp(x) == inf for x above this
E_BIG = 20.0                 # e > 20  =>  INV_SQRT_2PI * exp(-e^2/2) is exactly 0 in fp32
CLAMP_LO = -103.0
CLAMP_HI = 88.0
BINV = 1.0 / B


def _col_ap(flat_ap):
    """[2048]-flat AP -> [128, 16] (t = p*16 + f)."""
    return flat_ap.rearrange("(p f) -> p f", p=128)


def _build_nc():
    nc = bacc.Bacc("TRN2", target_bir_lowering=False)

    # lis_t / psi_t are pre-interleaved for DoubleRow: d = 256*g + 128*j + ki
    lisT_d = nc.dram_tensor("lis_t", [BC, 2, 128, 2, T], FP8, kind="ExternalInput")
    lisN_d = nc.dram_tensor("lis_n", [BC, T, D], FP8, kind="ExternalInput")
    decT_d = nc.dram_tensor("dec_t", [D, BC], BF16, kind="ExternalInput")
    psiT_d = nc.dram_tensor("psi_t", [2, 128, 2, M], FP8, kind="ExternalInput")
    phiT_d = nc.dram_tensor("phi_t", [D, M], BF16, kind="ExternalInput")
    psib_d = nc.dram_tensor("psi_bias", [M], F32, kind="ExternalInput")
    phib_d = nc.dram_tensor("phi_bias", [M], F32, kind="ExternalInput")

    att_d = nc.dram_tensor("att", [BC, T], F32, kind="ExternalOutput")
    ctx_d = nc.dram_tensor("ctx", [BC, D], F32, kind="ExternalOutput")
    flg_d = nc.dram_tensor("flg", [1, 1], F32, kind="ExternalOutput")

    with tile.TileContext(nc) as tc:
        _body(nc, tc, lisT_d, lisN_d, decT_d, psiT_d, phiT_d, psib_d, phib_d,
              att_d, ctx_d, flg_d)
    nc.compile()
    return nc


def _body(nc, tc, lisT_d, lisN_d, decT_d, psiT_d, phiT_d, psib_d, phib_d,
          att_d, ctx_d, flg_d):
    with (
        tc.tile_pool(name="wp", bufs=1) as wp,
        tc.tile_pool(name="lt_pool", bufs=4) as lt_pool,
        tc.tile_pool(name="ln_pool", bufs=14) as ln_pool,
        tc.tile_pool(name="cl_pool", bufs=2) as cl_pool,
        tc.tile_pool(name="big", bufs=1) as big,
        tc.tile_pool(name="st_pool", bufs=2) as st_pool,
        tc.tile_pool(name="col", bufs=1) as col,
        tc.tile_pool(name="pp", bufs=3, space="PSUM") as pp,
        tc.tile_pool(name="ep", bufs=2, space="PSUM") as ep,
        tc.tile_pool(name="dram", bufs=1, space="DRAM") as dram,
    ):
        # ---------------- constants / weights ----------------
        psiT = wp.tile([128, 2, 2, M], FP8)
        nc.sync.dma_start(out=psiT[:], in_=psiT_d[:].rearrange("g p j m -> p g j m"))
        phiT = wp.tile([128, NDT, M], BF16)
        nc.sync.dma_start(out=phiT[:], in_=phiT_d[:].rearrange("(dt p) m -> p dt m", p=128))
        decT = wp.tile([128, NDT, BC], BF16)
        nc.sync.dma_start(out=decT[:], in_=decT_d[:].rearrange("(dt p) b -> p dt b", p=128))
        psib = wp.tile([128, NMT], F32)
        nc.sync.dma_start(out=psib[:], in_=psib_d[:].rearrange("(mt p) -> p mt", p=128))
        phib = wp.tile([128, NMT], F32)
        nc.sync.dma_start(out=phib[:], in_=phib_d[:].rearrange("(mt p) -> p mt", p=128))

        ident = wp.tile([128, 128], F32)
        make_identity(nc, ident)
        ones8 = wp.tile([BC, 1], BF16)
        nc.vector.memset(ones8, 1.0)
        ones16 = wp.tile([128, TP], F32)
        nc.vector.memset(ones16, 1.0)
        nan1 = wp.tile([128, 1], F32)
        nc.vector.memset(nan1, float("nan"))
        zero1 = wp.tile([128, 1], F32)
        nc.vector.memset(zero1, 0.0)

        # ---------------- comp_dec = relu(dec @ phi_w.T + phi_b) ----------------
        comp_decT = wp.tile([128, NMT, 16], FP8)    # [m-part, mt, b (pad 16)]
        for mt in range(NMT):
            ps_dec = pp.tile([128, BC], F32, tag="ps_small")
            for dt in range(NDT):
                nc.tensor.matmul(
                    ps_dec[:],
                    phiT[:, dt, ts(mt, 128)],
                    decT[:, dt, :],
                    start=(dt == 0),
                    stop=(dt == NDT - 1),
                )
            nc.scalar.activation(out=comp_decT[:, mt, 0:BC], in_=ps_dec[:],
                                 func=AF.Relu, bias=phib[:, mt:mt + 1], scale=1.0)

        # ---------------- main loop: comp_lis + energy ----------------
        PREF = 14
        ln_tiles = {}
        e_sb = big.tile([BC, T], BF16)
        for b in range(BC):
            if b == 2:
                for i in range(PREF):
                    pb, pq = divmod(i, NCH // 4)
                    ln_pre = ln_pool.tile([128, 4, D], FP8, name="ln", tag="ln")
                    nc.sync.dma_start(
                        out=ln_pre[:],
                        in_=lisN_d[pb].rearrange("(ch p) d -> p ch d", p=128)[:, 4 * pq:4 * pq + 4, :],
                    )
                    ln_tiles[i] = ln_pre
            for tci in range(NTC):
                lt = lt_pool.tile([128, 2, 2, TC], FP8)
                for g in range(2):
                    nc.sync.dma_start(
                        out=lt[:, g, :, :],
                        in_=lisT_d[b, g][:, :, ts(tci, TC)],
                    )
                clis = cl_pool.tile([128, NMT, TC], FP8)
                for mt in range(NMT):
                    ps_l = pp.tile([128, TC], F32, tag="ps_mm1")
                    for g in range(2):
                        nc.tensor.matmul(
                            ps_l[:],
                            psiT[:, g, :, ts(mt, 128)],
                            lt[:, g, :, :],
                            start=(g == 0),
                            stop=(g == 1),
                            perf_mode=mybir.MatmulPerfMode.DoubleRow,
                        )
                    if mt % 2 == 0:
                        nc.scalar.activation(out=clis[:, mt, :], in_=ps_l[:],
                                             func=AF.Relu, bias=psib[:, mt:mt + 1], scale=1.0)
                    else:
                        nc.vector.tensor_scalar(clis[:, mt, :], ps_l[:],
                                                psib[:, mt:mt + 1], 0.0, OP.add, OP.max)
                ps_e = ep.tile([1, TC], F32, tag="eps")
                for g2 in range(2):
                    nc.tensor.matmul(
                        ps_e[:],
                        comp_decT[:, 2 * g2:2 * g2 + 2, b:b + 1],
                        clis[:, 2 * g2:2 * g2 + 2, :],
                        start=(g2 == 0),
                        stop=(g2 == 1),
                        perf_mode=mybir.MatmulPerfMode.DoubleRow,
                    )
                # engine accesses must start at a 32-aligned partition: copy
                # the M=1 PSUM row to an SBUF stage at partition 0, then
                # DMA-scatter into row b
                e_stage = st_pool.tile([1, TC], BF16, name="e_stage")
                if (b + tci) % 2 == 0:
                    nc.scalar.copy(out=e_stage[:], in_=ps_e[0:1, :])
                else:
                    nc.vector.tensor_copy(out=e_stage[:], in_=ps_e[0:1, :])
                nc.sync.dma_start(out=e_sb[b:b + 1, ts(tci, TC)], in_=e_stage[:])

        # ---------------- softmax helper ----------------
        def softmax_rows(dst, srcv, tag):
            rmax = wp.tile([BC, 1], F32, name=f"rmax_{tag}")
            nc.vector.reduce_max(rmax[:], srcv[:], axis=AxisListType.X)
            nc.vector.tensor_scalar(rmax[:], rmax[:], -1.0, None, OP.mult)
            nc.vector.tensor_scalar(dst[:], srcv[:], rmax[:], CLAMP_LO, OP.add, OP.max)
            nc.scalar.activation(out=dst[:], in_=dst[:], func=AF.Exp)
            rsum = wp.tile([BC, 1], F32, name=f"rsum_{tag}")
            nc.vector.reduce_sum(rsum[:], dst[:], axis=AxisListType.X)
            nc.vector.reciprocal(rsum[:], rsum[:])
            nc.vector.tensor_scalar(dst[:], dst[:], rsum[:], None, OP.mult)

        # ---------------- batch stats + AllReduce ----------------
        cc_in = dram.tile([3, T], F32)
        cc_out = dram.tile([3, T], F32)
        scr = dram.tile([8, T], F32)   # scratch rows: shifted-ss + coef rows + uo

        sq = big.tile([BC, T], BF16, tag="sqtag")
        nc.vector.tensor_mul(sq[:], e_sb[:], e_sb[:])
        prod = big.tile([BC, T], BF16)
        nc.vector.memset(prod[:, T - 1:T], 0.0)
        nc.vector.tensor_mul(prod[:, 0:T - 1], e_sb[:, 0:T - 1], e_sb[:, 1:T])

        for k, src in enumerate((e_sb, sq, prod)):
            st_row = st_pool.tile([1, T], F32)
            for tci in range(NTC):
                ps_s = ep.tile([1, TC], F32, tag="eps", name="ps_stat")
                nc.tensor.matmul(
                    ps_s[:],
                    ones8[:],
                    src[0:BC, ts(tci, TC)],
                    start=True,
                    stop=True,
                )
                if (k + tci) % 2 == 0:
                    nc.scalar.copy(out=st_row[:, ts(tci, TC)], in_=ps_s[0:1, :])
                else:
                    nc.vector.tensor_copy(out=st_row[:, ts(tci, TC)], in_=ps_s[0:1, :])
            nc.sync.dma_start(out=cc_in[k:k + 1, :], in_=st_row[:])

        nc.gpsimd.collective_compute(
            "AllReduce",
            OP.add,
            replica_groups=[list(range(NCORES))],
            ins=[cc_in.opt()],
            outs=[cc_out.opt()],
        )

        # ---------------- local-only work overlapping the AllReduce ------------
        # marginal = norm.pdf(e) with exact zeros where it underflows in fp32
        marg = big.tile([BC, T], F32, tag="shared_b")
        nc.vector.tensor_scalar(marg[:], sq[:], -0.5, CLAMP_LO, OP.mult, OP.max)
        nc.scalar.activation(out=marg[:], in_=marg[:], func=AF.Exp)
        mz = big.tile([BC, T], F32)
        nc.vector.tensor_scalar(mz[:], e_sb[:], E_BIG, None, OP.is_le)
        nc.vector.tensor_mul(marg[:], marg[:], mz[:])
        sqs = big.tile([BC, T - 2], F32)
        nc.vector.tensor_add(sqs[:], sq[:, 0:T - 2], sq[:, 1:T - 1])   # x1^2 + x2^2
        sme = big.tile([BC, T], F32)
        softmax_rows(sme, e_sb, "eng")   # plain-softmax fallback branch

        # ---------------- column stats in [128, 16] layout ----------------
        cc_flat = cc_out.rearrange("a b -> (a b)")
        gs1 = col.tile([128, TP], F32)
        nc.sync.dma_start(out=gs1[:], in_=_col_ap(cc_flat[0:T]))
        gs1S = col.tile([128, TP], F32)
        nc.sync.dma_start(out=gs1S[:], in_=_col_ap(cc_flat[1:T + 1]))
        gs2 = col.tile([128, TP], F32)
        nc.sync.dma_start(out=gs2[:], in_=_col_ap(cc_flat[T:2 * T]))
        gs12 = col.tile([128, TP], F32)
        nc.sync.dma_start(out=gs12[:], in_=_col_ap(cc_flat[2 * T:3 * T]))

        gs2S = col.tile([128, TP], F32)
        nc.sync.dma_start(out=gs2S[:], in_=_col_ap(cc_flat[T + 1:2 * T + 1]))
        scr_flat = scr.rearrange("a b -> (a b)")
        m2 = col.tile([128, TP], F32)
        nc.vector.scalar_tensor_tensor(m2[:], gs1[:], BINV, gs1[:], OP.mult, OP.mult)
        ss = col.tile([128, TP], F32)
        nc.vector.tensor_sub(ss[:], gs2[:], m2[:])
        ssS = col.tile([128, TP], F32)
        nc.vector.scalar_tensor_tensor(ssS[:], gs1S[:], BINV, gs1S[:], OP.mult, OP.mult)
        nc.vector.tensor_sub(ssS[:], gs2S[:], ssS[:])

        sprod = col.tile([128, TP], F32)
        nc.vector.scalar_tensor_tensor(sprod[:], gs1[:], BINV, gs1S[:], OP.mult, OP.mult)
        num = col.tile([128, TP], F32)
        nc.vector.tensor_sub(num[:], gs12[:], sprod[:])
        d2 = col.tile([128, TP], F32)
        nc.vector.tensor_mul(d2[:], ss[:], ssS[:])
        rsq = col.tile([128, TP], F32)
        nc.scalar.activation(out=rsq[:], in_=d2[:], func=AF.Sqrt)
        nc.vector.reciprocal(rsq[:], rsq[:])
        rv = col.tile([128, TP], F32)
        nc.vector.tensor_mul(rv[:], num[:], rsq[:])
        rr2 = col.tile([128, TP], F32)
        nc.vector.tensor_mul(rr2[:], rv[:], rv[:])
        det = col.tile([128, TP], F32)
        nc.vector.tensor_scalar(det[:], rr2[:], -1.0, 1.0, OP.mult, OP.add)
        degen = col.tile([128, TP], F32)
        nc.vector.tensor_scalar(degen[:], det[:], 0.01, None, OP.is_lt)
        ndeg = col.tile([128, TP], F32)
        nc.vector.tensor_scalar(ndeg[:], degen[:], -1.0, 1.0, OP.mult, OP.add)
        # safe_det = det where det >= 0.01 else 1.0 (masks are exactly 0/1)
        sd = col.tile([128, TP], F32)
        nc.vector.tensor_mul(sd[:], det[:], ndeg[:])
        nc.vector.tensor_add(sd[:], sd[:], degen[:])
        invsd = col.tile([128, TP], F32)
        nc.vector.reciprocal(invsd[:], sd[:])
        c1 = col.tile([128, TP], F32)
        nc.vector.tensor_mul(c1[:], rv[:], invsd[:])
        c2 = col.tile([128, TP], F32)
        nc.vector.scalar_tensor_tensor(c2[:], c1[:], 0.5, rv[:], OP.mult, OP.mult)
        ccoef = col.tile([128, TP], F32)
        nc.scalar.activation(out=ccoef[:], in_=sd[:], func=AF.Sqrt, scale=TWO_PI)
        nc.vector.reciprocal(ccoef[:], ccoef[:])
        # fold the degenerate-column branch into the coefficients:
        # expo uses c1*ndeg, c2*ndeg (degen cols get expo = 0), and
        # energy_f1 = marg * (exp(expo)*A + Bv) with A = ccoef*ndeg, Bv = 10*degen
        nc.vector.tensor_mul(c1[:], c1[:], ndeg[:])
        nc.vector.tensor_mul(c2[:], c2[:], ndeg[:])
        av = col.tile([128, TP], F32)
        nc.vector.tensor_mul(av[:], ccoef[:], ndeg[:])
        bv = col.tile([128, TP], F32)
        nc.vector.tensor_scalar(bv[:], degen[:], 10.0, None, OP.mult)

        # coef rows -> DRAM -> [BC, T-2] broadcasts
        for k, srct in enumerate((c1, c2, av, bv)):
            nc.sync.dma_start(out=_col_ap(scr_flat[(k + 1) * T:(k + 2) * T]), in_=srct[:])
        c1_8 = big.tile([BC, T - 2], F32)
        nc.gpsimd.dma_start(out=c1_8[:], in_=scr[1:2, 0:T - 2].to_broadcast([BC, T - 2]))
        c2_8 = big.tile([BC, T - 2], F32)
        nc.gpsimd.dma_start(out=c2_8[:], in_=scr[2:3, 0:T - 2].to_broadcast([BC, T - 2]))
        a8 = big.tile([BC, T - 2], F32)
        nc.gpsimd.dma_start(out=a8[:], in_=scr[3:4, 0:T - 2].to_broadcast([BC, T - 2]))
        b8 = big.tile([BC, T - 2], F32)
        nc.gpsimd.dma_start(out=b8[:], in_=scr[4:5, 0:T - 2].to_broadcast([BC, T - 2]))

        # ---------------- copula pdf grid [BC, T-2] ----------------
        # att column i in 1..T-2 uses x1 = e[:, i-1], x2 = e[:, i], r = rr[i-1].
        # Column-split every elementwise op: DVE takes 2/3, GpSimd (half DVE's
        # rate) takes 1/3, so both halves finish together.
        NT2 = T - 2
        SPL = 1364

        def split2(fn, n=NT2):
            fn(nc.vector, slice(0, n))

        x2 = e_sb[:, 1:T - 1]
        tA = mz[:, 0:T - 2]                  # mz is free after the marg mask

        expo = big.tile([BC, T - 2], F32)
        split2(lambda g, s: g.tensor_mul(expo[:, s], prod[:, 0:T - 2][:, s], c1_8[:, s]))
        split2(lambda g, s: g.tensor_mul(tA[:, s], sqs[:, s], c2_8[:, s]))
        split2(lambda g, s: g.tensor_sub(expo[:, s], expo[:, s], tA[:, s]))

        # NaN positions: fp32 exp(expo) overflows AND marginal underflowed to 0,
        # in a non-degenerate column.
        nc.vector.tensor_scalar(tA, expo[:], EXP_OVF, None, OP.is_gt)
        nc.vector.scalar_tensor_tensor(tA, x2, E_BIG, tA, OP.is_gt, OP.mult)
        nan_row = wp.tile([BC, 1], F32)
        nc.vector.reduce_max(nan_row[:], tA, axis=AxisListType.X)

        # cop (finite branch) and energy_f1
        nc.vector.tensor_scalar(expo[:], expo[:], CLAMP_HI, CLAMP_LO, OP.min, OP.max)
        nc.scalar.activation(out=expo[:], in_=expo[:], func=AF.Exp)
        ef1 = big.tile([BC, T], F32, tag="shared_a")
        nc.vector.tensor_mul(expo[:], expo[:], a8[:])
        nc.vector.tensor_add(expo[:], expo[:], b8[:])
        nc.vector.tensor_mul(ef1[:, 1:T - 1], marg[:, 1:T - 1], expo[:])
        nc.vector.tensor_copy(out=ef1[:, 0:1], in_=marg[:, 0:1])
        nc.vector.tensor_copy(out=ef1[:, T - 1:T], in_=marg[:, T - 1:T])

        # ---------------- fallback flag: any all-zero energy column ------------
        zc = col.tile([128, TP], F32)
        nc.vector.tensor_scalar(zc[:], gs2[:], 0.0, None, OP.is_le)
        zcr = col.tile([128, 1], F32)
        nc.vector.reduce_max(zcr[:], zc[:], axis=AxisListType.X)
        ps_z = pp.tile([1, 128], F32, tag="ps_small", name="ps_z")
        nc.tensor.transpose(ps_z[0:1, :], zcr[:], ident[:])
        uo1 = wp.tile([1, 1], F32)
        nc.vector.reduce_max(uo1[:], ps_z[0:1, :], axis=AxisListType.X)
        nc.sync.dma_start(out=flg_d[:], in_=uo1[:])
        nc.sync.dma_start(out=scr[5:6, 0:1], in_=uo1[:])
        uo8 = wp.tile([BC, 1], F32)
        nc.gpsimd.dma_start(out=uo8[:], in_=scr[5:6, 0:1].to_broadcast([BC, 1]))

        acs = big.tile([BC, T], F32, tag="shared_c")   # copula softmax
        softmax_rows(acs, ef1, "cop")

        # blend: att_safe = uo * softmax(e) + (1-uo) * copula_score
        uo8inv = wp.tile([BC, 1], F32)
        nc.vector.tensor_scalar(uo8inv[:], uo8[:], -1.0, 1.0, OP.mult, OP.add)
        nc.vector.tensor_scalar(acs[:], acs[:], uo8inv[:], None, OP.mult)
        nc.vector.tensor_scalar(sme[:], sme[:], uo8[:], None, OP.mult)
        split2(lambda g, s: g.tensor_add(acs[:, s], acs[:, s], sme[:, s]), n=T)

        # NaN rows (copula branch only)
        mask2 = wp.tile([BC, 1], F32)
        nc.vector.tensor_mul(mask2[:], nan_row[:], uo8inv[:])
        mask2u = wp.tile([BC, 1], mybir.dt.uint8)
        nc.vector.tensor_copy(out=mask2u[:], in_=mask2[:])
        nan_col = wp.tile([BC, 1], F32)
        nc.vector.select(nan_col[:], mask2u[:], nan1[0:BC, :], zero1[0:BC, :])

        att_out = big.tile([BC, T], F32, tag="shared_b")
        nc.vector.tensor_scalar(att_out[:], acs[:], nan_col[:], None, OP.add)
        nc.sync.dma_start(out=att_d[:], in_=att_out[:])

        # ---------------- context = sum_t att[b,t] * lis[b,t,:] ----------------
        attT = wp.tile([128, NCH, 16], FP8)
        for ch in range(NCH):
            ps_t = pp.tile([128, BC], F32, tag="ps_small", name="ps_tp")
            nc.tensor.transpose(ps_t[:, 0:BC], acs[0:BC, ts(ch, 128)], ident[0:BC, 0:BC])
            nc.scalar.activation(out=attT[:, ch, 0:BC], in_=ps_t[:, 0:BC],
                                 func=AF.Copy, scale=ATT_SCALE)

        ctx_sb = big.tile([BC, D], F32)
        for b in range(BC):
            ps_c = pp.tile([1, D], F32, tag="ps_small", name="ps_ctx")
            for q in range(NCH // 4):
                idx = (NCH // 4) * b + q
                if idx in ln_tiles:
                    ln = ln_tiles[idx]
                else:
                    ln = ln_pool.tile([128, 4, D], FP8, name="ln", tag="ln")
                    nc.sync.dma_start(
                        out=ln[:],
                        in_=lisN_d[b].rearrange("(ch p) d -> p ch d", p=128)[:, 4 * q:4 * q + 4, :],
                    )
                for j in range(2):
                    gq = 2 * q + j
                    nc.tensor.matmul(
                        ps_c[:],
                        attT[:, 2 * gq:2 * gq + 2, b:b + 1],
                        ln[:, 2 * j:2 * j + 2, :],
                        start=(gq == 0),
                        stop=(gq == NCH // 2 - 1),
                        perf_mode=mybir.MatmulPerfMode.DoubleRow,
                    )
            c_stage = st_pool.tile([1, D], F32, name="c_stage")
            if b % 2 == 0:
                nc.scalar.activation(out=c_stage[:], in_=ps_c[0:1, :],
                                     func=AF.Copy, scale=1.0 / ATT_SCALE)
            else:
                nc.vector.tensor_scalar(c_stage[:], ps_c[0:1, :], 1.0 / ATT_SCALE,
                                        None, OP.mult)
            nc.sync.dma_start(out=ctx_sb[b:b + 1, :], in_=c_stage[:])

        nc.vector.tensor_scalar(ctx_sb[:], ctx_sb[:], nan_col[:], None, OP.add)
        nc.sync.dma_start(out=ctx_d[:], in_=ctx_sb[:])


_NC_CACHE = {}
LAST_RESULTS = None


def _install_trace_shim():
    """The agent container's antenv stub lacks axon_hooks; register the NTFF
    profile hook ourselves so run_bass_kernel_spmd(trace=True) works."""
    import sys
    import types

    try:
        from antenv.axon_hooks import get_axon_ntff_profile_hook  # noqa: F401
    except ImportError:
        import antenv

        mod = types.ModuleType("antenv.axon_hooks")
        mod._hook = None
        mod.set_axon_ntff_profile_hook = lambda h: setattr(mod, "_hook", h)
        mod.get_axon_ntff_profile_hook = lambda: mod._hook
        sys.modules["antenv.axon_hooks"] = mod
        antenv.axon_hooks = mod
        try:
            from trn_agent_boot.trn_boot import _ntff_profile_via_ctypes
            mod._hook = _ntff_profile_via_ctypes("/opt/axon/libaxon_pjrt.so")
        except Exception:
            pass
    import concourse.bass_utils as bu
    bu.upload_artifacts = lambda tmpdir: tmpdir


def _get_nc():
    if "nc" not in _NC_CACHE:
        _NC_CACHE["nc"] = _build_nc()
    return _NC_CACHE["nc"]


def kernel(decoder_state, listener_feature, phi_w, phi_b, psi_w, psi_b):
    global LAST_RESULTS
    bf16 = ml_dtypes.bfloat16

    dec = np.asarray(decoder_state, np.float32)[:, 0, :]          # [B, D]
    lis_f32 = np.asarray(listener_feature, np.float32)            # [B, T, D]
    lis_f8 = lis_f32.astype(ml_dtypes.float8_e4m3)
    fp8 = ml_dtypes.float8_e4m3
    # DoubleRow interleave: index [g, ki, j, m] = psi_w.T[256g + 128j + ki, m]
    psiT = np.ascontiguousarray(
        np.asarray(psi_w, np.float32).T.reshape(2, 2, 128, M)
        .transpose(0, 2, 1, 3)).astype(fp8)
    phiT = np.ascontiguousarray(np.asarray(phi_w, np.float32).T).astype(bf16)
    psib = np.asarray(psi_b, np.float32)
    phib = np.asarray(phi_b, np.float32)

    in_maps = []
    for c in range(NCORES):
        bs = slice(c * BC, (c + 1) * BC)
        lis_t8 = (lis_f8[bs].transpose(0, 2, 1)           # [BC, D, T]
                  .reshape(BC, 2, 2, 128, T)              # [BC, g, j, ki, T]
                  .transpose(0, 1, 3, 2, 4))              # [BC, g, ki, j, T]
        in_maps.append({
            "lis_t": np.ascontiguousarray(lis_t8),
            "lis_n": np.ascontiguousarray(lis_f8[bs]),
            "dec_t": np.ascontiguousarray(dec[bs].T).astype(bf16),
            "psi_t": psiT,
            "phi_t": phiT,
            "psi_bias": psib,
            "phi_bias": phib,
        })

    trace = bool(os.environ.get("KERNEL_TRACE"))
    if trace:
        _install_trace_shim()
    nc = _get_nc()
    res = run_bass_kernel_spmd(
        nc,
        in_maps,
        core_ids=list(range(NCORES)),
        trace=trace,
    )
    LAST_RESULTS = res

    att = np.concatenate([res.results[c]["att"] for c in range(NCORES)], axis=0)
    ctx = np.concatenate([res.results[c]["ctx"] for c in range(NCORES)], axis=0)
    flag = float(res.results[0]["flg"][0, 0])
    count_original = np.int32(1 if flag > 0.5 else 0)
    count_copula = np.int32(1 - count_original)
    return att, ctx, count_original, count_copula
